# revision 17
# baseline (speedup 1.0000x reference)
"""Trainium2 Bass kernel for an AttentionBlock (GroupNorm + single-layer MHA + proj residual).

Reference computation (per batch b):
    xn = GroupNorm(x[b])                        # 8 groups over C=256, HW spatial
    qkv = w_qkv @ xn                            # per-pixel 1x1 conv
    per head h (4 heads, d=64):
        scores = q_h^T k_h * d^-0.5             # [HW, HW]
        attn = softmax(scores, axis=keys)
        out_h = v_h @ attn^T                    # [d, HW]
    y = xn + w_proj @ concat(out_h) + b_proj

Sharding: 8 cores = (batch b in {0,1}) x (query quarter s in {0..3}).  Each
core runs GroupNorm, computes k/v for ALL spatial positions and q for its
own quarter, then runs all 4 heads' attention for its own 1024 query
columns.  The head sum of the projection is a local PSUM accumulation, so
there is NO collective at all: each core writes its own [C, 1024] slice of
the output, with the residual fused into the PSUM drain.

Key kernel-level layout choices (v2):
 - x columns are permuted host-side so each core's OWN quarter comes first;
   attention is permutation-invariant over keys, so k/v/score column order
   doesn't matter.  This kills the separate x_own load and lets the
   residual slice come straight out of the x/xn tiles.
 - scores are computed TRANSPOSED (keys j on partitions, queries i on the
   free axis); softmax denominator comes free as a 65th "ones" column of V.
 - softmax skips max-subtraction; scores live in the log2 domain (q
   pre-scaled by d^-0.5*log2 e host-side).
 - each score PAIR (2 key tiles x 512 queries) lands in ONE 2-bank PSUM
   tile [128,2,512]; ONE pair-wide exp instruction (Scalar native EXP or
   Vector int8 bit-trick) converts it to fp8e5 `es`.  e5m2's 4 steps/octave
   means the bit-trick value range is always a safe positive int8.
 - PV runs as a single fp8 DoubleRow matmul per pair (v4 fp8e4 stationary,
   es fp8e5 moving), halving PE time vs two bf16 matmuls and keeping the
   PE dense enough for the HAM clock gate to hold 2.4 GHz.
 - projection accumulates in a score-pool PSUM slot; residual fused in the
   drain.  Prologue: interleaved x-chunk DMAs (both halves round-robin) so
   GroupNorm stats finish right after the load; a couple of discarded f32
   matmuls on late x chunks pre-warm the PE clock.
"""

import numpy as np

C = 256
NH = 4
D = 64
G = 8
EPS = 1e-5
B = 2
NCORES = 8
PDIM = 128  # partitions
VP = 68     # v4 per-(jt,head) stride: 4*68=272 bytes, dual-fp8 ldweights needs %16==0

PREWARM = True
# per-vchunk exp engine pattern (16 pairs): S=scalar native exp, V=vector trick
EXP_PATTERN = "SVSVSVSSVSVSVSSV"


def build_nc(HW: int):
    import concourse.bass as bass
    import concourse.mybir as mybir
    import concourse.tile as tile
    from concourse import bacc

    f32 = mybir.dt.float32
    bf16 = mybir.dt.bfloat16
    fp8e4 = mybir.dt.float8e4
    fp8e5 = mybir.dt.float8e5
    i8 = mybir.dt.int8
    DR = mybir.MatmulPerfMode.DoubleRow
    CW = min(512, HW)          # i-chunk width (matmul moving-operand max)
    NIC = HW // CW             # number of column chunks of the full image
    OWN = HW // 4              # query columns owned per core
    NOC = OWN // CW            # own-column chunks
    NJT = HW // PDIM           # number of key tiles (128 keys each)
    NP = NJT // 2              # pairs of key tiles
    LA = 3                     # pv lookahead in pairs

    nc = bacc.Bacc(
        "TRN2", target_bir_lowering=False, debug=False, num_devices=NCORES
    )

    xb = nc.declare_dram_parameter("xb", [C, HW], f32, isOutput=False)
    wq4 = nc.declare_dram_parameter("wq4", [PDIM, 2, C], fp8e4, isOutput=False)
    wk4 = nc.declare_dram_parameter("wk4", [PDIM, 2, C], fp8e4, isOutput=False)
    wv4 = nc.declare_dram_parameter("wv4", [PDIM, 2, C], fp8e4, isOutput=False)
    wpT = nc.declare_dram_parameter("wpT", [C, C], bf16, isOutput=False)
    gamma = nc.declare_dram_parameter("gamma", [C], f32, isOutput=False)
    beta = nc.declare_dram_parameter("beta", [C], f32, isOutput=False)
    bproj = nc.declare_dram_parameter("bproj", [C], f32, isOutput=False)
    indf = nc.declare_dram_parameter("indf", [2, PDIM, G], f32, isOutput=False)
    indb = nc.declare_dram_parameter("indb", [2, G, PDIM], f32, isOutput=False)
    y = nc.declare_dram_parameter("y", [C, OWN], f32, isOutput=True)

    Exp = mybir.ActivationFunctionType.Exp
    Sqrt = mybir.ActivationFunctionType.Sqrt
    Ident = mybir.ActivationFunctionType.Identity
    MUL = mybir.AluOpType.mult
    ADD = mybir.AluOpType.add

    BNW = min(512, HW)         # bn_stats max free dim
    NBN = HW // BNW
    LN2 = 0.6931471805599453

    with tile.TileContext(nc) as tc:
        with (
            tc.tile_pool(name="consts", bufs=1) as consts,
            tc.tile_pool(name="xpool", bufs=1) as xpool,
            tc.tile_pool(name="xnpool", bufs=1) as xnpool,
            tc.tile_pool(name="gn_sm", bufs=2) as gn_sm,
            tc.tile_pool(name="qkpool", bufs=1) as qkpool,
            tc.tile_pool(name="espool", bufs=6) as espool,
            tc.tile_pool(name="mlsm", bufs=3) as mlsm,
            tc.tile_pool(name="ypool", bufs=4) as ypool,
        ):
            # ---------------- x load (biggest transfer, gates GN) ----------------
            # Interleave the two channel-halves chunk-by-chunk across the three
            # DMA-capable queues so bn_stats for BOTH halves trail the load by
            # only one chunk.
            dma_engines = [nc.sync, nc.scalar, nc.gpsimd]
            x_sb = [
                xpool.tile([PDIM, HW], f32, tag=f"x{t}", name=f"x{t}") for t in range(2)
            ]
            di = 0
            for c in range(NIC):
                for t in range(2):
                    dma_engines[di % 3].dma_start(
                        out=x_sb[t][:, bass.ts(c, CW)],
                        in_=xb[bass.ts(t, PDIM), bass.ts(c, CW)],
                    )
                    di += 1

            # ---------------- constants / small loads ----------------
            eps_t = consts.tile([PDIM, 1], f32)
            nc.vector.memset(eps_t, EPS)
            nln2 = consts.tile([PDIM, 1], f32, tag="nln2")
            nc.vector.memset(nln2, -2.0 * 0.6931471805599453)

            indf_sb = []
            indb_sb = []
            gm_sb = []
            bt_sb = []
            bp_sb = []
            for t in range(2):
                it_ = consts.tile([PDIM, G], f32, tag=f"indf{t}")
                nc.sync.dma_start(out=it_, in_=indf[t])
                indf_sb.append(it_)
                ib_ = consts.tile([G, PDIM], f32, tag=f"indb{t}")
                nc.sync.dma_start(out=ib_, in_=indb[t])
                indb_sb.append(ib_)
                g_ = consts.tile([PDIM, 1], f32, tag=f"gm{t}")
                nc.sync.dma_start(out=g_, in_=gamma[bass.ts(t, PDIM)].rearrange("(p o) -> p o", o=1))
                gm_sb.append(g_)
                b_ = consts.tile([PDIM, 1], f32, tag=f"bt{t}")
                nc.sync.dma_start(out=b_, in_=beta[bass.ts(t, PDIM)].rearrange("(p o) -> p o", o=1))
                bt_sb.append(b_)
                bp_ = consts.tile([PDIM, 1], f32, tag=f"bp{t}")
                nc.sync.dma_start(out=bp_, in_=bproj[bass.ts(t, PDIM)].rearrange("(p o) -> p o", o=1))
                bp_sb.append(bp_)

            # weight tiles: fp8, [p, c-half, 256 outputs] (DR k-subtile layout)
            wq_t = consts.tile([PDIM, 2, C], fp8e4, tag="wq")
            nc.sync.dma_start(out=wq_t, in_=wq4[:, :, :])
            wk_t = consts.tile([PDIM, 2, C], fp8e4, tag="wk")
            nc.sync.dma_start(out=wk_t, in_=wk4[:, :, :])
            wv_t = consts.tile([PDIM, 2, C], fp8e4, tag="wv")
            nc.sync.dma_start(out=wv_t, in_=wv4[:, :, :])
            wp_sb = []
            for h in range(NH):
                wt = consts.tile([D, C], bf16, tag=f"wp{h}", name=f"wp{h}")
                nc.sync.dma_start(out=wt, in_=wpT[h * D : (h + 1) * D, :])
                wp_sb.append(wt)

            from contextlib import ExitStack

            ps_stack = ExitStack()
            gn_ps = ps_stack.enter_context(tc.tile_pool(name="gn_ps", bufs=1, space="PSUM"))

            # ---------------- PE pre-warm (discarded f32 matmuls) ----------------
            # The PE HAM clock gate needs ~3.4us of sustained activity to release
            # 2.4 GHz.  Two slow f32 matmuls on late x chunks put the PE in the
            # busy state right before the GN/QKV/attention stream begins.
            if PREWARM:
                warm = gn_ps.tile([PDIM, 2, CW], f32, tag="warm")
                for w in range(4):
                    nc.tensor.matmul(
                        out=warm[:, w % 2, :],
                        lhsT=x_sb[0][:, (NIC - 4 + w) * CW : (NIC - 4 + w) * CW + PDIM],
                        rhs=x_sb[1][:, bass.ts(NIC - 4 + w, CW)],
                        start=True,
                        stop=True,
                    )

            # ---------------- GroupNorm stats ----------------
            gst_full = gn_ps.tile([PDIM, 2], f32, tag="gnps")
            gst_ps = gst_full[0:G, :]
            for t in range(2):
                stats = gn_sm.tile([PDIM, NBN, 6], f32, tag="bnst")
                for s in range(NBN):
                    nc.vector.bn_stats(out=stats[:, s, :], in_=x_sb[t][:, bass.ts(s, BNW)])
                mv = gn_sm.tile([PDIM, 2], f32, tag="mv")
                nc.vector.bn_aggr(out=mv, in_=stats)
                st2 = gn_sm.tile([PDIM, 2], f32, tag="st2")
                nc.vector.tensor_copy(st2[:, 0:1], mv[:, 0:1])
                sq = gn_sm.tile([PDIM, 1], f32, tag="sq")
                nc.vector.tensor_mul(sq, mv[:, 0:1], mv[:, 0:1])
                nc.vector.tensor_add(st2[:, 1:2], mv[:, 1:2], sq)
                nc.tensor.matmul(
                    out=gst_ps, lhsT=indf_sb[t], rhs=st2, start=(t == 0), stop=(t == 1)
                )

            gst = gn_sm.tile([G, 2], f32, tag="gst_sb")
            nc.vector.tensor_copy(gst, gst_ps)
            mu2 = gn_sm.tile([G, 1], f32, tag="mu2")
            nc.vector.tensor_mul(mu2, gst[:, 0:1], gst[:, 0:1])
            var = gn_sm.tile([G, 1], f32, tag="var")
            nc.vector.tensor_sub(var, gst[:, 1:2], mu2)
            sd = gn_sm.tile([G, 1], f32, tag="sd")
            nc.scalar.activation(out=sd, in_=var, func=Sqrt, bias=eps_t[0:G, :], scale=1.0)
            rstd = gn_sm.tile([G, 1], f32, tag="rstd")
            nc.vector.reciprocal(out=rstd, in_=sd)
            gmr = gn_sm.tile([G, 2], f32, tag="gmr")
            nc.vector.tensor_copy(gmr[:, 0:1], gst[:, 0:1])
            nc.vector.tensor_copy(gmr[:, 1:2], rstd)

            # per-channel affine params + normalized x + residual slice
            xn4 = xnpool.tile([PDIM, 2, HW], fp8e4, tag="xn4")
            resid_sb = []
            for t in range(2):
                gb_ps = gn_ps.tile([PDIM, 2], f32, tag="gnps")
                nc.tensor.matmul(out=gb_ps, lhsT=indb_sb[t], rhs=gmr, start=True, stop=True)
                gb = gn_sm.tile([PDIM, 2], f32, tag="gb_sb")
                nc.vector.tensor_copy(gb, gb_ps)
                A_t = gn_sm.tile([PDIM, 1], f32, tag=f"A{t}")
                nc.vector.tensor_mul(A_t, gb[:, 1:2], gm_sb[t])
                tmp = gn_sm.tile([PDIM, 1], f32, tag="tmp")
                nc.vector.tensor_mul(tmp, gb[:, 0:1], A_t)
                B_t = gn_sm.tile([PDIM, 1], f32, tag=f"B{t}")
                nc.vector.tensor_sub(B_t, bt_sb[t], tmp)
                B2_t = gn_sm.tile([PDIM, 1], f32, tag=f"B2{t}")
                nc.vector.tensor_add(B2_t, B_t, bp_sb[t])

                A16 = gn_sm.tile([PDIM, 1], f32, tag=f"A16{t}")
                nc.vector.tensor_scalar(A16, A_t, 16.0, 0.0, MUL, ADD)
                B16 = gn_sm.tile([PDIM, 1], f32, tag=f"B16{t}")
                nc.vector.tensor_scalar(B16, B_t, 16.0, 0.0, MUL, ADD)
                # 16*xn in fp8e4 (|16 xn| <~ 100 << 240); per-chunk ops split
                # Scalar/DVE so the first qkv matmuls start early
                for cc in range(NIC):
                    if cc % 2 == 0:
                        nc.scalar.activation(
                            out=xn4[:, t, bass.ts(cc, CW)],
                            in_=x_sb[t][:, bass.ts(cc, CW)],
                            func=Ident, bias=B16, scale=A16,
                        )
                    else:
                        nc.vector.tensor_scalar(
                            xn4[:, t, bass.ts(cc, CW)],
                            x_sb[t][:, bass.ts(cc, CW)],
                            A16, B16, MUL, ADD,
                        )
                rs_t = xnpool.tile([PDIM, OWN], f32, tag=f"res{t}")
                nc.gpsimd.tensor_scalar(rs_t, x_sb[t][:, 0:OWN], A_t, B2_t, MUL, ADD)
                resid_sb.append(rs_t)

            ps_stack.close()  # release GN PSUM banks
            ps_stack = ExitStack()
            qk_ps = ps_stack.enter_context(tc.tile_pool(name="qk_ps", bufs=3, space="PSUM"))

            # pre-load the gpsimd partition_broadcast ucode lib while the PE/
            # engines are still in the prologue; the main loop's only gpsimd
            # compute is partition_broadcast, so the lib stays resident.
            warmbc = gn_sm.tile([D, G], f32, tag="warmbc")
            nc.gpsimd.partition_broadcast(warmbc, eps_t[0:1, :].broadcast_to([1, G]))

            # ---------------- k, q, v production ----------------
            # k/q: [256 out-ch = 4 heads x 64, cols]; heads 0,1 in out-half 0.
            # Each PSUM tile holds TWO column chunks -> one big drain each.
            drain_engs = [nc.scalar, nc.vector]
            dei = 0

            def drain(dst, src):
                # qkv ran on 16x-scaled fp8 operands: descale by 1/256
                nonlocal dei
                eng = drain_engs[dei % 2]
                dei += 1
                if eng is nc.scalar:
                    eng.activation(out=dst, in_=src, func=Ident, bias=0.0, scale=1.0 / 256.0)
                else:
                    eng.tensor_scalar(dst, src, 1.0 / 256.0, 0.0, MUL, ADD)

            ku = [qkpool.tile([PDIM, HW], bf16, tag=f"ku{co}", name=f"ku{co}") for co in range(2)]
            qu = [qkpool.tile([PDIM, OWN], bf16, tag=f"qu{co}", name=f"qu{co}") for co in range(2)]
            kx = [qkpool.tile([PDIM, HW], bf16, tag=f"kx{cp}", name=f"kx{cp}") for cp in range(2)]
            qx = [qkpool.tile([PDIM, OWN], bf16, tag=f"qx{cp}", name=f"qx{cp}") for cp in range(2)]
            v4 = qkpool.tile([PDIM, NJT, NH, VP], fp8e4, tag="v4")
            nc.vector.memset(v4[:, :, :, D : D + 1], 1.0)
            xdi = [0]

            def emit_k(co, c2, pool, tag):
                # one k2 tile = two column chunks; drain + the swapped-half
                # companion DMAs for those chunks (heads need k in BOTH halves)
                ps = pool.tile([PDIM, 2, CW], f32, tag=tag, name=f"k{co}_{c2}")
                for s in range(2):
                    nc.tensor.matmul(
                        out=ps[:, s, :],
                        lhsT=wk_t[:, :, bass.ts(co, PDIM)],
                        rhs=xn4[:, :, bass.ts(2 * c2 + s, CW)],
                        start=True,
                        stop=True,
                        perf_mode=DR,
                    )
                drain(ku[co][:, bass.ts(c2, 2 * CW)], ps.rearrange("p s w -> p (s w)"))
                for c in (2 * c2, 2 * c2 + 1):
                    eng = dma_engines[xdi[0] % 3]
                    xdi[0] += 1
                    eng.dma_start(out=kx[co][0:D, bass.ts(c, CW)], in_=ku[co][D : 2 * D, bass.ts(c, CW)])
                    eng.dma_start(out=kx[co][D : 2 * D, bass.ts(c, CW)], in_=ku[co][0:D, bass.ts(c, CW)])

            def emit_q(co, pool, tag):
                ps = pool.tile([PDIM, 2, CW], f32, tag=tag, name=f"q{co}")
                for s in range(NOC):
                    nc.tensor.matmul(
                        out=ps[:, s, :],
                        lhsT=wq_t[:, :, bass.ts(co, PDIM)],
                        rhs=xn4[:, :, bass.ts(s, CW)],
                        start=True,
                        stop=True,
                        perf_mode=DR,
                    )
                drain(qu[co], ps.rearrange("p s w -> p (s w)"))
                nc.scalar.dma_start(out=qx[co][0:D, :], in_=qu[co][D : 2 * D, :])
                nc.scalar.dma_start(out=qx[co][D : 2 * D, :], in_=qu[co][0:D, :])

            def emit_v(p, pool, tag, w=C, bufs=None):
                # v for key-tile pair p, all 4 heads, strided into v4 slots
                kw = {"bufs": bufs} if bufs else {}
                ps = pool.tile([PDIM, 2, w], f32, tag=tag, name=f"v{p}", **kw)
                for s in range(2):
                    for t in range(2):
                        nc.tensor.matmul(
                            out=ps[:, s, 0:C],
                            lhsT=xn4[:, t, bass.ts(2 * p + s, PDIM)],
                            rhs=wv_t[:, t, :],
                            start=(t == 0),
                            stop=(t == 1),
                        )
                drain(
                    v4[:, 2 * p : 2 * p + 2, :, 0:D],
                    ps[:, :, 0:C].rearrange("p s (h d) -> p s h d", h=NH),
                )

            # Upfront: only what vchunk (cc0, heads 0/1) needs -- k/q half co=0
            # and the first 6 v pairs.  k/q co=1 and v pairs 6..15 are DEFERRED
            # into the attention stream (psum via the score-pool ring), so
            # attention starts ~20us earlier and the drains overlap exp work.
            NUPV = 6
            for c2 in range(NIC // 2):
                emit_k(0, c2, qk_ps, "k2")
            emit_q(0, qk_ps, "k2")
            for p in range(NUPV):
                emit_v(p, qk_ps, "v2", bufs=2)

            def k_src(h, s):
                # head h's k at partition half s
                return (ku if (h % 2) == s else kx)[h // 2]

            def q_src(h, s):
                return (qu if (h % 2) == s else qx)[h // 2]

            # ---------------- main attention loop ----------------
            ps_stack.close()  # release GN/QKV PSUM banks
            ps_stack2 = ExitStack()
            sc_ps = ps_stack2.enter_context(tc.tile_pool(name="sc_ps", bufs=3, space="PSUM"))
            pv_ps_pool = ps_stack2.enter_context(tc.tile_pool(name="pv_ps", bufs=2, space="PSUM"))

            # Per (i-chunk, head) "vchunk": 16 score-pair/exp/PV-DR steps,
            # pipelined LA pairs deep.  Each vchunk's normalization chain
            # (recip -> broadcast -> onorm) is DEFERRED into the next vchunk's
            # pair loop; the projection (4-head PSUM accumulation in a
            # score-pool slot + fused residual) emits once its chunk's 4
            # onorms exist.
            onorms_by_cc = [[] for _ in range(NOC)]

            def emit_proj(cc):
                cslice = bass.ts(cc, CW)
                for co in range(2):
                    pj = sc_ps.tile([PDIM, 2, CW], f32, tag="sc", name=f"pj{co}")
                    for h in range(NH):
                        nc.tensor.matmul(
                            out=pj[:, 0, :],
                            lhsT=wp_sb[h][:, bass.ts(co, PDIM)],
                            rhs=onorms_by_cc[cc][h],
                            start=(h == 0),
                            stop=(h == NH - 1),
                        )
                    yf = ypool.tile([PDIM, CW], f32, tag="yf", name="yf")
                    nc.vector.tensor_add(yf, pj[:, 0, :], resid_sb[co][:, cslice])
                    nc.sync.dma_start(out=y[bass.ts(co, PDIM), cslice], in_=yf)

            def make_chain(cc, pv):
                state = {}

                def stage1():
                    den = mlsm.tile([1, CW], f32, tag="den", name="den")
                    nc.scalar.copy(den, pv[D : D + 1, :])
                    rden = mlsm.tile([1, CW], f32, tag="rden", name="rden")
                    nc.vector.reciprocal_approx_fast(out=rden, in_=den)
                    rdb = mlsm.tile([D, CW], f32, tag="rdb", name="rdb", bufs=2)
                    nc.gpsimd.partition_broadcast(rdb, rden[:, :])
                    state["rdb"] = rdb

                def stage2():
                    onorm = mlsm.tile([D, CW], bf16, tag="onorm", bufs=5, name="onorm")
                    nc.vector.tensor_mul(onorm, state["rdb"], pv[0:D, :])
                    onorms_by_cc[cc].append(onorm)
                    if len(onorms_by_cc[cc]) == NH:
                        emit_proj(cc)
                return stage1, stage2

            # One flat pair-stream over (chunk, head): the pend queue carries
            # ACROSS vchunk boundaries, so the final PV of one head interleaves
            # with the next head's first score/exp pairs and the exp engines
            # never drain at a boundary.  post_q holds the deferred norm-chain
            # stages, drained one per pair-step so they fill pipeline slack.
            # XQ holds the deferred qkv production (v pairs 6.., then k/q
            # half co=1, needed only from vchunk (cc0,h2) = step 32).
            XQ = [lambda p=p: emit_v(p, sc_ps, "sc", w=CW) for p in range(NUPV, NP)]
            XQ += [lambda c2=c2: emit_k(1, c2, sc_ps, "sc") for c2 in range(NIC // 2)]
            XQ += [lambda: emit_q(1, sc_ps, "sc")]
            pend = []
            post_q = []
            pv_cur = None
            stream = [(cc, h, p) for cc in range(NOC) for h in range(NH) for p in range(NP)]
            for idx, (cc, h, p) in enumerate(stream + [(None, None, q) for q in range(LA)]):
                tail = cc is None
                if not tail:
                    if XQ:
                        XQ.pop(0)()
                    if p == 0:
                        pv_cur = (pv_ps_pool.tile([D + 1, CW], f32, tag="pv", name="pv"), cc, h)
                    cslice = bass.ts(cc, CW)
                    # the pair's two K=64 score matmuls are row-packed into
                    # disjoint PE row-groups (base_partition 0/64) and run
                    # concurrently in one PE pass, writing the two banks of
                    # ONE PSUM tile; a single pair-wide exp drains both.
                    sc = sc_ps.tile([PDIM, 2, CW], f32, tag="sc", name="sc")
                    for s in range(2):
                        jt = 2 * p + s
                        nc.tensor.matmul(
                            out=sc[:, s, :],
                            lhsT=k_src(h, s)[s * D : (s + 1) * D, bass.ts(jt, PDIM)],
                            rhs=q_src(h, s)[s * D : (s + 1) * D, cslice],
                            start=True,
                            stop=True,
                        )
                    es = espool.tile([PDIM, 2, CW], fp8e5, tag="es")
                    if EXP_PATTERN[p % len(EXP_PATTERN)] == "S":
                        # q pre-scaled by d^-0.5*log2(e) host-side: 2^t = exp(ln2*t)
                        nc.scalar.activation(out=es, in_=sc, func=Exp, scale=LN2)
                    else:
                        # 2^t as fp8e5 bits: int8(4t + 60.5); t in [-8.4, 8.4] always
                        # maps to [27, 94] -- never negative/NaN codes.  (The e4m3
                        # variant is UNSAFE: int8 in [-128,-1] hits fp8e4 NaN codes.)
                        nc.vector.tensor_scalar(es.bitcast(i8), sc, 4.0, 60.5, MUL, ADD)
                    pend.append((pv_cur, p, es))
                while len(pend) > (0 if tail and p == LA - 1 else LA) or (tail and len(pend) > LA - 1 - p):
                    (pvt, pcc, ph), p0, es0 = pend.pop(0)
                    nc.tensor.matmul(
                        out=pvt,
                        lhsT=v4[:, 2 * p0 : 2 * p0 + 2, ph, 0 : D + 1],
                        rhs=es0,
                        start=(p0 == 0),
                        stop=(p0 == NP - 1),
                        perf_mode=DR,
                    )
                    if p0 == NP - 1:
                        st1, st2 = make_chain(pcc, pvt)
                        post_q.append(st1)
                        post_q.append(st2)
                if post_q:
                    post_q.pop(0)()
            while post_q:
                post_q.pop(0)()

            ps_stack2.close()

    nc.compile()
    return nc


def make_in_maps(x, gn_gamma, gn_beta, w_qkv, w_proj, b_proj, HW):
    """Per-core input dicts. Core c = (b = c//4, quarter s = c%4).
    x columns are rotated so the core's own quarter comes first."""
    import ml_dtypes

    bf16 = ml_dtypes.bfloat16
    OWN = HW // 4
    log2e = np.log2(np.e)
    x2 = np.ascontiguousarray(x.reshape(B, C, HW).astype(np.float32))
    w_qkv = np.asarray(w_qkv, dtype=np.float32)
    w_proj = np.asarray(w_proj, dtype=np.float32)
    indf = np.zeros((2, PDIM, G), dtype=np.float32)
    indb = np.zeros((2, G, PDIM), dtype=np.float32)
    gsz = C // G  # 32 channels per group
    for t in range(2):
        for p in range(PDIM):
            g = (t * PDIM + p) // gsz
            indf[t, p, g] = 1.0 / gsz
            indb[t, g, p] = 1.0
    fp8 = ml_dtypes.float8_e4m3

    def w4(wslice, scale):
        # [C_in, C_out] -> [128, 2, C_out] fp8, x16 (qkv runs on 16x operands)
        wT = wslice.T * scale
        return np.ascontiguousarray(wT.reshape(2, PDIM, C).transpose(1, 0, 2)).astype(fp8)

    wq4 = w4(w_qkv[0:C, :], 16.0 * (D ** -0.5 * log2e))
    wk4 = w4(w_qkv[C : 2 * C, :], 16.0)
    wv4 = w4(w_qkv[2 * C : 3 * C, :], 16.0)
    wpT = np.ascontiguousarray(w_proj.T).astype(bf16)
    in_maps = []
    for c in range(NCORES):
        b, s = c // 4, c % 4
        xrot = np.roll(x2[b], -s * OWN, axis=1)
        in_maps.append(
            {
                "xb": np.ascontiguousarray(xrot),
                "wq4": wq4,
                "wk4": wk4,
                "wv4": wv4,
                "wpT": wpT,
                "gamma": np.asarray(gn_gamma, dtype=np.float32),
                "beta": np.asarray(gn_beta, dtype=np.float32),
                "bproj": np.asarray(b_proj, dtype=np.float32),
                "indf": indf,
                "indb": indb,
            }
        )
    return in_maps


def assemble_output(results, HW, Himg, Wimg):
    OWN = HW // 4
    y = np.empty((B, C, HW), dtype=np.float32)
    for c in range(NCORES):
        b, s = c // 4, c % 4
        y[b][:, s * OWN : (s + 1) * OWN] = results[c]["y"]
    return y.reshape(B, C, Himg, Wimg)


_NC_CACHE = {}


def kernel(x, gn_gamma, gn_beta, w_qkv, w_proj, b_proj):
    from concourse.bass_utils import run_bass_kernel_spmd

    Himg, Wimg = x.shape[2], x.shape[3]
    HW = Himg * Wimg
    if HW not in _NC_CACHE:
        _NC_CACHE[HW] = build_nc(HW)
    nc = _NC_CACHE[HW]
    in_maps = make_in_maps(x, gn_gamma, gn_beta, w_qkv, w_proj, b_proj, HW)
    res = run_bass_kernel_spmd(nc, in_maps, list(range(NCORES)))
    return assemble_output(res.results, HW, Himg, Wimg)


# revision 18
# speedup vs baseline: 1.2174x; 1.2174x over previous
"""Trainium2 Bass kernel for an AttentionBlock (GroupNorm + single-layer MHA + proj residual).

Reference computation (per batch b):
    xn = GroupNorm(x[b])                        # 8 groups over C=256, HW spatial
    qkv = w_qkv @ xn                            # per-pixel 1x1 conv
    per head h (4 heads, d=64):
        scores = q_h^T k_h * d^-0.5             # [HW, HW]
        attn = softmax(scores, axis=keys)
        out_h = v_h @ attn^T                    # [d, HW]
    y = xn + w_proj @ concat(out_h) + b_proj

Sharding: 8 cores = (batch b in {0,1}) x (query quarter s in {0..3}).  Each
core runs GroupNorm, computes k/v for ALL spatial positions and q for its
own quarter, then runs all 4 heads' attention for its own 1024 query
columns.  The head sum of the projection is a local PSUM accumulation, so
there is NO collective at all: each core writes its own [C, 1024] slice of
the output, with the residual fused into the PSUM drain.

Key kernel-level layout choices (v2):
 - x columns are permuted host-side so each core's OWN quarter comes first;
   attention is permutation-invariant over keys, so k/v/score column order
   doesn't matter.  This kills the separate x_own load and lets the
   residual slice come straight out of the x/xn tiles.
 - scores are computed TRANSPOSED (keys j on partitions, queries i on the
   free axis); softmax denominator comes free as a 65th "ones" column of V.
 - softmax skips max-subtraction; scores live in the log2 domain (q
   pre-scaled by d^-0.5*log2 e host-side).
 - each score PAIR (2 key tiles x 512 queries) lands in ONE 2-bank PSUM
   tile [128,2,512]; ONE pair-wide exp instruction (Scalar native EXP or
   Vector int8 bit-trick) converts it to fp8e5 `es`.  e5m2's 4 steps/octave
   means the bit-trick value range is always a safe positive int8.
 - PV runs as a single fp8 DoubleRow matmul per pair (v4 fp8e4 stationary,
   es fp8e5 moving), halving PE time vs two bf16 matmuls and keeping the
   PE dense enough for the HAM clock gate to hold 2.4 GHz.
 - projection accumulates in a score-pool PSUM slot; residual fused in the
   drain.  Prologue: interleaved x-chunk DMAs (both halves round-robin) so
   GroupNorm stats finish right after the load; a couple of discarded f32
   matmuls on late x chunks pre-warm the PE clock.
"""

import numpy as np

C = 256
NH = 4
D = 64
G = 8
EPS = 1e-5
B = 2
NCORES = 8
PDIM = 128  # partitions
VP = 68     # v4 per-(jt,head) stride: 4*68=272 bytes, dual-fp8 ldweights needs %16==0

PREWARM = True
# per-vchunk exp engine pattern (16 pairs): S=scalar native exp, V=vector trick
EXP_PATTERN = "SVSVSVSSVSVSVSSV"


def build_nc(HW: int):
    import concourse.bass as bass
    import concourse.mybir as mybir
    import concourse.tile as tile
    from concourse import bacc

    f32 = mybir.dt.float32
    bf16 = mybir.dt.bfloat16
    fp8e4 = mybir.dt.float8e4
    fp8e5 = mybir.dt.float8e5
    i8 = mybir.dt.int8
    DR = mybir.MatmulPerfMode.DoubleRow
    CW = min(512, HW)          # i-chunk width (matmul moving-operand max)
    NIC = HW // CW             # number of column chunks of the full image
    OWN = HW // 4              # query columns owned per core
    NOC = OWN // CW            # own-column chunks
    NJT = HW // PDIM           # number of key tiles (128 keys each)
    NP = NJT // 2              # pairs of key tiles
    LA = 3                     # pv lookahead in pairs

    nc = bacc.Bacc(
        "TRN2", target_bir_lowering=False, debug=False, num_devices=NCORES
    )

    xb = nc.declare_dram_parameter("xb", [C, HW], f32, isOutput=False)
    wq4 = nc.declare_dram_parameter("wq4", [PDIM, 2, C], fp8e4, isOutput=False)
    wk4 = nc.declare_dram_parameter("wk4", [PDIM, 2, C], fp8e4, isOutput=False)
    wv4 = nc.declare_dram_parameter("wv4", [PDIM, 2, C], fp8e4, isOutput=False)
    wpT = nc.declare_dram_parameter("wpT", [C, C], bf16, isOutput=False)
    gamma = nc.declare_dram_parameter("gamma", [C], f32, isOutput=False)
    beta = nc.declare_dram_parameter("beta", [C], f32, isOutput=False)
    bproj = nc.declare_dram_parameter("bproj", [C], f32, isOutput=False)
    indf = nc.declare_dram_parameter("indf", [2, PDIM, G], f32, isOutput=False)
    indb = nc.declare_dram_parameter("indb", [2, G, PDIM], f32, isOutput=False)
    y = nc.declare_dram_parameter("y", [C, OWN], f32, isOutput=True)

    Exp = mybir.ActivationFunctionType.Exp
    Sqrt = mybir.ActivationFunctionType.Sqrt
    Ident = mybir.ActivationFunctionType.Identity
    MUL = mybir.AluOpType.mult
    ADD = mybir.AluOpType.add

    BNW = min(512, HW)         # bn_stats max free dim
    NBN = HW // BNW
    LN2 = 0.6931471805599453

    with tile.TileContext(nc) as tc:
        with (
            tc.tile_pool(name="consts", bufs=1) as consts,
            tc.tile_pool(name="xpool", bufs=1) as xpool,
            tc.tile_pool(name="xnpool", bufs=1) as xnpool,
            tc.tile_pool(name="gn_sm", bufs=2) as gn_sm,
            tc.tile_pool(name="qkpool", bufs=1) as qkpool,
            tc.tile_pool(name="espool", bufs=6) as espool,
            tc.tile_pool(name="mlsm", bufs=3) as mlsm,
            tc.tile_pool(name="ypool", bufs=4) as ypool,
        ):
            # ---------------- x load (biggest transfer, gates GN) ----------------
            # Interleave the two channel-halves chunk-by-chunk across the three
            # DMA-capable queues so bn_stats for BOTH halves trail the load by
            # only one chunk.
            dma_engines = [nc.sync, nc.scalar, nc.gpsimd]
            x_sb = [
                xpool.tile([PDIM, HW], f32, tag=f"x{t}", name=f"x{t}") for t in range(2)
            ]
            di = 0
            for c in range(NIC):
                for t in range(2):
                    dma_engines[di % 3].dma_start(
                        out=x_sb[t][:, bass.ts(c, CW)],
                        in_=xb[bass.ts(t, PDIM), bass.ts(c, CW)],
                    )
                    di += 1

            # ---------------- constants / small loads ----------------
            eps_t = consts.tile([PDIM, 1], f32)
            nc.vector.memset(eps_t, EPS)
            nln2 = consts.tile([PDIM, 1], f32, tag="nln2")
            nc.vector.memset(nln2, -2.0 * 0.6931471805599453)

            indf_sb = []
            indb_sb = []
            gm_sb = []
            bt_sb = []
            bp_sb = []
            for t in range(2):
                it_ = consts.tile([PDIM, G], f32, tag=f"indf{t}")
                nc.sync.dma_start(out=it_, in_=indf[t])
                indf_sb.append(it_)
                ib_ = consts.tile([G, PDIM], f32, tag=f"indb{t}")
                nc.sync.dma_start(out=ib_, in_=indb[t])
                indb_sb.append(ib_)
                g_ = consts.tile([PDIM, 1], f32, tag=f"gm{t}")
                nc.sync.dma_start(out=g_, in_=gamma[bass.ts(t, PDIM)].rearrange("(p o) -> p o", o=1))
                gm_sb.append(g_)
                b_ = consts.tile([PDIM, 1], f32, tag=f"bt{t}")
                nc.sync.dma_start(out=b_, in_=beta[bass.ts(t, PDIM)].rearrange("(p o) -> p o", o=1))
                bt_sb.append(b_)
                bp_ = consts.tile([PDIM, 1], f32, tag=f"bp{t}")
                nc.sync.dma_start(out=bp_, in_=bproj[bass.ts(t, PDIM)].rearrange("(p o) -> p o", o=1))
                bp_sb.append(bp_)

            # weight tiles: fp8, [p, c-half, 256 outputs] (DR k-subtile layout)
            wq_t = consts.tile([PDIM, 2, C], fp8e4, tag="wq")
            nc.sync.dma_start(out=wq_t, in_=wq4[:, :, :])
            wk_t = consts.tile([PDIM, 2, C], fp8e4, tag="wk")
            nc.sync.dma_start(out=wk_t, in_=wk4[:, :, :])
            wv_t = consts.tile([PDIM, 2, C], fp8e4, tag="wv")
            nc.sync.dma_start(out=wv_t, in_=wv4[:, :, :])
            wp_sb = []
            for h in range(NH):
                wt = consts.tile([D, C], bf16, tag=f"wp{h}", name=f"wp{h}")
                nc.sync.dma_start(out=wt, in_=wpT[h * D : (h + 1) * D, :])
                wp_sb.append(wt)

            from contextlib import ExitStack

            ps_stack = ExitStack()
            gn_ps = ps_stack.enter_context(tc.tile_pool(name="gn_ps", bufs=1, space="PSUM"))

            # ---------------- PE pre-warm (discarded f32 matmuls) ----------------
            # The PE HAM clock gate needs ~3.4us of sustained activity to release
            # 2.4 GHz.  Two slow f32 matmuls on late x chunks put the PE in the
            # busy state right before the GN/QKV/attention stream begins.
            if PREWARM:
                warm = gn_ps.tile([PDIM, 2, CW], f32, tag="warm")
                for w in range(4):
                    nc.tensor.matmul(
                        out=warm[:, w % 2, :],
                        lhsT=x_sb[0][:, (NIC - 4 + w) * CW : (NIC - 4 + w) * CW + PDIM],
                        rhs=x_sb[1][:, bass.ts(NIC - 4 + w, CW)],
                        start=True,
                        stop=True,
                    )

            # ---------------- GroupNorm stats ----------------
            gst_full = gn_ps.tile([PDIM, 2], f32, tag="gnps")
            gst_ps = gst_full[0:G, :]
            for t in range(2):
                stats = gn_sm.tile([PDIM, NBN, 6], f32, tag="bnst")
                for s in range(NBN):
                    nc.vector.bn_stats(out=stats[:, s, :], in_=x_sb[t][:, bass.ts(s, BNW)])
                mv = gn_sm.tile([PDIM, 2], f32, tag="mv")
                nc.vector.bn_aggr(out=mv, in_=stats)
                st2 = gn_sm.tile([PDIM, 2], f32, tag="st2")
                nc.vector.tensor_copy(st2[:, 0:1], mv[:, 0:1])
                sq = gn_sm.tile([PDIM, 1], f32, tag="sq")
                nc.vector.tensor_mul(sq, mv[:, 0:1], mv[:, 0:1])
                nc.vector.tensor_add(st2[:, 1:2], mv[:, 1:2], sq)
                nc.tensor.matmul(
                    out=gst_ps, lhsT=indf_sb[t], rhs=st2, start=(t == 0), stop=(t == 1)
                )

            gst = gn_sm.tile([G, 2], f32, tag="gst_sb")
            nc.vector.tensor_copy(gst, gst_ps)
            mu2 = gn_sm.tile([G, 1], f32, tag="mu2")
            nc.vector.tensor_mul(mu2, gst[:, 0:1], gst[:, 0:1])
            var = gn_sm.tile([G, 1], f32, tag="var")
            nc.vector.tensor_sub(var, gst[:, 1:2], mu2)
            sd = gn_sm.tile([G, 1], f32, tag="sd")
            nc.scalar.activation(out=sd, in_=var, func=Sqrt, bias=eps_t[0:G, :], scale=1.0)
            rstd = gn_sm.tile([G, 1], f32, tag="rstd")
            nc.vector.reciprocal(out=rstd, in_=sd)
            gmr = gn_sm.tile([G, 2], f32, tag="gmr")
            nc.vector.tensor_copy(gmr[:, 0:1], gst[:, 0:1])
            nc.vector.tensor_copy(gmr[:, 1:2], rstd)

            # per-channel affine params + normalized x + residual slice
            xn4 = xnpool.tile([PDIM, 2, HW], fp8e4, tag="xn4")
            resid_sb = []
            for t in range(2):
                gb_ps = gn_ps.tile([PDIM, 2], f32, tag="gnps")
                nc.tensor.matmul(out=gb_ps, lhsT=indb_sb[t], rhs=gmr, start=True, stop=True)
                gb = gn_sm.tile([PDIM, 2], f32, tag="gb_sb")
                nc.vector.tensor_copy(gb, gb_ps)
                A_t = gn_sm.tile([PDIM, 1], f32, tag=f"A{t}")
                nc.vector.tensor_mul(A_t, gb[:, 1:2], gm_sb[t])
                tmp = gn_sm.tile([PDIM, 1], f32, tag="tmp")
                nc.vector.tensor_mul(tmp, gb[:, 0:1], A_t)
                B_t = gn_sm.tile([PDIM, 1], f32, tag=f"B{t}")
                nc.vector.tensor_sub(B_t, bt_sb[t], tmp)
                B2_t = gn_sm.tile([PDIM, 1], f32, tag=f"B2{t}")
                nc.vector.tensor_add(B2_t, B_t, bp_sb[t])

                A16 = gn_sm.tile([PDIM, 1], f32, tag=f"A16{t}")
                nc.vector.tensor_scalar(A16, A_t, 16.0, 0.0, MUL, ADD)
                B16 = gn_sm.tile([PDIM, 1], f32, tag=f"B16{t}")
                nc.vector.tensor_scalar(B16, B_t, 16.0, 0.0, MUL, ADD)
                # 16*xn in fp8e4 (|16 xn| <~ 100 << 240); per-chunk ops split
                # Scalar/DVE so the first qkv matmuls start early
                for cc in range(NIC):
                    if cc % 2 == 0:
                        nc.scalar.activation(
                            out=xn4[:, t, bass.ts(cc, CW)],
                            in_=x_sb[t][:, bass.ts(cc, CW)],
                            func=Ident, bias=B16, scale=A16,
                        )
                    else:
                        nc.vector.tensor_scalar(
                            xn4[:, t, bass.ts(cc, CW)],
                            x_sb[t][:, bass.ts(cc, CW)],
                            A16, B16, MUL, ADD,
                        )
                rs_t = xnpool.tile([PDIM, OWN], f32, tag=f"res{t}")
                nc.gpsimd.tensor_scalar(rs_t, x_sb[t][:, 0:OWN], A_t, B2_t, MUL, ADD)
                resid_sb.append(rs_t)

            ps_stack.close()  # release GN PSUM banks
            ps_stack = ExitStack()
            qk_ps = ps_stack.enter_context(tc.tile_pool(name="qk_ps", bufs=3, space="PSUM"))

            # pre-load the gpsimd partition_broadcast ucode lib while the PE/
            # engines are still in the prologue; the main loop's only gpsimd
            # compute is partition_broadcast, so the lib stays resident.
            warmbc = gn_sm.tile([D, G], f32, tag="warmbc")
            nc.gpsimd.partition_broadcast(warmbc, eps_t[0:1, :].broadcast_to([1, G]))

            # ---------------- k, q, v production ----------------
            # k/q: [256 out-ch = 4 heads x 64, cols]; heads 0,1 in out-half 0.
            # Each PSUM tile holds TWO column chunks -> one big drain each.
            drain_engs = [nc.scalar, nc.vector]
            dei = 0

            def drain(dst, src):
                # qkv ran on 16x-scaled fp8 operands: descale by 1/256
                nonlocal dei
                eng = drain_engs[dei % 2]
                dei += 1
                if eng is nc.scalar:
                    eng.activation(out=dst, in_=src, func=Ident, bias=0.0, scale=1.0 / 256.0)
                else:
                    eng.tensor_scalar(dst, src, 1.0 / 256.0, 0.0, MUL, ADD)

            ku = [qkpool.tile([PDIM, HW], bf16, tag=f"ku{co}", name=f"ku{co}") for co in range(2)]
            qu = [qkpool.tile([PDIM, OWN], bf16, tag=f"qu{co}", name=f"qu{co}") for co in range(2)]
            kx = [qkpool.tile([PDIM, HW], bf16, tag=f"kx{cp}", name=f"kx{cp}") for cp in range(2)]
            qx = [qkpool.tile([PDIM, OWN], bf16, tag=f"qx{cp}", name=f"qx{cp}") for cp in range(2)]
            v4 = qkpool.tile([PDIM, NJT, NH, VP], fp8e4, tag="v4")
            nc.vector.memset(v4[:, :, :, D : D + 1], 1.0)
            xdi = [0]

            def emit_k(co, c2, pool, tag):
                # one k2 tile = two column chunks; drain + the swapped-half
                # companion DMAs for those chunks (heads need k in BOTH halves)
                ps = pool.tile([PDIM, 2, CW], f32, tag=tag, name=f"k{co}_{c2}")
                for s in range(2):
                    nc.tensor.matmul(
                        out=ps[:, s, :],
                        lhsT=wk_t[:, :, bass.ts(co, PDIM)],
                        rhs=xn4[:, :, bass.ts(2 * c2 + s, CW)],
                        start=True,
                        stop=True,
                        perf_mode=DR,
                    )
                drain(ku[co][:, bass.ts(c2, 2 * CW)], ps.rearrange("p s w -> p (s w)"))
                for c in (2 * c2, 2 * c2 + 1):
                    eng = dma_engines[xdi[0] % 3]
                    xdi[0] += 1
                    eng.dma_start(out=kx[co][0:D, bass.ts(c, CW)], in_=ku[co][D : 2 * D, bass.ts(c, CW)])
                    eng.dma_start(out=kx[co][D : 2 * D, bass.ts(c, CW)], in_=ku[co][0:D, bass.ts(c, CW)])

            def emit_q(co, pool, tag):
                ps = pool.tile([PDIM, 2, CW], f32, tag=tag, name=f"q{co}")
                for s in range(NOC):
                    nc.tensor.matmul(
                        out=ps[:, s, :],
                        lhsT=wq_t[:, :, bass.ts(co, PDIM)],
                        rhs=xn4[:, :, bass.ts(s, CW)],
                        start=True,
                        stop=True,
                        perf_mode=DR,
                    )
                drain(qu[co], ps.rearrange("p s w -> p (s w)"))
                nc.scalar.dma_start(out=qx[co][0:D, :], in_=qu[co][D : 2 * D, :])
                nc.scalar.dma_start(out=qx[co][D : 2 * D, :], in_=qu[co][0:D, :])

            def emit_v(p, pool, tag, w=C, bufs=None):
                # v for key-tile pair p, all 4 heads, strided into v4 slots
                kw = {"bufs": bufs} if bufs else {}
                ps = pool.tile([PDIM, 2, w], f32, tag=tag, name=f"v{p}", **kw)
                for s in range(2):
                    for t in range(2):
                        nc.tensor.matmul(
                            out=ps[:, s, 0:C],
                            lhsT=xn4[:, t, bass.ts(2 * p + s, PDIM)],
                            rhs=wv_t[:, t, :],
                            start=(t == 0),
                            stop=(t == 1),
                        )
                drain(
                    v4[:, 2 * p : 2 * p + 2, :, 0:D],
                    ps[:, :, 0:C].rearrange("p s (h d) -> p s h d", h=NH),
                )

            # Upfront: only what vchunk (cc0, heads 0/1) needs -- k/q half co=0
            # and the first 6 v pairs.  k/q co=1 and v pairs 6..15 are DEFERRED
            # into the attention stream (psum via the score-pool ring), so
            # attention starts ~20us earlier and the drains overlap exp work.
            NUPV = NP  # all v upfront; only k/q co=1 deferred
            for c2 in range(NIC // 2):
                emit_k(0, c2, qk_ps, "k2")
            emit_q(0, qk_ps, "k2")
            for p in range(NUPV):
                emit_v(p, qk_ps, "v2", bufs=2)

            def k_src(h, s):
                # head h's k at partition half s
                return (ku if (h % 2) == s else kx)[h // 2]

            def q_src(h, s):
                return (qu if (h % 2) == s else qx)[h // 2]

            # ---------------- main attention loop ----------------
            ps_stack.close()  # release GN/QKV PSUM banks
            ps_stack2 = ExitStack()
            sc_ps = ps_stack2.enter_context(tc.tile_pool(name="sc_ps", bufs=3, space="PSUM"))
            pv_ps_pool = ps_stack2.enter_context(tc.tile_pool(name="pv_ps", bufs=2, space="PSUM"))

            # Per (i-chunk, head) "vchunk": 16 score-pair/exp/PV-DR steps,
            # pipelined LA pairs deep.  Each vchunk's normalization chain
            # (recip -> broadcast -> onorm) is DEFERRED into the next vchunk's
            # pair loop; the projection (4-head PSUM accumulation in a
            # score-pool slot + fused residual) emits once its chunk's 4
            # onorms exist.
            onorms_by_cc = [[] for _ in range(NOC)]

            def emit_proj(cc):
                cslice = bass.ts(cc, CW)
                for co in range(2):
                    pj = sc_ps.tile([PDIM, 2, CW], f32, tag="sc", name=f"pj{co}")
                    for h in range(NH):
                        nc.tensor.matmul(
                            out=pj[:, 0, :],
                            lhsT=wp_sb[h][:, bass.ts(co, PDIM)],
                            rhs=onorms_by_cc[cc][h],
                            start=(h == 0),
                            stop=(h == NH - 1),
                        )
                    yf = ypool.tile([PDIM, CW], f32, tag="yf", name="yf")
                    nc.vector.tensor_add(yf, pj[:, 0, :], resid_sb[co][:, cslice])
                    nc.sync.dma_start(out=y[bass.ts(co, PDIM), cslice], in_=yf)

            def make_chain(cc, pv):
                state = {}

                def stage1():
                    den = mlsm.tile([1, CW], f32, tag="den", name="den")
                    nc.scalar.copy(den, pv[D : D + 1, :])
                    rden = mlsm.tile([1, CW], f32, tag="rden", name="rden")
                    nc.vector.reciprocal_approx_fast(out=rden, in_=den)
                    rdb = mlsm.tile([D, CW], f32, tag="rdb", name="rdb", bufs=2)
                    nc.gpsimd.partition_broadcast(rdb, rden[:, :])
                    state["rdb"] = rdb

                def stage2():
                    onorm = mlsm.tile([D, CW], bf16, tag="onorm", bufs=5, name="onorm")
                    nc.vector.tensor_mul(onorm, state["rdb"], pv[0:D, :])
                    onorms_by_cc[cc].append(onorm)
                    if len(onorms_by_cc[cc]) == NH:
                        emit_proj(cc)
                return stage1, stage2

            # One flat pair-stream over (chunk, head): the pend queue carries
            # ACROSS vchunk boundaries, so the final PV of one head interleaves
            # with the next head's first score/exp pairs and the exp engines
            # never drain at a boundary.  post_q holds the deferred norm-chain
            # stages, drained one per pair-step so they fill pipeline slack.
            # XQ holds the deferred qkv production (v pairs 6.., then k/q
            # half co=1, needed only from vchunk (cc0,h2) = step 32).
            XQ = [lambda c2=c2: emit_k(1, c2, sc_ps, "sc") for c2 in range(NIC // 2)]
            XQ += [lambda: emit_q(1, sc_ps, "sc")]
            pend = []
            post_q = []
            pv_cur = None
            stream = [(cc, h, p) for cc in range(NOC) for h in range(NH) for p in range(NP)]
            for idx, (cc, h, p) in enumerate(stream + [(None, None, q) for q in range(LA)]):
                tail = cc is None
                if not tail:
                    if XQ and idx % 2 == 0:
                        XQ.pop(0)()
                    if p == 0:
                        pv_cur = (pv_ps_pool.tile([D + 1, CW], f32, tag="pv", name="pv"), cc, h)
                    cslice = bass.ts(cc, CW)
                    # the pair's two K=64 score matmuls are row-packed into
                    # disjoint PE row-groups (base_partition 0/64) and run
                    # concurrently in one PE pass, writing the two banks of
                    # ONE PSUM tile; a single pair-wide exp drains both.
                    sc = sc_ps.tile([PDIM, 2, CW], f32, tag="sc", name="sc")
                    for s in range(2):
                        jt = 2 * p + s
                        nc.tensor.matmul(
                            out=sc[:, s, :],
                            lhsT=k_src(h, s)[s * D : (s + 1) * D, bass.ts(jt, PDIM)],
                            rhs=q_src(h, s)[s * D : (s + 1) * D, cslice],
                            start=True,
                            stop=True,
                        )
                    es = espool.tile([PDIM, 2, CW], fp8e5, tag="es")
                    if EXP_PATTERN[p % len(EXP_PATTERN)] == "S":
                        # q pre-scaled by d^-0.5*log2(e) host-side: 2^t = exp(ln2*t)
                        nc.scalar.activation(out=es, in_=sc, func=Exp, scale=LN2)
                    else:
                        # 2^t as fp8e5 bits: int8(4t + 60.5); t in [-8.4, 8.4] always
                        # maps to [27, 94] -- never negative/NaN codes.  (The e4m3
                        # variant is UNSAFE: int8 in [-128,-1] hits fp8e4 NaN codes.)
                        nc.vector.tensor_scalar(es.bitcast(i8), sc, 4.0, 60.5, MUL, ADD)
                    pend.append((pv_cur, p, es))
                while len(pend) > (0 if tail and p == LA - 1 else LA) or (tail and len(pend) > LA - 1 - p):
                    (pvt, pcc, ph), p0, es0 = pend.pop(0)
                    nc.tensor.matmul(
                        out=pvt,
                        lhsT=v4[:, 2 * p0 : 2 * p0 + 2, ph, 0 : D + 1],
                        rhs=es0,
                        start=(p0 == 0),
                        stop=(p0 == NP - 1),
                        perf_mode=DR,
                    )
                    if p0 == NP - 1:
                        st1, st2 = make_chain(pcc, pvt)
                        post_q.append(st1)
                        post_q.append(st2)
                if post_q:
                    post_q.pop(0)()
            while post_q:
                post_q.pop(0)()

            ps_stack2.close()

    nc.compile()
    return nc


def make_in_maps(x, gn_gamma, gn_beta, w_qkv, w_proj, b_proj, HW):
    """Per-core input dicts. Core c = (b = c//4, quarter s = c%4).
    x columns are rotated so the core's own quarter comes first."""
    import ml_dtypes

    bf16 = ml_dtypes.bfloat16
    OWN = HW // 4
    log2e = np.log2(np.e)
    x2 = np.ascontiguousarray(x.reshape(B, C, HW).astype(np.float32))
    w_qkv = np.asarray(w_qkv, dtype=np.float32)
    w_proj = np.asarray(w_proj, dtype=np.float32)
    indf = np.zeros((2, PDIM, G), dtype=np.float32)
    indb = np.zeros((2, G, PDIM), dtype=np.float32)
    gsz = C // G  # 32 channels per group
    for t in range(2):
        for p in range(PDIM):
            g = (t * PDIM + p) // gsz
            indf[t, p, g] = 1.0 / gsz
            indb[t, g, p] = 1.0
    fp8 = ml_dtypes.float8_e4m3

    def w4(wslice, scale):
        # [C_in, C_out] -> [128, 2, C_out] fp8, x16 (qkv runs on 16x operands)
        wT = wslice.T * scale
        return np.ascontiguousarray(wT.reshape(2, PDIM, C).transpose(1, 0, 2)).astype(fp8)

    wq4 = w4(w_qkv[0:C, :], 16.0 * (D ** -0.5 * log2e))
    wk4 = w4(w_qkv[C : 2 * C, :], 16.0)
    wv4 = w4(w_qkv[2 * C : 3 * C, :], 16.0)
    wpT = np.ascontiguousarray(w_proj.T).astype(bf16)
    in_maps = []
    for c in range(NCORES):
        b, s = c // 4, c % 4
        xrot = np.roll(x2[b], -s * OWN, axis=1)
        in_maps.append(
            {
                "xb": np.ascontiguousarray(xrot),
                "wq4": wq4,
                "wk4": wk4,
                "wv4": wv4,
                "wpT": wpT,
                "gamma": np.asarray(gn_gamma, dtype=np.float32),
                "beta": np.asarray(gn_beta, dtype=np.float32),
                "bproj": np.asarray(b_proj, dtype=np.float32),
                "indf": indf,
                "indb": indb,
            }
        )
    return in_maps


def assemble_output(results, HW, Himg, Wimg):
    OWN = HW // 4
    y = np.empty((B, C, HW), dtype=np.float32)
    for c in range(NCORES):
        b, s = c // 4, c % 4
        y[b][:, s * OWN : (s + 1) * OWN] = results[c]["y"]
    return y.reshape(B, C, Himg, Wimg)


_NC_CACHE = {}


def kernel(x, gn_gamma, gn_beta, w_qkv, w_proj, b_proj):
    from concourse.bass_utils import run_bass_kernel_spmd

    Himg, Wimg = x.shape[2], x.shape[3]
    HW = Himg * Wimg
    if HW not in _NC_CACHE:
        _NC_CACHE[HW] = build_nc(HW)
    nc = _NC_CACHE[HW]
    in_maps = make_in_maps(x, gn_gamma, gn_beta, w_qkv, w_proj, b_proj, HW)
    res = run_bass_kernel_spmd(nc, in_maps, list(range(NCORES)))
    return assemble_output(res.results, HW, Himg, Wimg)


# revision 19
# speedup vs baseline: 1.2297x; 1.0101x over previous
"""Trainium2 Bass kernel for an AttentionBlock (GroupNorm + single-layer MHA + proj residual).

Reference computation (per batch b):
    xn = GroupNorm(x[b])                        # 8 groups over C=256, HW spatial
    qkv = w_qkv @ xn                            # per-pixel 1x1 conv
    per head h (4 heads, d=64):
        scores = q_h^T k_h * d^-0.5             # [HW, HW]
        attn = softmax(scores, axis=keys)
        out_h = v_h @ attn^T                    # [d, HW]
    y = xn + w_proj @ concat(out_h) + b_proj

Sharding: 8 cores = (batch b in {0,1}) x (query quarter s in {0..3}).  Each
core runs GroupNorm, computes k/v for ALL spatial positions and q for its
own quarter, then runs all 4 heads' attention for its own 1024 query
columns.  The head sum of the projection is a local PSUM accumulation, so
there is NO collective at all: each core writes its own [C, 1024] slice of
the output, with the residual fused into the PSUM drain.

Key kernel-level layout choices (v2):
 - x columns are permuted host-side so each core's OWN quarter comes first;
   attention is permutation-invariant over keys, so k/v/score column order
   doesn't matter.  This kills the separate x_own load and lets the
   residual slice come straight out of the x/xn tiles.
 - scores are computed TRANSPOSED (keys j on partitions, queries i on the
   free axis); softmax denominator comes free as a 65th "ones" column of V.
 - softmax skips max-subtraction; scores live in the log2 domain (q
   pre-scaled by d^-0.5*log2 e host-side).
 - each score PAIR (2 key tiles x 512 queries) lands in ONE 2-bank PSUM
   tile [128,2,512]; ONE pair-wide exp instruction (Scalar native EXP or
   Vector int8 bit-trick) converts it to fp8e5 `es`.  e5m2's 4 steps/octave
   means the bit-trick value range is always a safe positive int8.
 - PV runs as a single fp8 DoubleRow matmul per pair (v4 fp8e4 stationary,
   es fp8e5 moving), halving PE time vs two bf16 matmuls and keeping the
   PE dense enough for the HAM clock gate to hold 2.4 GHz.
 - projection accumulates in a score-pool PSUM slot; residual fused in the
   drain.  Prologue: interleaved x-chunk DMAs (both halves round-robin) so
   GroupNorm stats finish right after the load; a couple of discarded f32
   matmuls on late x chunks pre-warm the PE clock.
"""

import numpy as np

C = 256
NH = 4
D = 64
G = 8
EPS = 1e-5
B = 2
NCORES = 8
PDIM = 128  # partitions
VP = 68     # v4 per-(jt,head) stride: 4*68=272 bytes, dual-fp8 ldweights needs %16==0

PREWARM = True
# per-vchunk exp engine pattern (16 pairs): S=scalar native exp, V=vector trick
EXP_PATTERN = "SVSVSVSSVSVSVSSV"


def build_nc(HW: int):
    import concourse.bass as bass
    import concourse.mybir as mybir
    import concourse.tile as tile
    from concourse import bacc

    f32 = mybir.dt.float32
    bf16 = mybir.dt.bfloat16
    fp8e4 = mybir.dt.float8e4
    fp8e5 = mybir.dt.float8e5
    i8 = mybir.dt.int8
    DR = mybir.MatmulPerfMode.DoubleRow
    CW = min(512, HW)          # i-chunk width (matmul moving-operand max)
    NIC = HW // CW             # number of column chunks of the full image
    OWN = HW // 4              # query columns owned per core
    NOC = OWN // CW            # own-column chunks
    NJT = HW // PDIM           # number of key tiles (128 keys each)
    NP = NJT // 2              # pairs of key tiles
    LA = 3                     # pv lookahead in pairs

    nc = bacc.Bacc(
        "TRN2", target_bir_lowering=False, debug=False, num_devices=NCORES
    )

    xb = nc.declare_dram_parameter("xb", [C, HW], f32, isOutput=False)
    wq4 = nc.declare_dram_parameter("wq4", [PDIM, 2, C], fp8e4, isOutput=False)
    wk4 = nc.declare_dram_parameter("wk4", [PDIM, 2, C], fp8e4, isOutput=False)
    wv4 = nc.declare_dram_parameter("wv4", [PDIM, 2, C], fp8e4, isOutput=False)
    wpT = nc.declare_dram_parameter("wpT", [C, C], bf16, isOutput=False)
    gamma = nc.declare_dram_parameter("gamma", [C], f32, isOutput=False)
    beta = nc.declare_dram_parameter("beta", [C], f32, isOutput=False)
    bproj = nc.declare_dram_parameter("bproj", [C], f32, isOutput=False)
    indf = nc.declare_dram_parameter("indf", [2, PDIM, G], f32, isOutput=False)
    indb = nc.declare_dram_parameter("indb", [2, G, PDIM], f32, isOutput=False)
    y = nc.declare_dram_parameter("y", [C, OWN], f32, isOutput=True)

    Exp = mybir.ActivationFunctionType.Exp
    Sqrt = mybir.ActivationFunctionType.Sqrt
    Ident = mybir.ActivationFunctionType.Identity
    MUL = mybir.AluOpType.mult
    ADD = mybir.AluOpType.add

    BNW = min(512, HW)         # bn_stats max free dim
    NBN = HW // BNW
    LN2 = 0.6931471805599453

    with tile.TileContext(nc) as tc:
        with (
            tc.tile_pool(name="consts", bufs=1) as consts,
            tc.tile_pool(name="xpool", bufs=1) as xpool,
            tc.tile_pool(name="xnpool", bufs=1) as xnpool,
            tc.tile_pool(name="gn_sm", bufs=2) as gn_sm,
            tc.tile_pool(name="qkpool", bufs=1) as qkpool,
            tc.tile_pool(name="espool", bufs=6) as espool,
            tc.tile_pool(name="mlsm", bufs=3) as mlsm,
            tc.tile_pool(name="ypool", bufs=4) as ypool,
        ):
            # ---------------- x load (biggest transfer, gates GN) ----------------
            # Interleave the two channel-halves chunk-by-chunk across the three
            # DMA-capable queues so bn_stats for BOTH halves trail the load by
            # only one chunk.
            dma_engines = [nc.sync, nc.scalar, nc.gpsimd]
            x_sb = [
                xpool.tile([PDIM, HW], f32, tag=f"x{t}", name=f"x{t}") for t in range(2)
            ]
            di = 0
            for c in range(NIC):
                for t in range(2):
                    dma_engines[di % 3].dma_start(
                        out=x_sb[t][:, bass.ts(c, CW)],
                        in_=xb[bass.ts(t, PDIM), bass.ts(c, CW)],
                    )
                    di += 1

            # ---------------- constants / small loads ----------------
            eps_t = consts.tile([PDIM, 1], f32)
            nc.vector.memset(eps_t, EPS)
            nln2 = consts.tile([PDIM, 1], f32, tag="nln2")
            nc.vector.memset(nln2, -2.0 * 0.6931471805599453)

            indf_sb = []
            indb_sb = []
            gm_sb = []
            bt_sb = []
            bp_sb = []
            for t in range(2):
                it_ = consts.tile([PDIM, G], f32, tag=f"indf{t}")
                nc.sync.dma_start(out=it_, in_=indf[t])
                indf_sb.append(it_)
                ib_ = consts.tile([G, PDIM], f32, tag=f"indb{t}")
                nc.sync.dma_start(out=ib_, in_=indb[t])
                indb_sb.append(ib_)
                g_ = consts.tile([PDIM, 1], f32, tag=f"gm{t}")
                nc.sync.dma_start(out=g_, in_=gamma[bass.ts(t, PDIM)].rearrange("(p o) -> p o", o=1))
                gm_sb.append(g_)
                b_ = consts.tile([PDIM, 1], f32, tag=f"bt{t}")
                nc.sync.dma_start(out=b_, in_=beta[bass.ts(t, PDIM)].rearrange("(p o) -> p o", o=1))
                bt_sb.append(b_)
                bp_ = consts.tile([PDIM, 1], f32, tag=f"bp{t}")
                nc.sync.dma_start(out=bp_, in_=bproj[bass.ts(t, PDIM)].rearrange("(p o) -> p o", o=1))
                bp_sb.append(bp_)

            # weight tiles: fp8, [p, c-half, 256 outputs] (DR k-subtile layout)
            wq_t = consts.tile([PDIM, 2, C], fp8e4, tag="wq")
            nc.sync.dma_start(out=wq_t, in_=wq4[:, :, :])
            wk_t = consts.tile([PDIM, 2, C], fp8e4, tag="wk")
            nc.sync.dma_start(out=wk_t, in_=wk4[:, :, :])
            wv_t = consts.tile([PDIM, 2, C], fp8e4, tag="wv")
            nc.sync.dma_start(out=wv_t, in_=wv4[:, :, :])
            wp_sb = []
            for h in range(NH):
                wt = consts.tile([D, C], bf16, tag=f"wp{h}", name=f"wp{h}")
                nc.sync.dma_start(out=wt, in_=wpT[h * D : (h + 1) * D, :])
                wp_sb.append(wt)

            from contextlib import ExitStack

            ps_stack = ExitStack()
            gn_ps = ps_stack.enter_context(tc.tile_pool(name="gn_ps", bufs=1, space="PSUM"))

            # ---------------- PE pre-warm (discarded f32 matmuls) ----------------
            # The PE HAM clock gate needs ~3.4us of sustained activity to release
            # 2.4 GHz.  Two slow f32 matmuls on late x chunks put the PE in the
            # busy state right before the GN/QKV/attention stream begins.
            if PREWARM:
                warm = gn_ps.tile([PDIM, 2, CW], f32, tag="warm")
                for w in range(4):
                    nc.tensor.matmul(
                        out=warm[:, w % 2, :],
                        lhsT=x_sb[0][:, (NIC - 4 + w) * CW : (NIC - 4 + w) * CW + PDIM],
                        rhs=x_sb[1][:, bass.ts(NIC - 4 + w, CW)],
                        start=True,
                        stop=True,
                    )

            # ---------------- GroupNorm stats ----------------
            gst_full = gn_ps.tile([PDIM, 2], f32, tag="gnps")
            gst_ps = gst_full[0:G, :]
            for t in range(2):
                stats = gn_sm.tile([PDIM, NBN, 6], f32, tag="bnst")
                for s in range(NBN):
                    nc.vector.bn_stats(out=stats[:, s, :], in_=x_sb[t][:, bass.ts(s, BNW)])
                mv = gn_sm.tile([PDIM, 2], f32, tag="mv")
                nc.vector.bn_aggr(out=mv, in_=stats)
                st2 = gn_sm.tile([PDIM, 2], f32, tag="st2")
                nc.vector.tensor_copy(st2[:, 0:1], mv[:, 0:1])
                sq = gn_sm.tile([PDIM, 1], f32, tag="sq")
                nc.vector.tensor_mul(sq, mv[:, 0:1], mv[:, 0:1])
                nc.vector.tensor_add(st2[:, 1:2], mv[:, 1:2], sq)
                nc.tensor.matmul(
                    out=gst_ps, lhsT=indf_sb[t], rhs=st2, start=(t == 0), stop=(t == 1)
                )

            gst = gn_sm.tile([G, 2], f32, tag="gst_sb")
            nc.vector.tensor_copy(gst, gst_ps)
            mu2 = gn_sm.tile([G, 1], f32, tag="mu2")
            nc.vector.tensor_mul(mu2, gst[:, 0:1], gst[:, 0:1])
            var = gn_sm.tile([G, 1], f32, tag="var")
            nc.vector.tensor_sub(var, gst[:, 1:2], mu2)
            sd = gn_sm.tile([G, 1], f32, tag="sd")
            nc.scalar.activation(out=sd, in_=var, func=Sqrt, bias=eps_t[0:G, :], scale=1.0)
            rstd = gn_sm.tile([G, 1], f32, tag="rstd")
            nc.vector.reciprocal(out=rstd, in_=sd)
            gmr = gn_sm.tile([G, 2], f32, tag="gmr")
            nc.vector.tensor_copy(gmr[:, 0:1], gst[:, 0:1])
            nc.vector.tensor_copy(gmr[:, 1:2], rstd)

            # per-channel affine params + normalized x + residual slice
            xn4 = xnpool.tile([PDIM, 2, HW], fp8e4, tag="xn4")
            resid_sb = []
            AB16 = []
            for t in range(2):
                gb_ps = gn_ps.tile([PDIM, 2], f32, tag="gnps")
                nc.tensor.matmul(out=gb_ps, lhsT=indb_sb[t], rhs=gmr, start=True, stop=True)
                gb = gn_sm.tile([PDIM, 2], f32, tag="gb_sb")
                nc.vector.tensor_copy(gb, gb_ps)
                A_t = gn_sm.tile([PDIM, 1], f32, tag=f"A{t}")
                nc.vector.tensor_mul(A_t, gb[:, 1:2], gm_sb[t])
                tmp = gn_sm.tile([PDIM, 1], f32, tag="tmp")
                nc.vector.tensor_mul(tmp, gb[:, 0:1], A_t)
                B_t = gn_sm.tile([PDIM, 1], f32, tag=f"B{t}")
                nc.vector.tensor_sub(B_t, bt_sb[t], tmp)
                B2_t = gn_sm.tile([PDIM, 1], f32, tag=f"B2{t}")
                nc.vector.tensor_add(B2_t, B_t, bp_sb[t])

                A16 = gn_sm.tile([PDIM, 1], f32, tag=f"A16{t}")
                nc.vector.tensor_scalar(A16, A_t, 16.0, 0.0, MUL, ADD)
                B16 = gn_sm.tile([PDIM, 1], f32, tag=f"B16{t}")
                nc.vector.tensor_scalar(B16, B_t, 16.0, 0.0, MUL, ADD)
                AB16.append((A16, B16))
                rs_t = xnpool.tile([PDIM, OWN], f32, tag=f"res{t}")
                nc.gpsimd.tensor_scalar(rs_t, x_sb[t][:, 0:OWN], A_t, B2_t, MUL, ADD)
                resid_sb.append(rs_t)

            ps_stack.close()  # release GN PSUM banks
            ps_stack = ExitStack()
            qk_ps = ps_stack.enter_context(tc.tile_pool(name="qk_ps", bufs=3, space="PSUM"))

            # pre-load the gpsimd partition_broadcast ucode lib while the PE/
            # engines are still in the prologue; the main loop's only gpsimd
            # compute is partition_broadcast, so the lib stays resident.
            warmbc = gn_sm.tile([D, G], f32, tag="warmbc")
            nc.gpsimd.partition_broadcast(warmbc, eps_t[0:1, :].broadcast_to([1, G]))

            # ---------------- k, q, v production ----------------
            # k/q: [256 out-ch = 4 heads x 64, cols]; heads 0,1 in out-half 0.
            # Each PSUM tile holds TWO column chunks -> one big drain each.
            drain_engs = [nc.scalar, nc.vector]
            dei = 0

            def drain(dst, src):
                # qkv ran on 16x-scaled fp8 operands: descale by 1/256
                nonlocal dei
                eng = drain_engs[dei % 2]
                dei += 1
                if eng is nc.scalar:
                    eng.activation(out=dst, in_=src, func=Ident, bias=0.0, scale=1.0 / 256.0)
                else:
                    eng.tensor_scalar(dst, src, 1.0 / 256.0, 0.0, MUL, ADD)

            ku = [qkpool.tile([PDIM, HW], bf16, tag=f"ku{co}", name=f"ku{co}") for co in range(2)]
            qu = [qkpool.tile([PDIM, OWN], bf16, tag=f"qu{co}", name=f"qu{co}") for co in range(2)]
            kx = [qkpool.tile([PDIM, HW], bf16, tag=f"kx{cp}", name=f"kx{cp}") for cp in range(2)]
            qx = [qkpool.tile([PDIM, OWN], bf16, tag=f"qx{cp}", name=f"qx{cp}") for cp in range(2)]
            v4 = qkpool.tile([PDIM, NJT, NH, VP], fp8e4, tag="v4")
            nc.vector.memset(v4[:, :, :, D : D + 1], 1.0)
            xdi = [0]

            def emit_k(co, c2, pool, tag):
                # one k2 tile = two column chunks; drain + the swapped-half
                # companion DMAs for those chunks (heads need k in BOTH halves)
                ps = pool.tile([PDIM, 2, CW], f32, tag=tag, name=f"k{co}_{c2}")
                for s in range(2):
                    nc.tensor.matmul(
                        out=ps[:, s, :],
                        lhsT=wk_t[:, :, bass.ts(co, PDIM)],
                        rhs=xn4[:, :, bass.ts(2 * c2 + s, CW)],
                        start=True,
                        stop=True,
                        perf_mode=DR,
                    )
                drain(ku[co][:, bass.ts(c2, 2 * CW)], ps.rearrange("p s w -> p (s w)"))
                for c in (2 * c2, 2 * c2 + 1):
                    eng = dma_engines[xdi[0] % 3]
                    xdi[0] += 1
                    eng.dma_start(out=kx[co][0:D, bass.ts(c, CW)], in_=ku[co][D : 2 * D, bass.ts(c, CW)])
                    eng.dma_start(out=kx[co][D : 2 * D, bass.ts(c, CW)], in_=ku[co][0:D, bass.ts(c, CW)])

            def emit_q(co, pool, tag):
                ps = pool.tile([PDIM, 2, CW], f32, tag=tag, name=f"q{co}")
                for s in range(NOC):
                    nc.tensor.matmul(
                        out=ps[:, s, :],
                        lhsT=wq_t[:, :, bass.ts(co, PDIM)],
                        rhs=xn4[:, :, bass.ts(s, CW)],
                        start=True,
                        stop=True,
                        perf_mode=DR,
                    )
                drain(qu[co], ps.rearrange("p s w -> p (s w)"))
                nc.scalar.dma_start(out=qx[co][0:D, :], in_=qu[co][D : 2 * D, :])
                nc.scalar.dma_start(out=qx[co][D : 2 * D, :], in_=qu[co][0:D, :])

            def emit_v(p, pool, tag, w=C, bufs=None):
                # v for key-tile pair p, all 4 heads, strided into v4 slots
                kw = {"bufs": bufs} if bufs else {}
                ps = pool.tile([PDIM, 2, w], f32, tag=tag, name=f"v{p}", **kw)
                for s in range(2):
                    for t in range(2):
                        nc.tensor.matmul(
                            out=ps[:, s, 0:C],
                            lhsT=xn4[:, t, bass.ts(2 * p + s, PDIM)],
                            rhs=wv_t[:, t, :],
                            start=(t == 0),
                            stop=(t == 1),
                        )
                drain(
                    v4[:, 2 * p : 2 * p + 2, :, 0:D],
                    ps[:, :, 0:C].rearrange("p s (h d) -> p s h d", h=NH),
                )

            def emit_xn(t, cc):
                A16, B16 = AB16[t]
                # 16*xn in fp8e4 (|16 xn| <~ 100 << 240); alternate Scalar/DVE
                if cc % 2 == 0:
                    nc.scalar.activation(
                        out=xn4[:, t, bass.ts(cc, CW)],
                        in_=x_sb[t][:, bass.ts(cc, CW)],
                        func=Ident, bias=B16, scale=A16,
                    )
                else:
                    nc.vector.tensor_scalar(
                        xn4[:, t, bass.ts(cc, CW)],
                        x_sb[t][:, bass.ts(cc, CW)],
                        A16, B16, MUL, ADD,
                    )

            # Weave xn chunks with the k/q/v production that consumes them, so
            # the whole qkv phase pipelines chunk-by-chunk right behind the GN
            # chain instead of serializing phase-by-phase.
            for c2 in range(NIC // 2):
                for c in (2 * c2, 2 * c2 + 1):
                    emit_xn(0, c)
                    emit_xn(1, c)
                if c2 == 0:
                    emit_q(0, qk_ps, "k2")
                emit_k(0, c2, qk_ps, "k2")
                for p in range(4 * c2, 4 * c2 + 4):
                    emit_v(p, qk_ps, "v2", bufs=2)
            for c2 in range(NIC // 2):
                emit_k(1, c2, qk_ps, "k2")
            emit_q(1, qk_ps, "k2")

            def k_src(h, s):
                # head h's k at partition half s
                return (ku if (h % 2) == s else kx)[h // 2]

            def q_src(h, s):
                return (qu if (h % 2) == s else qx)[h // 2]

            # ---------------- main attention loop ----------------
            ps_stack.close()  # release GN/QKV PSUM banks
            ps_stack2 = ExitStack()
            sc_ps = ps_stack2.enter_context(tc.tile_pool(name="sc_ps", bufs=3, space="PSUM"))
            pv_ps_pool = ps_stack2.enter_context(tc.tile_pool(name="pv_ps", bufs=2, space="PSUM"))

            # Per (i-chunk, head) "vchunk": 16 score-pair/exp/PV-DR steps,
            # pipelined LA pairs deep.  Each vchunk's normalization chain
            # (recip -> broadcast -> onorm) is DEFERRED into the next vchunk's
            # pair loop; the projection (4-head PSUM accumulation in a
            # score-pool slot + fused residual) emits once its chunk's 4
            # onorms exist.
            onorms_by_cc = [[] for _ in range(NOC)]

            def emit_proj(cc):
                cslice = bass.ts(cc, CW)
                for co in range(2):
                    pj = sc_ps.tile([PDIM, 2, CW], f32, tag="sc", name=f"pj{co}")
                    for h in range(NH):
                        nc.tensor.matmul(
                            out=pj[:, 0, :],
                            lhsT=wp_sb[h][:, bass.ts(co, PDIM)],
                            rhs=onorms_by_cc[cc][h],
                            start=(h == 0),
                            stop=(h == NH - 1),
                        )
                    yf = ypool.tile([PDIM, CW], f32, tag="yf", name="yf")
                    nc.vector.tensor_add(yf, pj[:, 0, :], resid_sb[co][:, cslice])
                    nc.sync.dma_start(out=y[bass.ts(co, PDIM), cslice], in_=yf)

            def make_chain(cc, pv):
                state = {}

                def stage1():
                    den = mlsm.tile([1, CW], f32, tag="den", name="den")
                    nc.scalar.copy(den, pv[D : D + 1, :])
                    rden = mlsm.tile([1, CW], f32, tag="rden", name="rden")
                    nc.vector.reciprocal_approx_fast(out=rden, in_=den)
                    rdb = mlsm.tile([D, CW], f32, tag="rdb", name="rdb", bufs=2)
                    nc.gpsimd.partition_broadcast(rdb, rden[:, :])
                    state["rdb"] = rdb

                def stage2():
                    onorm = mlsm.tile([D, CW], bf16, tag="onorm", bufs=5, name="onorm")
                    nc.vector.tensor_mul(onorm, state["rdb"], pv[0:D, :])
                    onorms_by_cc[cc].append(onorm)
                    if len(onorms_by_cc[cc]) == NH:
                        emit_proj(cc)
                return stage1, stage2

            # One flat pair-stream over (chunk, head): the pend queue carries
            # ACROSS vchunk boundaries, so the final PV of one head interleaves
            # with the next head's first score/exp pairs and the exp engines
            # never drain at a boundary.  post_q holds the deferred norm-chain
            # stages, drained one per pair-step so they fill pipeline slack.
            pend = []
            post_q = []
            pv_cur = None
            stream = [(cc, h, p) for cc in range(NOC) for h in range(NH) for p in range(NP)]
            for idx, (cc, h, p) in enumerate(stream + [(None, None, q) for q in range(LA)]):
                tail = cc is None
                if not tail:
                    if p == 0:
                        pv_cur = (pv_ps_pool.tile([D + 1, CW], f32, tag="pv", name="pv"), cc, h)
                    cslice = bass.ts(cc, CW)
                    # the pair's two K=64 score matmuls are row-packed into
                    # disjoint PE row-groups (base_partition 0/64) and run
                    # concurrently in one PE pass, writing the two banks of
                    # ONE PSUM tile; a single pair-wide exp drains both.
                    sc = sc_ps.tile([PDIM, 2, CW], f32, tag="sc", name="sc")
                    for s in range(2):
                        jt = 2 * p + s
                        nc.tensor.matmul(
                            out=sc[:, s, :],
                            lhsT=k_src(h, s)[s * D : (s + 1) * D, bass.ts(jt, PDIM)],
                            rhs=q_src(h, s)[s * D : (s + 1) * D, cslice],
                            start=True,
                            stop=True,
                        )
                    es = espool.tile([PDIM, 2, CW], fp8e5, tag="es")
                    if EXP_PATTERN[p % len(EXP_PATTERN)] == "S":
                        # q pre-scaled by d^-0.5*log2(e) host-side: 2^t = exp(ln2*t)
                        nc.scalar.activation(out=es, in_=sc, func=Exp, scale=LN2)
                    else:
                        # 2^t as fp8e5 bits: int8(4t + 60.5); t in [-8.4, 8.4] always
                        # maps to [27, 94] -- never negative/NaN codes.  (The e4m3
                        # variant is UNSAFE: int8 in [-128,-1] hits fp8e4 NaN codes.)
                        nc.vector.tensor_scalar(es.bitcast(i8), sc, 4.0, 60.5, MUL, ADD)
                    pend.append((pv_cur, p, es))
                while len(pend) > (0 if tail and p == LA - 1 else LA) or (tail and len(pend) > LA - 1 - p):
                    (pvt, pcc, ph), p0, es0 = pend.pop(0)
                    nc.tensor.matmul(
                        out=pvt,
                        lhsT=v4[:, 2 * p0 : 2 * p0 + 2, ph, 0 : D + 1],
                        rhs=es0,
                        start=(p0 == 0),
                        stop=(p0 == NP - 1),
                        perf_mode=DR,
                    )
                    if p0 == NP - 1:
                        st1, st2 = make_chain(pcc, pvt)
                        post_q.append(st1)
                        post_q.append(st2)
                if post_q:
                    post_q.pop(0)()
            while post_q:
                post_q.pop(0)()

            ps_stack2.close()

    nc.compile()
    return nc


def make_in_maps(x, gn_gamma, gn_beta, w_qkv, w_proj, b_proj, HW):
    """Per-core input dicts. Core c = (b = c//4, quarter s = c%4).
    x columns are rotated so the core's own quarter comes first."""
    import ml_dtypes

    bf16 = ml_dtypes.bfloat16
    OWN = HW // 4
    log2e = np.log2(np.e)
    x2 = np.ascontiguousarray(x.reshape(B, C, HW).astype(np.float32))
    w_qkv = np.asarray(w_qkv, dtype=np.float32)
    w_proj = np.asarray(w_proj, dtype=np.float32)
    indf = np.zeros((2, PDIM, G), dtype=np.float32)
    indb = np.zeros((2, G, PDIM), dtype=np.float32)
    gsz = C // G  # 32 channels per group
    for t in range(2):
        for p in range(PDIM):
            g = (t * PDIM + p) // gsz
            indf[t, p, g] = 1.0 / gsz
            indb[t, g, p] = 1.0
    fp8 = ml_dtypes.float8_e4m3

    def w4(wslice, scale):
        # [C_in, C_out] -> [128, 2, C_out] fp8, x16 (qkv runs on 16x operands)
        wT = wslice.T * scale
        return np.ascontiguousarray(wT.reshape(2, PDIM, C).transpose(1, 0, 2)).astype(fp8)

    wq4 = w4(w_qkv[0:C, :], 16.0 * (D ** -0.5 * log2e))
    wk4 = w4(w_qkv[C : 2 * C, :], 16.0)
    wv4 = w4(w_qkv[2 * C : 3 * C, :], 16.0)
    wpT = np.ascontiguousarray(w_proj.T).astype(bf16)
    in_maps = []
    for c in range(NCORES):
        b, s = c // 4, c % 4
        xrot = np.roll(x2[b], -s * OWN, axis=1)
        in_maps.append(
            {
                "xb": np.ascontiguousarray(xrot),
                "wq4": wq4,
                "wk4": wk4,
                "wv4": wv4,
                "wpT": wpT,
                "gamma": np.asarray(gn_gamma, dtype=np.float32),
                "beta": np.asarray(gn_beta, dtype=np.float32),
                "bproj": np.asarray(b_proj, dtype=np.float32),
                "indf": indf,
                "indb": indb,
            }
        )
    return in_maps


def assemble_output(results, HW, Himg, Wimg):
    OWN = HW // 4
    y = np.empty((B, C, HW), dtype=np.float32)
    for c in range(NCORES):
        b, s = c // 4, c % 4
        y[b][:, s * OWN : (s + 1) * OWN] = results[c]["y"]
    return y.reshape(B, C, Himg, Wimg)


_NC_CACHE = {}


def kernel(x, gn_gamma, gn_beta, w_qkv, w_proj, b_proj):
    from concourse.bass_utils import run_bass_kernel_spmd

    Himg, Wimg = x.shape[2], x.shape[3]
    HW = Himg * Wimg
    if HW not in _NC_CACHE:
        _NC_CACHE[HW] = build_nc(HW)
    nc = _NC_CACHE[HW]
    in_maps = make_in_maps(x, gn_gamma, gn_beta, w_qkv, w_proj, b_proj, HW)
    res = run_bass_kernel_spmd(nc, in_maps, list(range(NCORES)))
    return assemble_output(res.results, HW, Himg, Wimg)


# revision 21
# speedup vs baseline: 1.2651x; 1.0288x over previous
"""Trainium2 Bass kernel for an AttentionBlock (GroupNorm + single-layer MHA + proj residual).

Reference computation (per batch b):
    xn = GroupNorm(x[b])                        # 8 groups over C=256, HW spatial
    qkv = w_qkv @ xn                            # per-pixel 1x1 conv
    per head h (4 heads, d=64):
        scores = q_h^T k_h * d^-0.5             # [HW, HW]
        attn = softmax(scores, axis=keys)
        out_h = v_h @ attn^T                    # [d, HW]
    y = xn + w_proj @ concat(out_h) + b_proj

Sharding: 8 cores = (batch b in {0,1}) x (query quarter s in {0..3}).  Each
core runs GroupNorm, computes k/v for ALL spatial positions and q for its
own quarter, then runs all 4 heads' attention for its own 1024 query
columns.  The head sum of the projection is a local PSUM accumulation, so
there is NO collective at all: each core writes its own [C, 1024] slice of
the output, with the residual fused into the PSUM drain.

Key kernel-level layout choices (v2):
 - x columns are permuted host-side so each core's OWN quarter comes first;
   attention is permutation-invariant over keys, so k/v/score column order
   doesn't matter.  This kills the separate x_own load and lets the
   residual slice come straight out of the x/xn tiles.
 - scores are computed TRANSPOSED (keys j on partitions, queries i on the
   free axis); softmax denominator comes free as a 65th "ones" column of V.
 - softmax skips max-subtraction; scores live in the log2 domain (q
   pre-scaled by d^-0.5*log2 e host-side).
 - each score PAIR (2 key tiles x 512 queries) lands in ONE 2-bank PSUM
   tile [128,2,512]; ONE pair-wide exp instruction (Scalar native EXP or
   Vector int8 bit-trick) converts it to fp8e5 `es`.  e5m2's 4 steps/octave
   means the bit-trick value range is always a safe positive int8.
 - PV runs as a single fp8 DoubleRow matmul per pair (v4 fp8e4 stationary,
   es fp8e5 moving), halving PE time vs two bf16 matmuls and keeping the
   PE dense enough for the HAM clock gate to hold 2.4 GHz.
 - projection accumulates in a score-pool PSUM slot; residual fused in the
   drain.  Prologue: interleaved x-chunk DMAs (both halves round-robin) so
   GroupNorm stats finish right after the load; a couple of discarded f32
   matmuls on late x chunks pre-warm the PE clock.
"""

import numpy as np

C = 256
NH = 4
D = 64
G = 8
EPS = 1e-5
B = 2
NCORES = 8
PDIM = 128  # partitions
VP = 68     # v4 per-(jt,head) stride: 4*68=272 bytes, dual-fp8 ldweights needs %16==0

PREWARM = True
# per-vchunk exp engine pattern (16 pairs): S=scalar native exp, V=vector trick
EXP_PATTERN = "SVSVSVSSVSVSVSSV"


def build_nc(HW: int):
    import concourse.bass as bass
    import concourse.mybir as mybir
    import concourse.tile as tile
    from concourse import bacc

    f32 = mybir.dt.float32
    bf16 = mybir.dt.bfloat16
    fp8e4 = mybir.dt.float8e4
    fp8e5 = mybir.dt.float8e5
    i8 = mybir.dt.int8
    DR = mybir.MatmulPerfMode.DoubleRow
    CW = min(512, HW)          # i-chunk width (matmul moving-operand max)
    NIC = HW // CW             # number of column chunks of the full image
    OWN = HW // 4              # query columns owned per core
    NOC = OWN // CW            # own-column chunks
    NJT = HW // PDIM           # number of key tiles (128 keys each)
    NP = NJT // 2              # pairs of key tiles
    LA = 3                     # pv lookahead in pairs

    nc = bacc.Bacc(
        "TRN2", target_bir_lowering=False, debug=False, num_devices=NCORES
    )

    xb = nc.declare_dram_parameter("xb", [C, HW], f32, isOutput=False)
    wq4 = nc.declare_dram_parameter("wq4", [PDIM, 2, C], fp8e4, isOutput=False)
    wk4 = nc.declare_dram_parameter("wk4", [PDIM, 2, C], fp8e4, isOutput=False)
    wv4 = nc.declare_dram_parameter("wv4", [PDIM, 2, C], fp8e4, isOutput=False)
    wpT = nc.declare_dram_parameter("wpT", [C, C], bf16, isOutput=False)
    gamma = nc.declare_dram_parameter("gamma", [C], f32, isOutput=False)
    beta = nc.declare_dram_parameter("beta", [C], f32, isOutput=False)
    bproj = nc.declare_dram_parameter("bproj", [C], f32, isOutput=False)
    indf = nc.declare_dram_parameter("indf", [2, PDIM, G], f32, isOutput=False)
    indb = nc.declare_dram_parameter("indb", [2, G, PDIM], f32, isOutput=False)
    y = nc.declare_dram_parameter("y", [C, OWN], f32, isOutput=True)

    Exp = mybir.ActivationFunctionType.Exp
    Sqrt = mybir.ActivationFunctionType.Sqrt
    Ident = mybir.ActivationFunctionType.Identity
    MUL = mybir.AluOpType.mult
    ADD = mybir.AluOpType.add

    BNW = min(512, HW)         # bn_stats max free dim
    NBN = HW // BNW
    LN2 = 0.6931471805599453

    with tile.TileContext(nc) as tc:
        with (
            tc.tile_pool(name="consts", bufs=1) as consts,
            tc.tile_pool(name="xpool", bufs=1) as xpool,
            tc.tile_pool(name="xnpool", bufs=1) as xnpool,
            tc.tile_pool(name="gn_sm", bufs=2) as gn_sm,
            tc.tile_pool(name="qkpool", bufs=1) as qkpool,
            tc.tile_pool(name="espool", bufs=6) as espool,
            tc.tile_pool(name="mlsm", bufs=3) as mlsm,
            tc.tile_pool(name="ypool", bufs=4) as ypool,
        ):
            # ---------------- x load (biggest transfer, gates GN) ----------------
            # Interleave the two channel-halves chunk-by-chunk across the three
            # DMA-capable queues so bn_stats for BOTH halves trail the load by
            # only one chunk.
            dma_engines = [nc.sync, nc.scalar, nc.gpsimd]
            x_sb = [
                xpool.tile([PDIM, HW], f32, tag=f"x{t}", name=f"x{t}") for t in range(2)
            ]
            x4 = xnpool.tile([PDIM, 2, HW], fp8e4, tag="x4")
            di = 0
            for c in range(NIC):
                for t in range(2):
                    dma_engines[di % 3].dma_start(
                        out=x_sb[t][:, bass.ts(c, CW)],
                        in_=xb[bass.ts(t, PDIM), bass.ts(c, CW)],
                    )
                    di += 1

            # 4*x in fp8e4 (fixed scale -- no stats dependency, so it runs
            # inside the load window; the GN affine folds into the weights)
            for c in range(NIC):
                for t in range(2):
                    nc.scalar.activation(
                        out=x4[:, t, bass.ts(c, CW)],
                        in_=x_sb[t][:, bass.ts(c, CW)],
                        func=Ident, bias=0.0, scale=4.0,
                    )

            # ---------------- constants / small loads ----------------
            eps_t = consts.tile([PDIM, 1], f32)
            nc.vector.memset(eps_t, EPS)
            nln2 = consts.tile([PDIM, 1], f32, tag="nln2")
            nc.vector.memset(nln2, -2.0 * 0.6931471805599453)

            indf_sb = []
            indb_sb = []
            gm_sb = []
            bt_sb = []
            bp_sb = []
            for t in range(2):
                it_ = consts.tile([PDIM, G], f32, tag=f"indf{t}")
                nc.sync.dma_start(out=it_, in_=indf[t])
                indf_sb.append(it_)
                ib_ = consts.tile([G, PDIM], f32, tag=f"indb{t}")
                nc.sync.dma_start(out=ib_, in_=indb[t])
                indb_sb.append(ib_)
                g_ = consts.tile([PDIM, 1], f32, tag=f"gm{t}")
                nc.sync.dma_start(out=g_, in_=gamma[bass.ts(t, PDIM)].rearrange("(p o) -> p o", o=1))
                gm_sb.append(g_)
                b_ = consts.tile([PDIM, 1], f32, tag=f"bt{t}")
                nc.sync.dma_start(out=b_, in_=beta[bass.ts(t, PDIM)].rearrange("(p o) -> p o", o=1))
                bt_sb.append(b_)
                bp_ = consts.tile([PDIM, 1], f32, tag=f"bp{t}")
                nc.sync.dma_start(out=bp_, in_=bproj[bass.ts(t, PDIM)].rearrange("(p o) -> p o", o=1))
                bp_sb.append(bp_)

            # weight tiles: fp8, [p, c-half, 256 outputs] (DR k-subtile layout)
            wq_t = consts.tile([PDIM, 2, C], fp8e4, tag="wq")
            nc.sync.dma_start(out=wq_t, in_=wq4[:, :, :])
            wk_t = consts.tile([PDIM, 2, C], fp8e4, tag="wk")
            nc.sync.dma_start(out=wk_t, in_=wk4[:, :, :])
            wv_t = consts.tile([PDIM, 2, C], fp8e4, tag="wv")
            nc.sync.dma_start(out=wv_t, in_=wv4[:, :, :])
            r4 = consts.tile([PDIM, 2, 1], fp8e4, tag="r4")
            wq_s = consts.tile([PDIM, 2, C], fp8e4, tag="wqs")
            wk_s = consts.tile([PDIM, 2, C], fp8e4, tag="wks")
            wv_s = consts.tile([PDIM, 2, C], fp8e4, tag="wvs")
            wp_sb = []
            for h in range(NH):
                wt = consts.tile([D, C], bf16, tag=f"wp{h}", name=f"wp{h}")
                nc.sync.dma_start(out=wt, in_=wpT[h * D : (h + 1) * D, :])
                wp_sb.append(wt)

            from contextlib import ExitStack

            ps_stack = ExitStack()
            gn_ps = ps_stack.enter_context(tc.tile_pool(name="gn_ps", bufs=1, space="PSUM"))

            # ---------------- PE pre-warm (discarded f32 matmuls) ----------------
            # The PE HAM clock gate needs ~3.4us of sustained activity to release
            # 2.4 GHz.  Two slow f32 matmuls on late x chunks put the PE in the
            # busy state right before the GN/QKV/attention stream begins.
            if PREWARM:
                warm = gn_ps.tile([PDIM, 2, CW], f32, tag="warm")
                for w in range(4):
                    nc.tensor.matmul(
                        out=warm[:, w % 2, :],
                        lhsT=x_sb[0][:, (NIC - 4 + w) * CW : (NIC - 4 + w) * CW + PDIM],
                        rhs=x_sb[1][:, bass.ts(NIC - 4 + w, CW)],
                        start=True,
                        stop=True,
                    )

            # ---------------- GroupNorm stats ----------------
            gst_full = gn_ps.tile([PDIM, 2], f32, tag="gnps")
            gst_ps = gst_full[0:G, :]
            for t in range(2):
                stats = gn_sm.tile([PDIM, NBN, 6], f32, tag="bnst")
                for s in range(NBN):
                    nc.vector.bn_stats(out=stats[:, s, :], in_=x_sb[t][:, bass.ts(s, BNW)])
                mv = gn_sm.tile([PDIM, 2], f32, tag="mv")
                nc.vector.bn_aggr(out=mv, in_=stats)
                st2 = gn_sm.tile([PDIM, 2], f32, tag="st2")
                nc.vector.tensor_copy(st2[:, 0:1], mv[:, 0:1])
                sq = gn_sm.tile([PDIM, 1], f32, tag="sq")
                nc.vector.tensor_mul(sq, mv[:, 0:1], mv[:, 0:1])
                nc.vector.tensor_add(st2[:, 1:2], mv[:, 1:2], sq)
                nc.tensor.matmul(
                    out=gst_ps, lhsT=indf_sb[t], rhs=st2, start=(t == 0), stop=(t == 1)
                )

            gst = gn_sm.tile([G, 2], f32, tag="gst_sb")
            nc.vector.tensor_copy(gst, gst_ps)
            mu2 = gn_sm.tile([G, 1], f32, tag="mu2")
            nc.vector.tensor_mul(mu2, gst[:, 0:1], gst[:, 0:1])
            var = gn_sm.tile([G, 1], f32, tag="var")
            nc.vector.tensor_sub(var, gst[:, 1:2], mu2)
            sd = gn_sm.tile([G, 1], f32, tag="sd")
            nc.scalar.activation(out=sd, in_=var, func=Sqrt, bias=eps_t[0:G, :], scale=1.0)
            rstd = gn_sm.tile([G, 1], f32, tag="rstd")
            nc.vector.reciprocal(out=rstd, in_=sd)
            gmr = gn_sm.tile([G, 2], f32, tag="gmr")
            nc.vector.tensor_copy(gmr[:, 0:1], gst[:, 0:1])
            nc.vector.tensor_copy(gmr[:, 1:2], rstd)

            # per-channel affine params.  The normalized-x pass is GONE: the
            # A-scale folds into the fp8 weights (per-partition multiply) and
            # the B-offset folds into per-out-channel drain biases.
            ABs = []
            for t in range(2):
                gb_ps = gn_ps.tile([PDIM, 2], f32, tag="gnps")
                nc.tensor.matmul(out=gb_ps, lhsT=indb_sb[t], rhs=gmr, start=True, stop=True)
                gb = gn_sm.tile([PDIM, 2], f32, tag="gb_sb")
                nc.vector.tensor_copy(gb, gb_ps)
                A_t = gn_sm.tile([PDIM, 1], f32, tag=f"A{t}")
                nc.vector.tensor_mul(A_t, gb[:, 1:2], gm_sb[t])
                tmp = gn_sm.tile([PDIM, 1], f32, tag="tmp")
                nc.vector.tensor_mul(tmp, gb[:, 0:1], A_t)
                B_t = gn_sm.tile([PDIM, 1], f32, tag=f"B{t}")
                nc.vector.tensor_sub(B_t, bt_sb[t], tmp)
                B2_t = gn_sm.tile([PDIM, 1], f32, tag=f"B2{t}")
                nc.vector.tensor_add(B2_t, B_t, bp_sb[t])
                # r' = 256*B/A, fp8, feeds the bias mini-matmuls
                rA = gn_sm.tile([PDIM, 1], f32, tag=f"rA{t}")
                nc.vector.reciprocal(out=rA, in_=A_t)
                rB = gn_sm.tile([PDIM, 1], f32, tag=f"rB{t}")
                nc.vector.tensor_mul(rB, B_t, rA)
                nc.vector.tensor_scalar(r4[:, t, :], rB, 256.0, 0.0, MUL, ADD)
                ABs.append((A_t, B_t, B2_t))

            # scale weights by A (per input channel = per partition, per half)
            for wsrc, wdst in ((wq_t, wq_s), (wk_t, wk_s), (wv_t, wv_s)):
                for t in range(2):
                    nc.vector.tensor_scalar(
                        wdst[:, t, :], wsrc[:, t, :], ABs[t][0], 0.0, MUL, ADD
                    )

            ps_stack.close()  # release GN PSUM banks
            ps_stack = ExitStack()
            qk_ps = ps_stack.enter_context(tc.tile_pool(name="qk_ps", bufs=3, space="PSUM"))

            # bias mini-matmuls: bias_o = sum_c w_oc * B_c, computed from the
            # scaled weights against r' (psum = 16384 * bias -> tiny descale)
            def bias_minis(w_s, dst0, dst1, dscale):
                ps = qk_ps.tile([PDIM, 2, CW], f32, tag="k2", name="bmini")
                for co in range(2):
                    for t in range(2):
                        nc.tensor.matmul(
                            out=ps[:, co, 0:1],
                            lhsT=w_s[:, t, bass.ts(co, PDIM)],
                            rhs=r4[:, t, :],
                            start=(t == 0),
                            stop=(t == 1),
                        )
                nc.vector.tensor_scalar(dst0, ps[:, 0, 0:1], dscale, 0.0, MUL, ADD)
                nc.vector.tensor_scalar(dst1, ps[:, 1, 0:1], dscale, 0.0, MUL, ADD)

            qb = [gn_sm.tile([PDIM, 1], f32, tag=f"qb{co}", name=f"qb{co}") for co in range(2)]
            kb = [gn_sm.tile([PDIM, 1], f32, tag=f"kb{co}", name=f"kb{co}") for co in range(2)]
            vbb = [gn_sm.tile([PDIM, 1], bf16, tag=f"vb{co}", name=f"vb{co}") for co in range(2)]
            bias_minis(wq_s, qb[0], qb[1], 1.0 / 16384.0)
            bias_minis(wk_s, kb[0], kb[1], 1.0 / 16384.0)
            bias_minis(wv_s, vbb[0], vbb[1], 1.0 / 16384.0)
            # v-bias propagates through softmax normalization unchanged, so it
            # folds into the residual constant: B2' = B2 + wp @ vb
            vbh_odd = [gn_sm.tile([D, 1], bf16, tag=f"vbh{i}", name=f"vbh{i}") for i in range(2)]
            for i in range(2):
                nc.vector.tensor_copy(vbh_odd[i], vbb[i][D : 2 * D, :])
            wpvb = qk_ps.tile([PDIM, 2, CW], f32, tag="k2", name="wpvb")
            for co in range(2):
                for h in range(NH):
                    nc.tensor.matmul(
                        out=wpvb[:, co, 0:1],
                        lhsT=wp_sb[h][:, bass.ts(co, PDIM)],
                        rhs=vbb[h // 2][0:D, :] if h % 2 == 0 else vbh_odd[h // 2],
                        start=(h == 0),
                        stop=(h == NH - 1),
                    )
            resid_sb = []
            for t in range(2):
                B2f = gn_sm.tile([PDIM, 1], f32, tag=f"B2f{t}")
                nc.vector.tensor_scalar(B2f, wpvb[:, t, 0:1], 1.0, ABs[t][2], MUL, ADD)
                rs_t = xnpool.tile([PDIM, OWN], f32, tag=f"res{t}")
                nc.gpsimd.tensor_scalar(rs_t, x_sb[t][:, 0:OWN], ABs[t][0], B2f, MUL, ADD)
                resid_sb.append(rs_t)

            # pre-load the gpsimd partition_broadcast ucode lib while the PE/
            # engines are still in the prologue; the main loop's only gpsimd
            # compute is partition_broadcast, so the lib stays resident.
            warmbc = gn_sm.tile([D, G], f32, tag="warmbc")
            nc.gpsimd.partition_broadcast(warmbc, eps_t[0:1, :].broadcast_to([1, G]))

            # ---------------- k, q, v production ----------------
            # k/q: [256 out-ch = 4 heads x 64, cols]; heads 0,1 in out-half 0.
            # Each PSUM tile holds TWO column chunks -> one big drain each.
            drain_engs = [nc.scalar, nc.vector]
            dei = 0

            def drain(dst, src, bias=None):
                # qkv ran on 256x-scaled fp8 operands: descale + GN bias
                nonlocal dei
                eng = drain_engs[dei % 2]
                dei += 1
                if eng is nc.scalar:
                    eng.activation(
                        out=dst, in_=src, func=Ident,
                        bias=0.0 if bias is None else bias, scale=1.0 / 256.0,
                    )
                else:
                    eng.tensor_scalar(
                        dst, src, 1.0 / 256.0, 0.0 if bias is None else bias, MUL, ADD
                    )

            ku = [qkpool.tile([PDIM, HW], bf16, tag=f"ku{co}", name=f"ku{co}") for co in range(2)]
            qu = [qkpool.tile([PDIM, OWN], bf16, tag=f"qu{co}", name=f"qu{co}") for co in range(2)]
            kx = [qkpool.tile([PDIM, HW], bf16, tag=f"kx{cp}", name=f"kx{cp}") for cp in range(2)]
            qx = [qkpool.tile([PDIM, OWN], bf16, tag=f"qx{cp}", name=f"qx{cp}") for cp in range(2)]
            v4 = qkpool.tile([PDIM, NJT, NH, VP], fp8e4, tag="v4")
            nc.vector.memset(v4[:, :, :, D : D + 1], 1.0)
            xdi = [0]

            def emit_k(co, c2, pool, tag):
                # one k2 tile = two column chunks; drain + the swapped-half
                # companion DMAs for those chunks (heads need k in BOTH halves)
                ps = pool.tile([PDIM, 2, CW], f32, tag=tag, name=f"k{co}_{c2}")
                for s in range(2):
                    nc.tensor.matmul(
                        out=ps[:, s, :],
                        lhsT=wk_s[:, :, bass.ts(co, PDIM)],
                        rhs=x4[:, :, bass.ts(2 * c2 + s, CW)],
                        start=True,
                        stop=True,
                        perf_mode=DR,
                    )
                drain(ku[co][:, bass.ts(c2, 2 * CW)], ps.rearrange("p s w -> p (s w)"), kb[co])
                for c in (2 * c2, 2 * c2 + 1):
                    eng = dma_engines[xdi[0] % 3]
                    xdi[0] += 1
                    eng.dma_start(out=kx[co][0:D, bass.ts(c, CW)], in_=ku[co][D : 2 * D, bass.ts(c, CW)])
                    eng.dma_start(out=kx[co][D : 2 * D, bass.ts(c, CW)], in_=ku[co][0:D, bass.ts(c, CW)])

            def emit_q(co, pool, tag):
                ps = pool.tile([PDIM, 2, CW], f32, tag=tag, name=f"q{co}")
                for s in range(NOC):
                    nc.tensor.matmul(
                        out=ps[:, s, :],
                        lhsT=wq_s[:, :, bass.ts(co, PDIM)],
                        rhs=x4[:, :, bass.ts(s, CW)],
                        start=True,
                        stop=True,
                        perf_mode=DR,
                    )
                drain(qu[co], ps.rearrange("p s w -> p (s w)"), qb[co])
                nc.scalar.dma_start(out=qx[co][0:D, :], in_=qu[co][D : 2 * D, :])
                nc.scalar.dma_start(out=qx[co][D : 2 * D, :], in_=qu[co][0:D, :])

            def emit_v(p, pool, tag, w=C, bufs=None):
                # v for key-tile pair p, all 4 heads, strided into v4 slots
                kw = {"bufs": bufs} if bufs else {}
                ps = pool.tile([PDIM, 2, w], f32, tag=tag, name=f"v{p}", **kw)
                for s in range(2):
                    for t in range(2):
                        nc.tensor.matmul(
                            out=ps[:, s, 0:C],
                            lhsT=x4[:, t, bass.ts(2 * p + s, PDIM)],
                            rhs=wv_s[:, t, :],
                            start=(t == 0),
                            stop=(t == 1),
                        )
                drain(
                    v4[:, 2 * p : 2 * p + 2, :, 0:D],
                    ps[:, :, 0:C].rearrange("p s (h d) -> p s h d", h=NH),
                )

            # chunk-woven production (x4 is ready from the load window)
            for c2 in range(NIC // 2):
                if c2 == 0:
                    emit_q(0, qk_ps, "k2")
                emit_k(0, c2, qk_ps, "k2")
                for p in range(4 * c2, 4 * c2 + 4):
                    emit_v(p, qk_ps, "v2", bufs=2)
            for c2 in range(NIC // 2):
                emit_k(1, c2, qk_ps, "k2")
            emit_q(1, qk_ps, "k2")

            def k_src(h, s):
                # head h's k at partition half s
                return (ku if (h % 2) == s else kx)[h // 2]

            def q_src(h, s):
                return (qu if (h % 2) == s else qx)[h // 2]

            # ---------------- main attention loop ----------------
            ps_stack.close()  # release GN/QKV PSUM banks
            ps_stack2 = ExitStack()
            sc_ps = ps_stack2.enter_context(tc.tile_pool(name="sc_ps", bufs=3, space="PSUM"))
            pv_ps_pool = ps_stack2.enter_context(tc.tile_pool(name="pv_ps", bufs=2, space="PSUM"))

            # Per (i-chunk, head) "vchunk": 16 score-pair/exp/PV-DR steps,
            # pipelined LA pairs deep.  Each vchunk's normalization chain
            # (recip -> broadcast -> onorm) is DEFERRED into the next vchunk's
            # pair loop; the projection (4-head PSUM accumulation in a
            # score-pool slot + fused residual) emits once its chunk's 4
            # onorms exist.
            onorms_by_cc = [[] for _ in range(NOC)]

            def emit_proj(cc):
                cslice = bass.ts(cc, CW)
                for co in range(2):
                    pj = sc_ps.tile([PDIM, 2, CW], f32, tag="sc", name=f"pj{co}")
                    for h in range(NH):
                        nc.tensor.matmul(
                            out=pj[:, 0, :],
                            lhsT=wp_sb[h][:, bass.ts(co, PDIM)],
                            rhs=onorms_by_cc[cc][h],
                            start=(h == 0),
                            stop=(h == NH - 1),
                        )
                    yf = ypool.tile([PDIM, CW], f32, tag="yf", name="yf")
                    nc.vector.tensor_add(yf, pj[:, 0, :], resid_sb[co][:, cslice])
                    nc.sync.dma_start(out=y[bass.ts(co, PDIM), cslice], in_=yf)

            def make_chain(cc, pv):
                state = {}

                def stage1():
                    den = mlsm.tile([1, CW], f32, tag="den", name="den")
                    nc.scalar.copy(den, pv[D : D + 1, :])
                    rden = mlsm.tile([1, CW], f32, tag="rden", name="rden")
                    nc.vector.reciprocal_approx_fast(out=rden, in_=den)
                    rdb = mlsm.tile([D, CW], f32, tag="rdb", name="rdb", bufs=2)
                    nc.gpsimd.partition_broadcast(rdb, rden[:, :])
                    state["rdb"] = rdb

                def stage2():
                    onorm = mlsm.tile([D, CW], bf16, tag="onorm", bufs=5, name="onorm")
                    nc.vector.tensor_mul(onorm, state["rdb"], pv[0:D, :])
                    onorms_by_cc[cc].append(onorm)
                    if len(onorms_by_cc[cc]) == NH:
                        emit_proj(cc)
                return stage1, stage2

            # One flat pair-stream over (chunk, head): the pend queue carries
            # ACROSS vchunk boundaries, so the final PV of one head interleaves
            # with the next head's first score/exp pairs and the exp engines
            # never drain at a boundary.  post_q holds the deferred norm-chain
            # stages, drained one per pair-step so they fill pipeline slack.
            pend = []
            post_q = []
            pv_cur = None
            stream = [(cc, h, p) for cc in range(NOC) for h in range(NH) for p in range(NP)]
            for idx, (cc, h, p) in enumerate(stream + [(None, None, q) for q in range(LA)]):
                tail = cc is None
                if not tail:
                    if p == 0:
                        pv_cur = (pv_ps_pool.tile([D + 1, CW], f32, tag="pv", name="pv"), cc, h)
                    cslice = bass.ts(cc, CW)
                    # the pair's two K=64 score matmuls are row-packed into
                    # disjoint PE row-groups (base_partition 0/64) and run
                    # concurrently in one PE pass, writing the two banks of
                    # ONE PSUM tile; a single pair-wide exp drains both.
                    sc = sc_ps.tile([PDIM, 2, CW], f32, tag="sc", name="sc")
                    for s in range(2):
                        jt = 2 * p + s
                        nc.tensor.matmul(
                            out=sc[:, s, :],
                            lhsT=k_src(h, s)[s * D : (s + 1) * D, bass.ts(jt, PDIM)],
                            rhs=q_src(h, s)[s * D : (s + 1) * D, cslice],
                            start=True,
                            stop=True,
                        )
                    es = espool.tile([PDIM, 2, CW], fp8e5, tag="es")
                    if EXP_PATTERN[p % len(EXP_PATTERN)] == "S":
                        # q pre-scaled by d^-0.5*log2(e) host-side: 2^t = exp(ln2*t)
                        nc.scalar.activation(out=es, in_=sc, func=Exp, scale=LN2)
                    else:
                        # 2^t as fp8e5 bits: int8(4t + 60.5); t in [-8.4, 8.4] always
                        # maps to [27, 94] -- never negative/NaN codes.  (The e4m3
                        # variant is UNSAFE: int8 in [-128,-1] hits fp8e4 NaN codes.)
                        nc.vector.tensor_scalar(es.bitcast(i8), sc, 4.0, 60.5, MUL, ADD)
                    pend.append((pv_cur, p, es))
                while len(pend) > (0 if tail and p == LA - 1 else LA) or (tail and len(pend) > LA - 1 - p):
                    (pvt, pcc, ph), p0, es0 = pend.pop(0)
                    nc.tensor.matmul(
                        out=pvt,
                        lhsT=v4[:, 2 * p0 : 2 * p0 + 2, ph, 0 : D + 1],
                        rhs=es0,
                        start=(p0 == 0),
                        stop=(p0 == NP - 1),
                        perf_mode=DR,
                    )
                    if p0 == NP - 1:
                        st1, st2 = make_chain(pcc, pvt)
                        post_q.append(st1)
                        post_q.append(st2)
                if post_q:
                    post_q.pop(0)()
            while post_q:
                post_q.pop(0)()

            ps_stack2.close()

    nc.compile()
    return nc


def make_in_maps(x, gn_gamma, gn_beta, w_qkv, w_proj, b_proj, HW):
    """Per-core input dicts. Core c = (b = c//4, quarter s = c%4).
    x columns are rotated so the core's own quarter comes first."""
    import ml_dtypes

    bf16 = ml_dtypes.bfloat16
    OWN = HW // 4
    log2e = np.log2(np.e)
    x2 = np.ascontiguousarray(x.reshape(B, C, HW).astype(np.float32))
    w_qkv = np.asarray(w_qkv, dtype=np.float32)
    w_proj = np.asarray(w_proj, dtype=np.float32)
    indf = np.zeros((2, PDIM, G), dtype=np.float32)
    indb = np.zeros((2, G, PDIM), dtype=np.float32)
    gsz = C // G  # 32 channels per group
    for t in range(2):
        for p in range(PDIM):
            g = (t * PDIM + p) // gsz
            indf[t, p, g] = 1.0 / gsz
            indb[t, g, p] = 1.0
    fp8 = ml_dtypes.float8_e4m3

    def w4(wslice, scale):
        # [C_in, C_out] -> [128, 2, C_out] fp8, x16 (qkv runs on 16x operands)
        wT = wslice.T * scale
        return np.ascontiguousarray(wT.reshape(2, PDIM, C).transpose(1, 0, 2)).astype(fp8)

    wq4 = w4(w_qkv[0:C, :], 64.0 * (D ** -0.5 * log2e))
    wk4 = w4(w_qkv[C : 2 * C, :], 64.0)
    wv4 = w4(w_qkv[2 * C : 3 * C, :], 64.0)
    wpT = np.ascontiguousarray(w_proj.T).astype(bf16)
    in_maps = []
    for c in range(NCORES):
        b, s = c // 4, c % 4
        xrot = np.roll(x2[b], -s * OWN, axis=1)
        in_maps.append(
            {
                "xb": np.ascontiguousarray(xrot),
                "wq4": wq4,
                "wk4": wk4,
                "wv4": wv4,
                "wpT": wpT,
                "gamma": np.asarray(gn_gamma, dtype=np.float32),
                "beta": np.asarray(gn_beta, dtype=np.float32),
                "bproj": np.asarray(b_proj, dtype=np.float32),
                "indf": indf,
                "indb": indb,
            }
        )
    return in_maps


def assemble_output(results, HW, Himg, Wimg):
    OWN = HW // 4
    y = np.empty((B, C, HW), dtype=np.float32)
    for c in range(NCORES):
        b, s = c // 4, c % 4
        y[b][:, s * OWN : (s + 1) * OWN] = results[c]["y"]
    return y.reshape(B, C, Himg, Wimg)


_NC_CACHE = {}


def kernel(x, gn_gamma, gn_beta, w_qkv, w_proj, b_proj):
    from concourse.bass_utils import run_bass_kernel_spmd

    Himg, Wimg = x.shape[2], x.shape[3]
    HW = Himg * Wimg
    if HW not in _NC_CACHE:
        _NC_CACHE[HW] = build_nc(HW)
    nc = _NC_CACHE[HW]
    in_maps = make_in_maps(x, gn_gamma, gn_beta, w_qkv, w_proj, b_proj, HW)
    res = run_bass_kernel_spmd(nc, in_maps, list(range(NCORES)))
    return assemble_output(res.results, HW, Himg, Wimg)


# revision 22
# speedup vs baseline: 1.2671x; 1.0016x over previous
"""Trainium2 Bass kernel for an AttentionBlock (GroupNorm + single-layer MHA + proj residual).

Reference computation (per batch b):
    xn = GroupNorm(x[b])                        # 8 groups over C=256, HW spatial
    qkv = w_qkv @ xn                            # per-pixel 1x1 conv
    per head h (4 heads, d=64):
        scores = q_h^T k_h * d^-0.5             # [HW, HW]
        attn = softmax(scores, axis=keys)
        out_h = v_h @ attn^T                    # [d, HW]
    y = xn + w_proj @ concat(out_h) + b_proj

Sharding: 8 cores = (batch b in {0,1}) x (query quarter s in {0..3}).  Each
core runs GroupNorm, computes k/v for ALL spatial positions and q for its
own quarter, then runs all 4 heads' attention for its own 1024 query
columns.  The head sum of the projection is a local PSUM accumulation, so
there is NO collective at all: each core writes its own [C, 1024] slice of
the output, with the residual fused into the PSUM drain.

Key kernel-level layout choices (v2):
 - x columns are permuted host-side so each core's OWN quarter comes first;
   attention is permutation-invariant over keys, so k/v/score column order
   doesn't matter.  This kills the separate x_own load and lets the
   residual slice come straight out of the x/xn tiles.
 - scores are computed TRANSPOSED (keys j on partitions, queries i on the
   free axis); softmax denominator comes free as a 65th "ones" column of V.
 - softmax skips max-subtraction; scores live in the log2 domain (q
   pre-scaled by d^-0.5*log2 e host-side).
 - each score PAIR (2 key tiles x 512 queries) lands in ONE 2-bank PSUM
   tile [128,2,512]; ONE pair-wide exp instruction (Scalar native EXP or
   Vector int8 bit-trick) converts it to fp8e5 `es`.  e5m2's 4 steps/octave
   means the bit-trick value range is always a safe positive int8.
 - PV runs as a single fp8 DoubleRow matmul per pair (v4 fp8e4 stationary,
   es fp8e5 moving), halving PE time vs two bf16 matmuls and keeping the
   PE dense enough for the HAM clock gate to hold 2.4 GHz.
 - projection accumulates in a score-pool PSUM slot; residual fused in the
   drain.  Prologue: interleaved x-chunk DMAs (both halves round-robin) so
   GroupNorm stats finish right after the load; a couple of discarded f32
   matmuls on late x chunks pre-warm the PE clock.
"""

import numpy as np

C = 256
NH = 4
D = 64
G = 8
EPS = 1e-5
B = 2
NCORES = 8
PDIM = 128  # partitions
VP = 68     # v4 per-(jt,head) stride: 4*68=272 bytes, dual-fp8 ldweights needs %16==0

PREWARM = True
# per-vchunk exp engine pattern (16 pairs): S=scalar native exp, V=vector trick
EXP_PATTERN = "SVSVSVSSVSVSVSSV"


def build_nc(HW: int):
    import concourse.bass as bass
    import concourse.mybir as mybir
    import concourse.tile as tile
    from concourse import bacc

    f32 = mybir.dt.float32
    bf16 = mybir.dt.bfloat16
    fp8e4 = mybir.dt.float8e4
    fp8e5 = mybir.dt.float8e5
    i8 = mybir.dt.int8
    DR = mybir.MatmulPerfMode.DoubleRow
    CW = min(512, HW)          # i-chunk width (matmul moving-operand max)
    NIC = HW // CW             # number of column chunks of the full image
    OWN = HW // 4              # query columns owned per core
    NOC = OWN // CW            # own-column chunks
    NJT = HW // PDIM           # number of key tiles (128 keys each)
    NP = NJT // 2              # pairs of key tiles
    LA = 3                     # pv lookahead in pairs

    nc = bacc.Bacc(
        "TRN2", target_bir_lowering=False, debug=False, num_devices=NCORES
    )

    xb = nc.declare_dram_parameter("xb", [C, HW], f32, isOutput=False)
    wq4 = nc.declare_dram_parameter("wq4", [PDIM, 2, C], fp8e4, isOutput=False)
    wk4 = nc.declare_dram_parameter("wk4", [PDIM, 2, C], fp8e4, isOutput=False)
    wv4 = nc.declare_dram_parameter("wv4", [PDIM, 2, C], fp8e4, isOutput=False)
    wpT = nc.declare_dram_parameter("wpT", [C, C], bf16, isOutput=False)
    gamma = nc.declare_dram_parameter("gamma", [C], f32, isOutput=False)
    beta = nc.declare_dram_parameter("beta", [C], f32, isOutput=False)
    bproj = nc.declare_dram_parameter("bproj", [C], f32, isOutput=False)
    indf = nc.declare_dram_parameter("indf", [2, PDIM, G], f32, isOutput=False)
    indb = nc.declare_dram_parameter("indb", [2, G, PDIM], f32, isOutput=False)
    y = nc.declare_dram_parameter("y", [C, OWN], f32, isOutput=True)

    Exp = mybir.ActivationFunctionType.Exp
    Sqrt = mybir.ActivationFunctionType.Sqrt
    Ident = mybir.ActivationFunctionType.Identity
    MUL = mybir.AluOpType.mult
    ADD = mybir.AluOpType.add

    BNW = min(512, HW)         # bn_stats max free dim
    NBN = HW // BNW
    LN2 = 0.6931471805599453

    with tile.TileContext(nc) as tc:
        with (
            tc.tile_pool(name="consts", bufs=1) as consts,
            tc.tile_pool(name="xpool", bufs=1) as xpool,
            tc.tile_pool(name="xnpool", bufs=1) as xnpool,
            tc.tile_pool(name="gn_sm", bufs=2) as gn_sm,
            tc.tile_pool(name="qkpool", bufs=1) as qkpool,
            tc.tile_pool(name="espool", bufs=6) as espool,
            tc.tile_pool(name="mlsm", bufs=3) as mlsm,
            tc.tile_pool(name="ypool", bufs=4) as ypool,
        ):
            # ---------------- x load (biggest transfer, gates GN) ----------------
            # Interleave the two channel-halves chunk-by-chunk across the three
            # DMA-capable queues so bn_stats for BOTH halves trail the load by
            # only one chunk.
            dma_engines = [nc.sync, nc.scalar, nc.gpsimd]
            x_sb = [
                xpool.tile([PDIM, HW], f32, tag=f"x{t}", name=f"x{t}") for t in range(2)
            ]
            x4 = xnpool.tile([PDIM, 2, HW], fp8e4, tag="x4")
            di = 0
            for c in range(NIC):
                for t in range(2):
                    dma_engines[di % 3].dma_start(
                        out=x_sb[t][:, bass.ts(c, CW)],
                        in_=xb[bass.ts(t, PDIM), bass.ts(c, CW)],
                    )
                    di += 1

            # 4*x in fp8e4 (fixed scale -- no stats dependency, so it runs
            # inside the load window; the GN affine folds into the weights)
            for c in range(NIC):
                for t in range(2):
                    nc.scalar.activation(
                        out=x4[:, t, bass.ts(c, CW)],
                        in_=x_sb[t][:, bass.ts(c, CW)],
                        func=Ident, bias=0.0, scale=4.0,
                    )

            # ---------------- constants / small loads ----------------
            eps_t = consts.tile([PDIM, 1], f32)
            nc.vector.memset(eps_t, EPS)
            nln2 = consts.tile([PDIM, 1], f32, tag="nln2")
            nc.vector.memset(nln2, -2.0 * 0.6931471805599453)

            indf_sb = []
            indb_sb = []
            gm_sb = []
            bt_sb = []
            bp_sb = []
            for t in range(2):
                it_ = consts.tile([PDIM, G], f32, tag=f"indf{t}")
                nc.sync.dma_start(out=it_, in_=indf[t])
                indf_sb.append(it_)
                ib_ = consts.tile([G, PDIM], f32, tag=f"indb{t}")
                nc.sync.dma_start(out=ib_, in_=indb[t])
                indb_sb.append(ib_)
                g_ = consts.tile([PDIM, 1], f32, tag=f"gm{t}")
                nc.sync.dma_start(out=g_, in_=gamma[bass.ts(t, PDIM)].rearrange("(p o) -> p o", o=1))
                gm_sb.append(g_)
                b_ = consts.tile([PDIM, 1], f32, tag=f"bt{t}")
                nc.sync.dma_start(out=b_, in_=beta[bass.ts(t, PDIM)].rearrange("(p o) -> p o", o=1))
                bt_sb.append(b_)
                bp_ = consts.tile([PDIM, 1], f32, tag=f"bp{t}")
                nc.sync.dma_start(out=bp_, in_=bproj[bass.ts(t, PDIM)].rearrange("(p o) -> p o", o=1))
                bp_sb.append(bp_)

            # weight tiles: fp8, [p, c-half, 256 outputs] (DR k-subtile layout)
            wq_t = consts.tile([PDIM, 2, C], fp8e4, tag="wq")
            nc.sync.dma_start(out=wq_t, in_=wq4[:, :, :])
            wk_t = consts.tile([PDIM, 2, C], fp8e4, tag="wk")
            nc.sync.dma_start(out=wk_t, in_=wk4[:, :, :])
            wv_t = consts.tile([PDIM, 2, C], fp8e4, tag="wv")
            nc.sync.dma_start(out=wv_t, in_=wv4[:, :, :])
            r4 = consts.tile([PDIM, 2, 1], fp8e4, tag="r4")
            wq_s = consts.tile([PDIM, 2, C], fp8e4, tag="wqs")
            wk_s = consts.tile([PDIM, 2, C], fp8e4, tag="wks")
            wv_s = consts.tile([PDIM, 2, C], fp8e4, tag="wvs")
            wp_sb = []
            for h in range(NH):
                wt = consts.tile([D, C], bf16, tag=f"wp{h}", name=f"wp{h}")
                nc.sync.dma_start(out=wt, in_=wpT[h * D : (h + 1) * D, :])
                wp_sb.append(wt)

            from contextlib import ExitStack

            ps_stack = ExitStack()
            gn_ps = ps_stack.enter_context(tc.tile_pool(name="gn_ps", bufs=1, space="PSUM"))

            # ---------------- PE pre-warm (discarded f32 matmuls) ----------------
            # The PE HAM clock gate needs ~3.4us of sustained activity to release
            # 2.4 GHz.  Two slow f32 matmuls on late x chunks put the PE in the
            # busy state right before the GN/QKV/attention stream begins.
            if PREWARM:
                warm = gn_ps.tile([PDIM, 2, CW], f32, tag="warm")
                for w in range(4):
                    nc.tensor.matmul(
                        out=warm[:, w % 2, :],
                        lhsT=x_sb[0][:, (NIC - 4 + w) * CW : (NIC - 4 + w) * CW + PDIM],
                        rhs=x_sb[1][:, bass.ts(NIC - 4 + w, CW)],
                        start=True,
                        stop=True,
                    )

            # ---------------- GroupNorm stats ----------------
            gst_full = gn_ps.tile([PDIM, 2], f32, tag="gnps")
            gst_ps = gst_full[0:G, :]
            for t in range(2):
                stats = gn_sm.tile([PDIM, NBN, 6], f32, tag="bnst")
                for s in range(NBN):
                    nc.vector.bn_stats(out=stats[:, s, :], in_=x_sb[t][:, bass.ts(s, BNW)])
                mv = gn_sm.tile([PDIM, 2], f32, tag="mv")
                nc.vector.bn_aggr(out=mv, in_=stats)
                st2 = gn_sm.tile([PDIM, 2], f32, tag="st2")
                nc.vector.tensor_copy(st2[:, 0:1], mv[:, 0:1])
                sq = gn_sm.tile([PDIM, 1], f32, tag="sq")
                nc.vector.tensor_mul(sq, mv[:, 0:1], mv[:, 0:1])
                nc.vector.tensor_add(st2[:, 1:2], mv[:, 1:2], sq)
                nc.tensor.matmul(
                    out=gst_ps, lhsT=indf_sb[t], rhs=st2, start=(t == 0), stop=(t == 1)
                )

            gst = gn_sm.tile([G, 2], f32, tag="gst_sb")
            nc.vector.tensor_copy(gst, gst_ps)
            mu2 = gn_sm.tile([G, 1], f32, tag="mu2")
            nc.vector.tensor_mul(mu2, gst[:, 0:1], gst[:, 0:1])
            var = gn_sm.tile([G, 1], f32, tag="var")
            nc.vector.tensor_sub(var, gst[:, 1:2], mu2)
            sd = gn_sm.tile([G, 1], f32, tag="sd")
            nc.scalar.activation(out=sd, in_=var, func=Sqrt, bias=eps_t[0:G, :], scale=1.0)
            rstd = gn_sm.tile([G, 1], f32, tag="rstd")
            nc.vector.reciprocal(out=rstd, in_=sd)
            gmr = gn_sm.tile([G, 2], f32, tag="gmr")
            nc.vector.tensor_copy(gmr[:, 0:1], gst[:, 0:1])
            nc.vector.tensor_copy(gmr[:, 1:2], rstd)

            # per-channel affine params.  The normalized-x pass is GONE: the
            # A-scale folds into the fp8 weights (per-partition multiply) and
            # the B-offset folds into per-out-channel drain biases.
            ABs = []
            for t in range(2):
                gb_ps = gn_ps.tile([PDIM, 2], f32, tag="gnps")
                nc.tensor.matmul(out=gb_ps, lhsT=indb_sb[t], rhs=gmr, start=True, stop=True)
                gb = gn_sm.tile([PDIM, 2], f32, tag="gb_sb")
                nc.vector.tensor_copy(gb, gb_ps)
                A_t = gn_sm.tile([PDIM, 1], f32, tag=f"A{t}")
                nc.vector.tensor_mul(A_t, gb[:, 1:2], gm_sb[t])
                tmp = gn_sm.tile([PDIM, 1], f32, tag="tmp")
                nc.vector.tensor_mul(tmp, gb[:, 0:1], A_t)
                B_t = gn_sm.tile([PDIM, 1], f32, tag=f"B{t}")
                nc.vector.tensor_sub(B_t, bt_sb[t], tmp)
                B2_t = gn_sm.tile([PDIM, 1], f32, tag=f"B2{t}")
                nc.vector.tensor_add(B2_t, B_t, bp_sb[t])
                # r' = 256*B/A, fp8, feeds the bias mini-matmuls
                rA = gn_sm.tile([PDIM, 1], f32, tag=f"rA{t}")
                nc.vector.reciprocal(out=rA, in_=A_t)
                rB = gn_sm.tile([PDIM, 1], f32, tag=f"rB{t}")
                nc.vector.tensor_mul(rB, B_t, rA)
                nc.vector.tensor_scalar(r4[:, t, :], rB, 256.0, 0.0, MUL, ADD)
                ABs.append((A_t, B_t, B2_t))

            # scale weights by A (per input channel = per partition, per half)
            for wsrc, wdst in ((wq_t, wq_s), (wk_t, wk_s), (wv_t, wv_s)):
                for t in range(2):
                    nc.scalar.activation(
                        out=wdst[:, t, :], in_=wsrc[:, t, :],
                        func=Ident, bias=0.0, scale=ABs[t][0],
                    )

            ps_stack.close()  # release GN PSUM banks
            ps_stack = ExitStack()
            qk_ps = ps_stack.enter_context(tc.tile_pool(name="qk_ps", bufs=3, space="PSUM"))

            # bias mini-matmuls: bias_o = sum_c w_oc * B_c, computed from the
            # scaled weights against r' (psum = 16384 * bias -> tiny descale)
            def bias_minis(w_s, dst0, dst1, dscale):
                ps = qk_ps.tile([PDIM, 2, CW], f32, tag="k2", name="bmini")
                for co in range(2):
                    for t in range(2):
                        nc.tensor.matmul(
                            out=ps[:, co, 0:1],
                            lhsT=w_s[:, t, bass.ts(co, PDIM)],
                            rhs=r4[:, t, :],
                            start=(t == 0),
                            stop=(t == 1),
                        )
                nc.scalar.activation(out=dst0, in_=ps[:, 0, 0:1], func=Ident, bias=0.0, scale=dscale)
                nc.scalar.activation(out=dst1, in_=ps[:, 1, 0:1], func=Ident, bias=0.0, scale=dscale)

            qb = [gn_sm.tile([PDIM, 1], f32, tag=f"qb{co}", name=f"qb{co}") for co in range(2)]
            kb = [gn_sm.tile([PDIM, 1], f32, tag=f"kb{co}", name=f"kb{co}") for co in range(2)]
            vbb = [gn_sm.tile([PDIM, 1], bf16, tag=f"vb{co}", name=f"vb{co}") for co in range(2)]
            bias_minis(wq_s, qb[0], qb[1], 1.0 / 16384.0)
            bias_minis(wk_s, kb[0], kb[1], 1.0 / 16384.0)
            bias_minis(wv_s, vbb[0], vbb[1], 1.0 / 16384.0)
            # v-bias propagates through softmax normalization unchanged, so it
            # folds into the residual constant: B2' = B2 + wp @ vb
            vbh_odd = [gn_sm.tile([D, 1], bf16, tag=f"vbh{i}", name=f"vbh{i}") for i in range(2)]
            for i in range(2):
                nc.vector.tensor_copy(vbh_odd[i], vbb[i][D : 2 * D, :])
            wpvb = qk_ps.tile([PDIM, 2, CW], f32, tag="k2", name="wpvb")
            for co in range(2):
                for h in range(NH):
                    nc.tensor.matmul(
                        out=wpvb[:, co, 0:1],
                        lhsT=wp_sb[h][:, bass.ts(co, PDIM)],
                        rhs=vbb[h // 2][0:D, :] if h % 2 == 0 else vbh_odd[h // 2],
                        start=(h == 0),
                        stop=(h == NH - 1),
                    )
            resid_sb = []
            for t in range(2):
                B2f = gn_sm.tile([PDIM, 1], f32, tag=f"B2f{t}")
                nc.vector.tensor_scalar(B2f, wpvb[:, t, 0:1], 1.0, ABs[t][2], MUL, ADD)
                rs_t = xnpool.tile([PDIM, OWN], f32, tag=f"res{t}")
                nc.gpsimd.tensor_scalar(rs_t, x_sb[t][:, 0:OWN], ABs[t][0], B2f, MUL, ADD)
                resid_sb.append(rs_t)

            # pre-load the gpsimd partition_broadcast ucode lib while the PE/
            # engines are still in the prologue; the main loop's only gpsimd
            # compute is partition_broadcast, so the lib stays resident.
            warmbc = gn_sm.tile([D, G], f32, tag="warmbc")
            nc.gpsimd.partition_broadcast(warmbc, eps_t[0:1, :].broadcast_to([1, G]))

            # ---------------- k, q, v production ----------------
            # k/q: [256 out-ch = 4 heads x 64, cols]; heads 0,1 in out-half 0.
            # Each PSUM tile holds TWO column chunks -> one big drain each.
            drain_engs = [nc.scalar, nc.vector, nc.vector]
            dei = 0

            def drain(dst, src, bias=None):
                # qkv ran on 256x-scaled fp8 operands: descale + GN bias
                nonlocal dei
                eng = drain_engs[dei % 3]
                dei += 1
                if eng is nc.scalar:
                    eng.activation(
                        out=dst, in_=src, func=Ident,
                        bias=0.0 if bias is None else bias, scale=1.0 / 256.0,
                    )
                else:
                    eng.tensor_scalar(
                        dst, src, 1.0 / 256.0, 0.0 if bias is None else bias, MUL, ADD
                    )

            ku = [qkpool.tile([PDIM, HW], bf16, tag=f"ku{co}", name=f"ku{co}") for co in range(2)]
            qu = [qkpool.tile([PDIM, OWN], bf16, tag=f"qu{co}", name=f"qu{co}") for co in range(2)]
            kx = [qkpool.tile([PDIM, HW], bf16, tag=f"kx{cp}", name=f"kx{cp}") for cp in range(2)]
            qx = [qkpool.tile([PDIM, OWN], bf16, tag=f"qx{cp}", name=f"qx{cp}") for cp in range(2)]
            v4 = qkpool.tile([PDIM, NJT, NH, VP], fp8e4, tag="v4")
            nc.vector.memset(v4[:, :, :, D : D + 1], 1.0)
            xdi = [0]

            def emit_k(co, c2, pool, tag):
                # one k2 tile = two column chunks; drain + the swapped-half
                # companion DMAs for those chunks (heads need k in BOTH halves)
                ps = pool.tile([PDIM, 2, CW], f32, tag=tag, name=f"k{co}_{c2}")
                for s in range(2):
                    nc.tensor.matmul(
                        out=ps[:, s, :],
                        lhsT=wk_s[:, :, bass.ts(co, PDIM)],
                        rhs=x4[:, :, bass.ts(2 * c2 + s, CW)],
                        start=True,
                        stop=True,
                        perf_mode=DR,
                    )
                drain(ku[co][:, bass.ts(c2, 2 * CW)], ps.rearrange("p s w -> p (s w)"), kb[co])
                for c in (2 * c2, 2 * c2 + 1):
                    eng = [nc.sync, nc.gpsimd][xdi[0] % 2]
                    xdi[0] += 1
                    eng.dma_start(out=kx[co][0:D, bass.ts(c, CW)], in_=ku[co][D : 2 * D, bass.ts(c, CW)])
                    eng.dma_start(out=kx[co][D : 2 * D, bass.ts(c, CW)], in_=ku[co][0:D, bass.ts(c, CW)])

            def emit_q(co, pool, tag):
                ps = pool.tile([PDIM, 2, CW], f32, tag=tag, name=f"q{co}")
                for s in range(NOC):
                    nc.tensor.matmul(
                        out=ps[:, s, :],
                        lhsT=wq_s[:, :, bass.ts(co, PDIM)],
                        rhs=x4[:, :, bass.ts(s, CW)],
                        start=True,
                        stop=True,
                        perf_mode=DR,
                    )
                drain(qu[co], ps.rearrange("p s w -> p (s w)"), qb[co])
                nc.sync.dma_start(out=qx[co][0:D, :], in_=qu[co][D : 2 * D, :])
                nc.gpsimd.dma_start(out=qx[co][D : 2 * D, :], in_=qu[co][0:D, :])

            def emit_v(p, pool, tag, w=C, bufs=None):
                # v for key-tile pair p, all 4 heads, strided into v4 slots
                kw = {"bufs": bufs} if bufs else {}
                ps = pool.tile([PDIM, 2, w], f32, tag=tag, name=f"v{p}", **kw)
                for s in range(2):
                    for t in range(2):
                        nc.tensor.matmul(
                            out=ps[:, s, 0:C],
                            lhsT=x4[:, t, bass.ts(2 * p + s, PDIM)],
                            rhs=wv_s[:, t, :],
                            start=(t == 0),
                            stop=(t == 1),
                        )
                drain(
                    v4[:, 2 * p : 2 * p + 2, :, 0:D],
                    ps[:, :, 0:C].rearrange("p s (h d) -> p s h d", h=NH),
                )

            # chunk-woven production (x4 is ready from the load window)
            for c2 in range(NIC // 2):
                if c2 == 0:
                    emit_q(0, qk_ps, "k2")
                emit_k(0, c2, qk_ps, "k2")
                for p in range(4 * c2, 4 * c2 + 4):
                    emit_v(p, qk_ps, "v2", bufs=2)
            for c2 in range(NIC // 2):
                emit_k(1, c2, qk_ps, "k2")
            emit_q(1, qk_ps, "k2")

            def k_src(h, s):
                # head h's k at partition half s
                return (ku if (h % 2) == s else kx)[h // 2]

            def q_src(h, s):
                return (qu if (h % 2) == s else qx)[h // 2]

            # ---------------- main attention loop ----------------
            ps_stack.close()  # release GN/QKV PSUM banks
            ps_stack2 = ExitStack()
            sc_ps = ps_stack2.enter_context(tc.tile_pool(name="sc_ps", bufs=3, space="PSUM"))
            pv_ps_pool = ps_stack2.enter_context(tc.tile_pool(name="pv_ps", bufs=2, space="PSUM"))

            # Per (i-chunk, head) "vchunk": 16 score-pair/exp/PV-DR steps,
            # pipelined LA pairs deep.  Each vchunk's normalization chain
            # (recip -> broadcast -> onorm) is DEFERRED into the next vchunk's
            # pair loop; the projection (4-head PSUM accumulation in a
            # score-pool slot + fused residual) emits once its chunk's 4
            # onorms exist.
            onorms_by_cc = [[] for _ in range(NOC)]

            def emit_proj(cc):
                cslice = bass.ts(cc, CW)
                for co in range(2):
                    pj = sc_ps.tile([PDIM, 2, CW], f32, tag="sc", name=f"pj{co}")
                    for h in range(NH):
                        nc.tensor.matmul(
                            out=pj[:, 0, :],
                            lhsT=wp_sb[h][:, bass.ts(co, PDIM)],
                            rhs=onorms_by_cc[cc][h],
                            start=(h == 0),
                            stop=(h == NH - 1),
                        )
                    yf = ypool.tile([PDIM, CW], f32, tag="yf", name="yf")
                    nc.vector.tensor_add(yf, pj[:, 0, :], resid_sb[co][:, cslice])
                    nc.sync.dma_start(out=y[bass.ts(co, PDIM), cslice], in_=yf)

            def make_chain(cc, pv):
                state = {}

                def stage1():
                    den = mlsm.tile([1, CW], f32, tag="den", name="den")
                    nc.scalar.copy(den, pv[D : D + 1, :])
                    rden = mlsm.tile([1, CW], f32, tag="rden", name="rden")
                    nc.vector.reciprocal_approx_fast(out=rden, in_=den)
                    rdb = mlsm.tile([D, CW], f32, tag="rdb", name="rdb", bufs=2)
                    nc.gpsimd.partition_broadcast(rdb, rden[:, :])
                    state["rdb"] = rdb

                def stage2():
                    onorm = mlsm.tile([D, CW], bf16, tag="onorm", bufs=5, name="onorm")
                    nc.vector.tensor_mul(onorm, state["rdb"], pv[0:D, :])
                    onorms_by_cc[cc].append(onorm)
                    if len(onorms_by_cc[cc]) == NH:
                        emit_proj(cc)
                return stage1, stage2

            # One flat pair-stream over (chunk, head): the pend queue carries
            # ACROSS vchunk boundaries, so the final PV of one head interleaves
            # with the next head's first score/exp pairs and the exp engines
            # never drain at a boundary.  post_q holds the deferred norm-chain
            # stages, drained one per pair-step so they fill pipeline slack.
            pend = []
            post_q = []
            pv_cur = None
            stream = [(cc, h, p) for cc in range(NOC) for h in range(NH) for p in range(NP)]
            for idx, (cc, h, p) in enumerate(stream + [(None, None, q) for q in range(LA)]):
                tail = cc is None
                if not tail:
                    if p == 0:
                        pv_cur = (pv_ps_pool.tile([D + 1, CW], f32, tag="pv", name="pv"), cc, h)
                    cslice = bass.ts(cc, CW)
                    # the pair's two K=64 score matmuls are row-packed into
                    # disjoint PE row-groups (base_partition 0/64) and run
                    # concurrently in one PE pass, writing the two banks of
                    # ONE PSUM tile; a single pair-wide exp drains both.
                    sc = sc_ps.tile([PDIM, 2, CW], f32, tag="sc", name="sc")
                    for s in range(2):
                        jt = 2 * p + s
                        nc.tensor.matmul(
                            out=sc[:, s, :],
                            lhsT=k_src(h, s)[s * D : (s + 1) * D, bass.ts(jt, PDIM)],
                            rhs=q_src(h, s)[s * D : (s + 1) * D, cslice],
                            start=True,
                            stop=True,
                        )
                    es = espool.tile([PDIM, 2, CW], fp8e5, tag="es")
                    if EXP_PATTERN[p % len(EXP_PATTERN)] == "S":
                        # q pre-scaled by d^-0.5*log2(e) host-side: 2^t = exp(ln2*t)
                        nc.scalar.activation(out=es, in_=sc, func=Exp, scale=LN2)
                    else:
                        # 2^t as fp8e5 bits: int8(4t + 60.5); t in [-8.4, 8.4] always
                        # maps to [27, 94] -- never negative/NaN codes.  (The e4m3
                        # variant is UNSAFE: int8 in [-128,-1] hits fp8e4 NaN codes.)
                        nc.vector.tensor_scalar(es.bitcast(i8), sc, 4.0, 60.5, MUL, ADD)
                    pend.append((pv_cur, p, es))
                while len(pend) > (0 if tail and p == LA - 1 else LA) or (tail and len(pend) > LA - 1 - p):
                    (pvt, pcc, ph), p0, es0 = pend.pop(0)
                    nc.tensor.matmul(
                        out=pvt,
                        lhsT=v4[:, 2 * p0 : 2 * p0 + 2, ph, 0 : D + 1],
                        rhs=es0,
                        start=(p0 == 0),
                        stop=(p0 == NP - 1),
                        perf_mode=DR,
                    )
                    if p0 == NP - 1:
                        st1, st2 = make_chain(pcc, pvt)
                        post_q.append(st1)
                        post_q.append(st2)
                if post_q:
                    post_q.pop(0)()
            while post_q:
                post_q.pop(0)()

            ps_stack2.close()

    nc.compile()
    return nc


def make_in_maps(x, gn_gamma, gn_beta, w_qkv, w_proj, b_proj, HW):
    """Per-core input dicts. Core c = (b = c//4, quarter s = c%4).
    x columns are rotated so the core's own quarter comes first."""
    import ml_dtypes

    bf16 = ml_dtypes.bfloat16
    OWN = HW // 4
    log2e = np.log2(np.e)
    x2 = np.ascontiguousarray(x.reshape(B, C, HW).astype(np.float32))
    w_qkv = np.asarray(w_qkv, dtype=np.float32)
    w_proj = np.asarray(w_proj, dtype=np.float32)
    indf = np.zeros((2, PDIM, G), dtype=np.float32)
    indb = np.zeros((2, G, PDIM), dtype=np.float32)
    gsz = C // G  # 32 channels per group
    for t in range(2):
        for p in range(PDIM):
            g = (t * PDIM + p) // gsz
            indf[t, p, g] = 1.0 / gsz
            indb[t, g, p] = 1.0
    fp8 = ml_dtypes.float8_e4m3

    def w4(wslice, scale):
        # [C_in, C_out] -> [128, 2, C_out] fp8, x16 (qkv runs on 16x operands)
        wT = wslice.T * scale
        return np.ascontiguousarray(wT.reshape(2, PDIM, C).transpose(1, 0, 2)).astype(fp8)

    wq4 = w4(w_qkv[0:C, :], 64.0 * (D ** -0.5 * log2e))
    wk4 = w4(w_qkv[C : 2 * C, :], 64.0)
    wv4 = w4(w_qkv[2 * C : 3 * C, :], 64.0)
    wpT = np.ascontiguousarray(w_proj.T).astype(bf16)
    in_maps = []
    for c in range(NCORES):
        b, s = c // 4, c % 4
        xrot = np.roll(x2[b], -s * OWN, axis=1)
        in_maps.append(
            {
                "xb": np.ascontiguousarray(xrot),
                "wq4": wq4,
                "wk4": wk4,
                "wv4": wv4,
                "wpT": wpT,
                "gamma": np.asarray(gn_gamma, dtype=np.float32),
                "beta": np.asarray(gn_beta, dtype=np.float32),
                "bproj": np.asarray(b_proj, dtype=np.float32),
                "indf": indf,
                "indb": indb,
            }
        )
    return in_maps


def assemble_output(results, HW, Himg, Wimg):
    OWN = HW // 4
    y = np.empty((B, C, HW), dtype=np.float32)
    for c in range(NCORES):
        b, s = c // 4, c % 4
        y[b][:, s * OWN : (s + 1) * OWN] = results[c]["y"]
    return y.reshape(B, C, Himg, Wimg)


_NC_CACHE = {}


def kernel(x, gn_gamma, gn_beta, w_qkv, w_proj, b_proj):
    from concourse.bass_utils import run_bass_kernel_spmd

    Himg, Wimg = x.shape[2], x.shape[3]
    HW = Himg * Wimg
    if HW not in _NC_CACHE:
        _NC_CACHE[HW] = build_nc(HW)
    nc = _NC_CACHE[HW]
    in_maps = make_in_maps(x, gn_gamma, gn_beta, w_qkv, w_proj, b_proj, HW)
    res = run_bass_kernel_spmd(nc, in_maps, list(range(NCORES)))
    return assemble_output(res.results, HW, Himg, Wimg)


# revision 23
# speedup vs baseline: 1.2711x; 1.0031x over previous
"""Trainium2 Bass kernel for an AttentionBlock (GroupNorm + single-layer MHA + proj residual).

Reference computation (per batch b):
    xn = GroupNorm(x[b])                        # 8 groups over C=256, HW spatial
    qkv = w_qkv @ xn                            # per-pixel 1x1 conv
    per head h (4 heads, d=64):
        scores = q_h^T k_h * d^-0.5             # [HW, HW]
        attn = softmax(scores, axis=keys)
        out_h = v_h @ attn^T                    # [d, HW]
    y = xn + w_proj @ concat(out_h) + b_proj

Sharding: 8 cores = (batch b in {0,1}) x (query quarter s in {0..3}).  Each
core runs GroupNorm, computes k/v for ALL spatial positions and q for its
own quarter, then runs all 4 heads' attention for its own 1024 query
columns.  The head sum of the projection is a local PSUM accumulation, so
there is NO collective at all: each core writes its own [C, 1024] slice of
the output, with the residual fused into the PSUM drain.

Key kernel-level layout choices:
 - x columns are permuted host-side so each core's OWN quarter comes first;
   attention is permutation-invariant over keys, so k/v/score column order
   doesn't matter (no separate x_own load; residual slices x directly).
 - x is converted to fp8e4 (fixed 4x scale) DURING the HBM load; the
   GroupNorm affine folds into the fp8 qkv weights (per-partition A-scale)
   and per-out-channel drain biases (B via tiny bias matmuls); the v-bias
   passes through softmax normalization unchanged and folds into the
   residual constant via wp @ vb.
 - q/k are fp8 DoubleRow matmuls (256-deep contraction per instruction);
   v is plain fp8 matmuls (its stationary operand changes every tile, so
   DoubleRow's ldweights cost would dominate).
 - scores are computed TRANSPOSED (keys j on partitions, queries i on the
   free axis); softmax denominator comes free as a 65th "ones" column of V.
 - softmax skips max-subtraction; scores live in the log2 domain (q
   pre-scaled by d^-0.5*log2 e host-side).
 - each score PAIR (2 key tiles x 512 queries) lands in ONE 2-bank PSUM
   tile [128,2,512]; ONE pair-wide exp instruction (Scalar native EXP or
   Vector int8 bit-trick) converts it to fp8e5 `es`.  e5m2's 4 steps/octave
   means the bit-trick value range is always a safe positive int8 (an e4m3
   variant is UNSAFE: int8 values in [-128,-1] alias fp8e4 NaN codes).
 - PV runs as a single fp8 DoubleRow matmul per pair (v4 fp8e4 stationary,
   es fp8e5 moving), halving PE time vs two bf16 matmuls.
 - the norm chain's only gpsimd op is partition_broadcast and its ucode
   lib is pre-loaded in the prologue (lib swaps cost ~7us stalls).
 - projection accumulates in a score-pool PSUM slot; residual fused in
   the drain.  Discarded f32 matmuls on late x chunks pre-warm the PE
   HAM clock gate.
"""

import numpy as np

C = 256
NH = 4
D = 64
G = 8
EPS = 1e-5
B = 2
NCORES = 8
PDIM = 128  # partitions
VP = 68     # v4 per-(jt,head) stride: 4*68=272 bytes, dual-fp8 ldweights needs %16==0

PREWARM = True
# per-vchunk exp engine pattern (16 pairs): S=scalar native exp, V=vector trick
EXP_PATTERN = "SVSVSVSSVSVSVSSV"


def build_nc(HW: int):
    import concourse.bass as bass
    import concourse.mybir as mybir
    import concourse.tile as tile
    from concourse import bacc

    f32 = mybir.dt.float32
    bf16 = mybir.dt.bfloat16
    fp8e4 = mybir.dt.float8e4
    fp8e5 = mybir.dt.float8e5
    i8 = mybir.dt.int8
    DR = mybir.MatmulPerfMode.DoubleRow
    CW = min(512, HW)          # i-chunk width (matmul moving-operand max)
    NIC = HW // CW             # number of column chunks of the full image
    OWN = HW // 4              # query columns owned per core
    NOC = OWN // CW            # own-column chunks
    NJT = HW // PDIM           # number of key tiles (128 keys each)
    NP = NJT // 2              # pairs of key tiles
    LA = 3                     # pv lookahead in pairs

    nc = bacc.Bacc(
        "TRN2", target_bir_lowering=False, debug=False, num_devices=NCORES
    )

    xb = nc.declare_dram_parameter("xb", [C, HW], f32, isOutput=False)
    wq4 = nc.declare_dram_parameter("wq4", [PDIM, 2, C], fp8e4, isOutput=False)
    wk4 = nc.declare_dram_parameter("wk4", [PDIM, 2, C], fp8e4, isOutput=False)
    wv4 = nc.declare_dram_parameter("wv4", [PDIM, 2, C], fp8e4, isOutput=False)
    wpT = nc.declare_dram_parameter("wpT", [C, C], bf16, isOutput=False)
    gamma = nc.declare_dram_parameter("gamma", [C], f32, isOutput=False)
    beta = nc.declare_dram_parameter("beta", [C], f32, isOutput=False)
    bproj = nc.declare_dram_parameter("bproj", [C], f32, isOutput=False)
    indf = nc.declare_dram_parameter("indf", [2, PDIM, G], f32, isOutput=False)
    indb = nc.declare_dram_parameter("indb", [2, G, PDIM], f32, isOutput=False)
    y = nc.declare_dram_parameter("y", [C, OWN], f32, isOutput=True)

    Exp = mybir.ActivationFunctionType.Exp
    Sqrt = mybir.ActivationFunctionType.Sqrt
    Ident = mybir.ActivationFunctionType.Identity
    MUL = mybir.AluOpType.mult
    ADD = mybir.AluOpType.add

    BNW = min(512, HW)         # bn_stats max free dim
    NBN = HW // BNW
    LN2 = 0.6931471805599453

    with tile.TileContext(nc) as tc:
        with (
            tc.tile_pool(name="consts", bufs=1) as consts,
            tc.tile_pool(name="xpool", bufs=1) as xpool,
            tc.tile_pool(name="xnpool", bufs=1) as xnpool,
            tc.tile_pool(name="gn_sm", bufs=2) as gn_sm,
            tc.tile_pool(name="qkpool", bufs=1) as qkpool,
            tc.tile_pool(name="espool", bufs=6) as espool,
            tc.tile_pool(name="mlsm", bufs=3) as mlsm,
            tc.tile_pool(name="ypool", bufs=4) as ypool,
        ):
            # ---------------- x load (biggest transfer, gates GN) ----------------
            # Interleave the two channel-halves chunk-by-chunk across the three
            # DMA-capable queues so bn_stats for BOTH halves trail the load by
            # only one chunk.
            dma_engines = [nc.sync, nc.scalar, nc.gpsimd]
            x_sb = [
                xpool.tile([PDIM, HW], f32, tag=f"x{t}", name=f"x{t}") for t in range(2)
            ]
            x4 = xnpool.tile([PDIM, 2, HW], fp8e4, tag="x4")
            di = 0
            for c in range(NIC):
                for t in range(2):
                    dma_engines[di % 3].dma_start(
                        out=x_sb[t][:, bass.ts(c, CW)],
                        in_=xb[bass.ts(t, PDIM), bass.ts(c, CW)],
                    )
                    di += 1

            # 4*x in fp8e4 (fixed scale -- no stats dependency, so it runs
            # inside the load window; the GN affine folds into the weights)
            for c in range(NIC):
                for t in range(2):
                    nc.scalar.activation(
                        out=x4[:, t, bass.ts(c, CW)],
                        in_=x_sb[t][:, bass.ts(c, CW)],
                        func=Ident, bias=0.0, scale=4.0,
                    )

            # ---------------- constants / small loads ----------------
            eps_t = consts.tile([PDIM, 1], f32)
            nc.vector.memset(eps_t, EPS)
            nln2 = consts.tile([PDIM, 1], f32, tag="nln2")
            nc.vector.memset(nln2, -2.0 * 0.6931471805599453)

            indf_sb = []
            indb_sb = []
            gm_sb = []
            bt_sb = []
            bp_sb = []
            for t in range(2):
                it_ = consts.tile([PDIM, G], f32, tag=f"indf{t}")
                nc.sync.dma_start(out=it_, in_=indf[t])
                indf_sb.append(it_)
                ib_ = consts.tile([G, PDIM], f32, tag=f"indb{t}")
                nc.sync.dma_start(out=ib_, in_=indb[t])
                indb_sb.append(ib_)
                g_ = consts.tile([PDIM, 1], f32, tag=f"gm{t}")
                nc.sync.dma_start(out=g_, in_=gamma[bass.ts(t, PDIM)].rearrange("(p o) -> p o", o=1))
                gm_sb.append(g_)
                b_ = consts.tile([PDIM, 1], f32, tag=f"bt{t}")
                nc.sync.dma_start(out=b_, in_=beta[bass.ts(t, PDIM)].rearrange("(p o) -> p o", o=1))
                bt_sb.append(b_)
                bp_ = consts.tile([PDIM, 1], f32, tag=f"bp{t}")
                nc.sync.dma_start(out=bp_, in_=bproj[bass.ts(t, PDIM)].rearrange("(p o) -> p o", o=1))
                bp_sb.append(bp_)

            # weight tiles: fp8, [p, c-half, 256 outputs] (DR k-subtile layout)
            wq_t = consts.tile([PDIM, 2, C], fp8e4, tag="wq")
            nc.sync.dma_start(out=wq_t, in_=wq4[:, :, :])
            wk_t = consts.tile([PDIM, 2, C], fp8e4, tag="wk")
            nc.sync.dma_start(out=wk_t, in_=wk4[:, :, :])
            wv_t = consts.tile([PDIM, 2, C], fp8e4, tag="wv")
            nc.sync.dma_start(out=wv_t, in_=wv4[:, :, :])
            r4 = consts.tile([PDIM, 2, 1], fp8e4, tag="r4")
            wq_s = consts.tile([PDIM, 2, C], fp8e4, tag="wqs")
            wk_s = consts.tile([PDIM, 2, C], fp8e4, tag="wks")
            wv_s = consts.tile([PDIM, 2, C], fp8e4, tag="wvs")
            wp_sb = []
            for h in range(NH):
                wt = consts.tile([D, C], bf16, tag=f"wp{h}", name=f"wp{h}")
                nc.sync.dma_start(out=wt, in_=wpT[h * D : (h + 1) * D, :])
                wp_sb.append(wt)

            from contextlib import ExitStack

            ps_stack = ExitStack()
            gn_ps = ps_stack.enter_context(tc.tile_pool(name="gn_ps", bufs=1, space="PSUM"))

            # ---------------- PE pre-warm (discarded f32 matmuls) ----------------
            # The PE HAM clock gate needs ~3.4us of sustained activity to release
            # 2.4 GHz.  Two slow f32 matmuls on late x chunks put the PE in the
            # busy state right before the GN/QKV/attention stream begins.
            if PREWARM:
                warm = gn_ps.tile([PDIM, 2, CW], f32, tag="warm")
                for w in range(4):
                    nc.tensor.matmul(
                        out=warm[:, w % 2, :],
                        lhsT=x_sb[0][:, (NIC - 4 + w) * CW : (NIC - 4 + w) * CW + PDIM],
                        rhs=x_sb[1][:, bass.ts(NIC - 4 + w, CW)],
                        start=True,
                        stop=True,
                    )

            # ---------------- GroupNorm stats ----------------
            gst_full = gn_ps.tile([PDIM, 2], f32, tag="gnps")
            gst_ps = gst_full[0:G, :]
            for t in range(2):
                stats = gn_sm.tile([PDIM, NBN, 6], f32, tag="bnst")
                for s in range(NBN):
                    nc.vector.bn_stats(out=stats[:, s, :], in_=x_sb[t][:, bass.ts(s, BNW)])
                mv = gn_sm.tile([PDIM, 2], f32, tag="mv")
                nc.vector.bn_aggr(out=mv, in_=stats)
                st2 = gn_sm.tile([PDIM, 2], f32, tag="st2")
                nc.vector.tensor_copy(st2[:, 0:1], mv[:, 0:1])
                sq = gn_sm.tile([PDIM, 1], f32, tag="sq")
                nc.vector.tensor_mul(sq, mv[:, 0:1], mv[:, 0:1])
                nc.vector.tensor_add(st2[:, 1:2], mv[:, 1:2], sq)
                nc.tensor.matmul(
                    out=gst_ps, lhsT=indf_sb[t], rhs=st2, start=(t == 0), stop=(t == 1)
                )

            gst = gn_sm.tile([G, 2], f32, tag="gst_sb")
            nc.vector.tensor_copy(gst, gst_ps)
            mu2 = gn_sm.tile([G, 1], f32, tag="mu2")
            nc.vector.tensor_mul(mu2, gst[:, 0:1], gst[:, 0:1])
            var = gn_sm.tile([G, 1], f32, tag="var")
            nc.vector.tensor_sub(var, gst[:, 1:2], mu2)
            sd = gn_sm.tile([G, 1], f32, tag="sd")
            nc.scalar.activation(out=sd, in_=var, func=Sqrt, bias=eps_t[0:G, :], scale=1.0)
            rstd = gn_sm.tile([G, 1], f32, tag="rstd")
            nc.vector.reciprocal(out=rstd, in_=sd)
            gmr = gn_sm.tile([G, 2], f32, tag="gmr")
            nc.vector.tensor_copy(gmr[:, 0:1], gst[:, 0:1])
            nc.vector.tensor_copy(gmr[:, 1:2], rstd)

            # per-channel affine params.  The normalized-x pass is GONE: the
            # A-scale folds into the fp8 weights (per-partition multiply) and
            # the B-offset folds into per-out-channel drain biases.
            ABs = []
            for t in range(2):
                gb_ps = gn_ps.tile([PDIM, 2], f32, tag="gnps")
                nc.tensor.matmul(out=gb_ps, lhsT=indb_sb[t], rhs=gmr, start=True, stop=True)
                gb = gn_sm.tile([PDIM, 2], f32, tag="gb_sb")
                nc.vector.tensor_copy(gb, gb_ps)
                A_t = gn_sm.tile([PDIM, 1], f32, tag=f"A{t}")
                nc.vector.tensor_mul(A_t, gb[:, 1:2], gm_sb[t])
                tmp = gn_sm.tile([PDIM, 1], f32, tag="tmp")
                nc.vector.tensor_mul(tmp, gb[:, 0:1], A_t)
                B_t = gn_sm.tile([PDIM, 1], f32, tag=f"B{t}")
                nc.vector.tensor_sub(B_t, bt_sb[t], tmp)
                B2_t = gn_sm.tile([PDIM, 1], f32, tag=f"B2{t}")
                nc.vector.tensor_add(B2_t, B_t, bp_sb[t])
                # r' = 256*B/A, fp8, feeds the bias mini-matmuls
                rA = gn_sm.tile([PDIM, 1], f32, tag=f"rA{t}")
                nc.vector.reciprocal(out=rA, in_=A_t)
                rB = gn_sm.tile([PDIM, 1], f32, tag=f"rB{t}")
                nc.vector.tensor_mul(rB, B_t, rA)
                nc.vector.tensor_scalar(r4[:, t, :], rB, 256.0, 0.0, MUL, ADD)
                ABs.append((A_t, B_t, B2_t))

            # scale weights by A (per input channel = per partition, per half)
            for wsrc, wdst in ((wq_t, wq_s), (wk_t, wk_s), (wv_t, wv_s)):
                for t in range(2):
                    nc.scalar.activation(
                        out=wdst[:, t, :], in_=wsrc[:, t, :],
                        func=Ident, bias=0.0, scale=ABs[t][0],
                    )

            ps_stack.close()  # release GN PSUM banks
            ps_stack = ExitStack()
            qk_ps = ps_stack.enter_context(tc.tile_pool(name="qk_ps", bufs=3, space="PSUM"))

            # bias mini-matmuls: bias_o = sum_c w_oc * B_c, computed from the
            # scaled weights against r' (psum = 16384 * bias -> tiny descale)
            def bias_minis(w_s, dst0, dst1, dscale):
                ps = qk_ps.tile([PDIM, 2, CW], f32, tag="k2", name="bmini")
                for co in range(2):
                    for t in range(2):
                        nc.tensor.matmul(
                            out=ps[:, co, 0:1],
                            lhsT=w_s[:, t, bass.ts(co, PDIM)],
                            rhs=r4[:, t, :],
                            start=(t == 0),
                            stop=(t == 1),
                        )
                nc.scalar.activation(out=dst0, in_=ps[:, 0, 0:1], func=Ident, bias=0.0, scale=dscale)
                nc.scalar.activation(out=dst1, in_=ps[:, 1, 0:1], func=Ident, bias=0.0, scale=dscale)

            qb = [gn_sm.tile([PDIM, 1], f32, tag=f"qb{co}", name=f"qb{co}") for co in range(2)]
            kb = [gn_sm.tile([PDIM, 1], f32, tag=f"kb{co}", name=f"kb{co}") for co in range(2)]
            vbb = [gn_sm.tile([PDIM, 1], bf16, tag=f"vb{co}", name=f"vb{co}") for co in range(2)]
            bias_minis(wq_s, qb[0], qb[1], 1.0 / 16384.0)
            bias_minis(wk_s, kb[0], kb[1], 1.0 / 16384.0)
            bias_minis(wv_s, vbb[0], vbb[1], 1.0 / 16384.0)
            # v-bias propagates through softmax normalization unchanged, so it
            # folds into the residual constant: B2' = B2 + wp @ vb
            vbh_odd = [gn_sm.tile([D, 1], bf16, tag=f"vbh{i}", name=f"vbh{i}") for i in range(2)]
            for i in range(2):
                nc.vector.tensor_copy(vbh_odd[i], vbb[i][D : 2 * D, :])
            wpvb = qk_ps.tile([PDIM, 2, CW], f32, tag="k2", name="wpvb")
            for co in range(2):
                for h in range(NH):
                    nc.tensor.matmul(
                        out=wpvb[:, co, 0:1],
                        lhsT=wp_sb[h][:, bass.ts(co, PDIM)],
                        rhs=vbb[h // 2][0:D, :] if h % 2 == 0 else vbh_odd[h // 2],
                        start=(h == 0),
                        stop=(h == NH - 1),
                    )
            resid_sb = []
            for t in range(2):
                B2f = gn_sm.tile([PDIM, 1], f32, tag=f"B2f{t}")
                nc.vector.tensor_scalar(B2f, wpvb[:, t, 0:1], 1.0, ABs[t][2], MUL, ADD)
                rs_t = xnpool.tile([PDIM, OWN], f32, tag=f"res{t}")
                nc.gpsimd.tensor_scalar(rs_t, x_sb[t][:, 0:OWN], ABs[t][0], B2f, MUL, ADD)
                resid_sb.append(rs_t)

            # pre-load the gpsimd partition_broadcast ucode lib while the PE/
            # engines are still in the prologue; the main loop's only gpsimd
            # compute is partition_broadcast, so the lib stays resident.
            warmbc = gn_sm.tile([D, G], f32, tag="warmbc")
            nc.gpsimd.partition_broadcast(warmbc, eps_t[0:1, :].broadcast_to([1, G]))

            # ---------------- k, q, v production ----------------
            # k/q: [256 out-ch = 4 heads x 64, cols]; heads 0,1 in out-half 0.
            # Each PSUM tile holds TWO column chunks -> one big drain each.
            drain_engs = [nc.scalar, nc.vector, nc.vector]
            dei = 0

            def drain(dst, src, bias=None):
                # qkv ran on 256x-scaled fp8 operands: descale + GN bias
                nonlocal dei
                eng = drain_engs[dei % 3]
                dei += 1
                if eng is nc.scalar:
                    eng.activation(
                        out=dst, in_=src, func=Ident,
                        bias=0.0 if bias is None else bias, scale=1.0 / 256.0,
                    )
                else:
                    eng.tensor_scalar(
                        dst, src, 1.0 / 256.0, 0.0 if bias is None else bias, MUL, ADD
                    )

            ku = [qkpool.tile([PDIM, HW], bf16, tag=f"ku{co}", name=f"ku{co}") for co in range(2)]
            qu = [qkpool.tile([PDIM, OWN], bf16, tag=f"qu{co}", name=f"qu{co}") for co in range(2)]
            kx = [qkpool.tile([PDIM, HW], bf16, tag=f"kx{cp}", name=f"kx{cp}") for cp in range(2)]
            qx = [qkpool.tile([PDIM, OWN], bf16, tag=f"qx{cp}", name=f"qx{cp}") for cp in range(2)]
            v4 = qkpool.tile([PDIM, NJT, NH, VP], fp8e4, tag="v4")
            nc.vector.memset(v4[:, :, :, D : D + 1], 1.0)
            xdi = [0]

            def emit_k(co, c2, pool, tag):
                # one k2 tile = two column chunks; drain + the swapped-half
                # companion DMAs for those chunks (heads need k in BOTH halves)
                ps = pool.tile([PDIM, 2, CW], f32, tag=tag, name=f"k{co}_{c2}")
                for s in range(2):
                    nc.tensor.matmul(
                        out=ps[:, s, :],
                        lhsT=wk_s[:, :, bass.ts(co, PDIM)],
                        rhs=x4[:, :, bass.ts(2 * c2 + s, CW)],
                        start=True,
                        stop=True,
                        perf_mode=DR,
                    )
                drain(ku[co][:, bass.ts(c2, 2 * CW)], ps.rearrange("p s w -> p (s w)"), kb[co])
                for c in (2 * c2, 2 * c2 + 1):
                    eng = [nc.sync, nc.gpsimd][xdi[0] % 2]
                    xdi[0] += 1
                    eng.dma_start(out=kx[co][0:D, bass.ts(c, CW)], in_=ku[co][D : 2 * D, bass.ts(c, CW)])
                    eng.dma_start(out=kx[co][D : 2 * D, bass.ts(c, CW)], in_=ku[co][0:D, bass.ts(c, CW)])

            def emit_q(co, pool, tag):
                ps = pool.tile([PDIM, 2, CW], f32, tag=tag, name=f"q{co}")
                for s in range(NOC):
                    nc.tensor.matmul(
                        out=ps[:, s, :],
                        lhsT=wq_s[:, :, bass.ts(co, PDIM)],
                        rhs=x4[:, :, bass.ts(s, CW)],
                        start=True,
                        stop=True,
                        perf_mode=DR,
                    )
                drain(qu[co], ps.rearrange("p s w -> p (s w)"), qb[co])
                nc.sync.dma_start(out=qx[co][0:D, :], in_=qu[co][D : 2 * D, :])
                nc.gpsimd.dma_start(out=qx[co][D : 2 * D, :], in_=qu[co][0:D, :])

            def emit_v(p, pool, tag, w=C, bufs=None):
                # v for key-tile pair p, all 4 heads, strided into v4 slots
                kw = {"bufs": bufs} if bufs else {}
                ps = pool.tile([PDIM, 2, w], f32, tag=tag, name=f"v{p}", **kw)
                for s in range(2):
                    for t in range(2):
                        nc.tensor.matmul(
                            out=ps[:, s, 0:C],
                            lhsT=x4[:, t, bass.ts(2 * p + s, PDIM)],
                            rhs=wv_s[:, t, :],
                            start=(t == 0),
                            stop=(t == 1),
                        )
                drain(
                    v4[:, 2 * p : 2 * p + 2, :, 0:D],
                    ps[:, :, 0:C].rearrange("p s (h d) -> p s h d", h=NH),
                )

            # chunk-woven production (x4 is ready from the load window)
            for c2 in range(NIC // 2):
                if c2 == 0:
                    emit_q(0, qk_ps, "k2")
                emit_k(0, c2, qk_ps, "k2")
                for p in range(4 * c2, 4 * c2 + 4):
                    emit_v(p, qk_ps, "v2", bufs=2)
            for c2 in range(NIC // 2):
                emit_k(1, c2, qk_ps, "k2")
            emit_q(1, qk_ps, "k2")

            def k_src(h, s):
                # head h's k at partition half s
                return (ku if (h % 2) == s else kx)[h // 2]

            def q_src(h, s):
                return (qu if (h % 2) == s else qx)[h // 2]

            # ---------------- main attention loop ----------------
            ps_stack.close()  # release GN/QKV PSUM banks
            ps_stack2 = ExitStack()
            sc_ps = ps_stack2.enter_context(tc.tile_pool(name="sc_ps", bufs=3, space="PSUM"))
            pv_ps_pool = ps_stack2.enter_context(tc.tile_pool(name="pv_ps", bufs=2, space="PSUM"))

            # Per (i-chunk, head) "vchunk": 16 score-pair/exp/PV-DR steps,
            # pipelined LA pairs deep.  Each vchunk's normalization chain
            # (recip -> broadcast -> onorm) is DEFERRED into the next vchunk's
            # pair loop; the projection (4-head PSUM accumulation in a
            # score-pool slot + fused residual) emits once its chunk's 4
            # onorms exist.
            onorms_by_cc = [[] for _ in range(NOC)]

            def emit_proj(cc):
                cslice = bass.ts(cc, CW)
                for co in range(2):
                    pj = sc_ps.tile([PDIM, 2, CW], f32, tag="sc", name=f"pj{co}")
                    for h in range(NH):
                        nc.tensor.matmul(
                            out=pj[:, 0, :],
                            lhsT=wp_sb[h][:, bass.ts(co, PDIM)],
                            rhs=onorms_by_cc[cc][h],
                            start=(h == 0),
                            stop=(h == NH - 1),
                        )
                    yf = ypool.tile([PDIM, CW], f32, tag="yf", name="yf")
                    nc.vector.tensor_add(yf, pj[:, 0, :], resid_sb[co][:, cslice])
                    nc.sync.dma_start(out=y[bass.ts(co, PDIM), cslice], in_=yf)

            def make_chain(cc, pv):
                state = {}

                def stage1():
                    den = mlsm.tile([1, CW], f32, tag="den", name="den")
                    nc.scalar.copy(den, pv[D : D + 1, :])
                    rden = mlsm.tile([1, CW], f32, tag="rden", name="rden")
                    nc.vector.reciprocal_approx_fast(out=rden, in_=den)
                    rdb = mlsm.tile([D, CW], f32, tag="rdb", name="rdb", bufs=2)
                    nc.gpsimd.partition_broadcast(rdb, rden[:, :])
                    state["rdb"] = rdb

                def stage2():
                    onorm = mlsm.tile([D, CW], bf16, tag="onorm", bufs=5, name="onorm")
                    nc.vector.tensor_mul(onorm, state["rdb"], pv[0:D, :])
                    onorms_by_cc[cc].append(onorm)
                    if len(onorms_by_cc[cc]) == NH:
                        emit_proj(cc)
                return stage1, stage2

            # One flat pair-stream over (chunk, head): the pend queue carries
            # ACROSS vchunk boundaries, so the final PV of one head interleaves
            # with the next head's first score/exp pairs and the exp engines
            # never drain at a boundary.  post_q holds the deferred norm-chain
            # stages, drained one per pair-step so they fill pipeline slack.
            pend = []
            post_q = []
            pv_cur = None
            stream = [(cc, h, p) for cc in range(NOC) for h in range(NH) for p in range(NP)]
            for idx, (cc, h, p) in enumerate(stream + [(None, None, q) for q in range(LA)]):
                tail = cc is None
                if not tail:
                    if p == 0:
                        pv_cur = (pv_ps_pool.tile([D + 1, CW], f32, tag="pv", name="pv"), cc, h)
                    cslice = bass.ts(cc, CW)
                    # the pair's two K=64 score matmuls are row-packed into
                    # disjoint PE row-groups (base_partition 0/64) and run
                    # concurrently in one PE pass, writing the two banks of
                    # ONE PSUM tile; a single pair-wide exp drains both.
                    sc = sc_ps.tile([PDIM, 2, CW], f32, tag="sc", name="sc")
                    for s in range(2):
                        jt = 2 * p + s
                        nc.tensor.matmul(
                            out=sc[:, s, :],
                            lhsT=k_src(h, s)[s * D : (s + 1) * D, bass.ts(jt, PDIM)],
                            rhs=q_src(h, s)[s * D : (s + 1) * D, cslice],
                            start=True,
                            stop=True,
                        )
                    es = espool.tile([PDIM, 2, CW], fp8e5, tag="es")
                    if EXP_PATTERN[p % len(EXP_PATTERN)] == "S":
                        # q pre-scaled by d^-0.5*log2(e) host-side: 2^t = exp(ln2*t)
                        nc.scalar.activation(out=es, in_=sc, func=Exp, scale=LN2)
                    else:
                        # 2^t as fp8e5 bits: int8(4t + 60.5); t in [-8.4, 8.4] always
                        # maps to [27, 94] -- never negative/NaN codes.  (The e4m3
                        # variant is UNSAFE: int8 in [-128,-1] hits fp8e4 NaN codes.)
                        nc.vector.tensor_scalar(es.bitcast(i8), sc, 4.0, 60.5, MUL, ADD)
                    pend.append((pv_cur, p, es))
                while len(pend) > (0 if tail and p == LA - 1 else LA) or (tail and len(pend) > LA - 1 - p):
                    (pvt, pcc, ph), p0, es0 = pend.pop(0)
                    nc.tensor.matmul(
                        out=pvt,
                        lhsT=v4[:, 2 * p0 : 2 * p0 + 2, ph, 0 : D + 1],
                        rhs=es0,
                        start=(p0 == 0),
                        stop=(p0 == NP - 1),
                        perf_mode=DR,
                    )
                    if p0 == NP - 1:
                        st1, st2 = make_chain(pcc, pvt)
                        post_q.append(st1)
                        post_q.append(st2)
                if post_q:
                    post_q.pop(0)()
            while post_q:
                post_q.pop(0)()

            ps_stack2.close()

    nc.compile()
    return nc


def make_in_maps(x, gn_gamma, gn_beta, w_qkv, w_proj, b_proj, HW):
    """Per-core input dicts. Core c = (b = c//4, quarter s = c%4).
    x columns are rotated so the core's own quarter comes first."""
    import ml_dtypes

    bf16 = ml_dtypes.bfloat16
    OWN = HW // 4
    log2e = np.log2(np.e)
    x2 = np.ascontiguousarray(x.reshape(B, C, HW).astype(np.float32))
    w_qkv = np.asarray(w_qkv, dtype=np.float32)
    w_proj = np.asarray(w_proj, dtype=np.float32)
    indf = np.zeros((2, PDIM, G), dtype=np.float32)
    indb = np.zeros((2, G, PDIM), dtype=np.float32)
    gsz = C // G  # 32 channels per group
    for t in range(2):
        for p in range(PDIM):
            g = (t * PDIM + p) // gsz
            indf[t, p, g] = 1.0 / gsz
            indb[t, g, p] = 1.0
    fp8 = ml_dtypes.float8_e4m3

    def w4(wslice, scale):
        # [C_in, C_out] -> [128, 2, C_out] fp8, x16 (qkv runs on 16x operands)
        wT = wslice.T * scale
        return np.ascontiguousarray(wT.reshape(2, PDIM, C).transpose(1, 0, 2)).astype(fp8)

    wq4 = w4(w_qkv[0:C, :], 64.0 * (D ** -0.5 * log2e))
    wk4 = w4(w_qkv[C : 2 * C, :], 64.0)
    wv4 = w4(w_qkv[2 * C : 3 * C, :], 64.0)
    wpT = np.ascontiguousarray(w_proj.T).astype(bf16)
    in_maps = []
    for c in range(NCORES):
        b, s = c // 4, c % 4
        xrot = np.roll(x2[b], -s * OWN, axis=1)
        in_maps.append(
            {
                "xb": np.ascontiguousarray(xrot),
                "wq4": wq4,
                "wk4": wk4,
                "wv4": wv4,
                "wpT": wpT,
                "gamma": np.asarray(gn_gamma, dtype=np.float32),
                "beta": np.asarray(gn_beta, dtype=np.float32),
                "bproj": np.asarray(b_proj, dtype=np.float32),
                "indf": indf,
                "indb": indb,
            }
        )
    return in_maps


def assemble_output(results, HW, Himg, Wimg):
    OWN = HW // 4
    y = np.empty((B, C, HW), dtype=np.float32)
    for c in range(NCORES):
        b, s = c // 4, c % 4
        y[b][:, s * OWN : (s + 1) * OWN] = results[c]["y"]
    return y.reshape(B, C, Himg, Wimg)


_NC_CACHE = {}


def kernel(x, gn_gamma, gn_beta, w_qkv, w_proj, b_proj):
    from concourse.bass_utils import run_bass_kernel_spmd

    Himg, Wimg = x.shape[2], x.shape[3]
    HW = Himg * Wimg
    if HW not in _NC_CACHE:
        _NC_CACHE[HW] = build_nc(HW)
    nc = _NC_CACHE[HW]
    in_maps = make_in_maps(x, gn_gamma, gn_beta, w_qkv, w_proj, b_proj, HW)
    res = run_bass_kernel_spmd(nc, in_maps, list(range(NCORES)))
    return assemble_output(res.results, HW, Himg, Wimg)


# revision 24
# speedup vs baseline: 1.2787x; 1.0060x over previous
"""Trainium2 Bass kernel for an AttentionBlock (GroupNorm + single-layer MHA + proj residual).

Reference computation (per batch b):
    xn = GroupNorm(x[b])                        # 8 groups over C=256, HW spatial
    qkv = w_qkv @ xn                            # per-pixel 1x1 conv
    per head h (4 heads, d=64):
        scores = q_h^T k_h * d^-0.5             # [HW, HW]
        attn = softmax(scores, axis=keys)
        out_h = v_h @ attn^T                    # [d, HW]
    y = xn + w_proj @ concat(out_h) + b_proj

Sharding: 8 cores = (batch b in {0,1}) x (query quarter s in {0..3}).  Each
core runs GroupNorm, computes k/v for ALL spatial positions and q for its
own quarter, then runs all 4 heads' attention for its own 1024 query
columns.  The head sum of the projection is a local PSUM accumulation, so
there is NO collective at all: each core writes its own [C, 1024] slice of
the output, with the residual fused into the PSUM drain.

Key kernel-level layout choices:
 - x columns are permuted host-side so each core's OWN quarter comes first;
   attention is permutation-invariant over keys, so k/v/score column order
   doesn't matter (no separate x_own load; residual slices x directly).
 - x is converted to fp8e4 (fixed 4x scale) DURING the HBM load; the
   GroupNorm affine folds into the fp8 qkv weights (per-partition A-scale)
   and per-out-channel drain biases (B via tiny bias matmuls); the v-bias
   passes through softmax normalization unchanged and folds into the
   residual constant via wp @ vb.
 - q/k are fp8 DoubleRow matmuls (256-deep contraction per instruction);
   v is plain fp8 matmuls (its stationary operand changes every tile, so
   DoubleRow's ldweights cost would dominate).
 - scores are computed TRANSPOSED (keys j on partitions, queries i on the
   free axis); softmax denominator comes free as a 65th "ones" column of V.
 - softmax skips max-subtraction; scores live in the log2 domain (q
   pre-scaled by d^-0.5*log2 e host-side).
 - each score PAIR (2 key tiles x 512 queries) lands in ONE 2-bank PSUM
   tile [128,2,512]; ONE pair-wide exp instruction (Scalar native EXP or
   Vector int8 bit-trick) converts it to fp8e5 `es`.  e5m2's 4 steps/octave
   means the bit-trick value range is always a safe positive int8 (an e4m3
   variant is UNSAFE: int8 values in [-128,-1] alias fp8e4 NaN codes).
 - PV runs as a single fp8 DoubleRow matmul per pair (v4 fp8e4 stationary,
   es fp8e5 moving), halving PE time vs two bf16 matmuls.
 - the norm chain's only gpsimd op is partition_broadcast and its ucode
   lib is pre-loaded in the prologue (lib swaps cost ~7us stalls).
 - projection accumulates in a score-pool PSUM slot; residual fused in
   the drain.  Discarded f32 matmuls on late x chunks pre-warm the PE
   HAM clock gate.
"""

import numpy as np

C = 256
NH = 4
D = 64
G = 8
EPS = 1e-5
B = 2
NCORES = 8
PDIM = 128  # partitions
VP = 68     # v4 per-(jt,head) stride: 4*68=272 bytes, dual-fp8 ldweights needs %16==0

PREWARM = True
# per-vchunk exp engine pattern (16 pairs): S=scalar native exp, V=vector trick
EXP_PATTERN = "SVSVSVSSVSVSVSSV"


def build_nc(HW: int):
    import concourse.bass as bass
    import concourse.mybir as mybir
    import concourse.tile as tile
    from concourse import bacc

    f32 = mybir.dt.float32
    bf16 = mybir.dt.bfloat16
    fp8e4 = mybir.dt.float8e4
    fp8e5 = mybir.dt.float8e5
    i8 = mybir.dt.int8
    DR = mybir.MatmulPerfMode.DoubleRow
    CW = min(512, HW)          # i-chunk width (matmul moving-operand max)
    NIC = HW // CW             # number of column chunks of the full image
    OWN = HW // 4              # query columns owned per core
    NOC = OWN // CW            # own-column chunks
    NJT = HW // PDIM           # number of key tiles (128 keys each)
    NP = NJT // 2              # pairs of key tiles
    LA = 4                     # pv lookahead in pairs

    nc = bacc.Bacc(
        "TRN2", target_bir_lowering=False, debug=False, num_devices=NCORES
    )

    xb = nc.declare_dram_parameter("xb", [C, HW], f32, isOutput=False)
    wq4 = nc.declare_dram_parameter("wq4", [PDIM, 2, C], fp8e4, isOutput=False)
    wk4 = nc.declare_dram_parameter("wk4", [PDIM, 2, C], fp8e4, isOutput=False)
    wv4 = nc.declare_dram_parameter("wv4", [PDIM, 2, C], fp8e4, isOutput=False)
    wpT = nc.declare_dram_parameter("wpT", [C, C], bf16, isOutput=False)
    gamma = nc.declare_dram_parameter("gamma", [C], f32, isOutput=False)
    beta = nc.declare_dram_parameter("beta", [C], f32, isOutput=False)
    bproj = nc.declare_dram_parameter("bproj", [C], f32, isOutput=False)
    indf = nc.declare_dram_parameter("indf", [2, PDIM, G], f32, isOutput=False)
    indb = nc.declare_dram_parameter("indb", [2, G, PDIM], f32, isOutput=False)
    y = nc.declare_dram_parameter("y", [C, OWN], f32, isOutput=True)

    Exp = mybir.ActivationFunctionType.Exp
    Sqrt = mybir.ActivationFunctionType.Sqrt
    Ident = mybir.ActivationFunctionType.Identity
    MUL = mybir.AluOpType.mult
    ADD = mybir.AluOpType.add

    BNW = min(512, HW)         # bn_stats max free dim
    NBN = HW // BNW
    LN2 = 0.6931471805599453

    with tile.TileContext(nc) as tc:
        with (
            tc.tile_pool(name="consts", bufs=1) as consts,
            tc.tile_pool(name="xpool", bufs=1) as xpool,
            tc.tile_pool(name="xnpool", bufs=1) as xnpool,
            tc.tile_pool(name="gn_sm", bufs=2) as gn_sm,
            tc.tile_pool(name="qkpool", bufs=1) as qkpool,
            tc.tile_pool(name="espool", bufs=8) as espool,
            tc.tile_pool(name="mlsm", bufs=3) as mlsm,
            tc.tile_pool(name="ypool", bufs=4) as ypool,
        ):
            # ---------------- x load (biggest transfer, gates GN) ----------------
            # Interleave the two channel-halves chunk-by-chunk across the three
            # DMA-capable queues so bn_stats for BOTH halves trail the load by
            # only one chunk.
            dma_engines = [nc.sync, nc.scalar, nc.gpsimd]
            x_sb = [
                xpool.tile([PDIM, HW], f32, tag=f"x{t}", name=f"x{t}") for t in range(2)
            ]
            x4 = xnpool.tile([PDIM, 2, HW], fp8e4, tag="x4")
            di = 0
            for c in range(NIC):
                for t in range(2):
                    dma_engines[di % 3].dma_start(
                        out=x_sb[t][:, bass.ts(c, CW)],
                        in_=xb[bass.ts(t, PDIM), bass.ts(c, CW)],
                    )
                    di += 1

            # 4*x in fp8e4 (fixed scale -- no stats dependency, so it runs
            # inside the load window; the GN affine folds into the weights)
            for c in range(NIC):
                for t in range(2):
                    nc.scalar.activation(
                        out=x4[:, t, bass.ts(c, CW)],
                        in_=x_sb[t][:, bass.ts(c, CW)],
                        func=Ident, bias=0.0, scale=4.0,
                    )

            # ---------------- constants / small loads ----------------
            eps_t = consts.tile([PDIM, 1], f32)
            nc.vector.memset(eps_t, EPS)
            nln2 = consts.tile([PDIM, 1], f32, tag="nln2")
            nc.vector.memset(nln2, -2.0 * 0.6931471805599453)

            indf_sb = []
            indb_sb = []
            gm_sb = []
            bt_sb = []
            bp_sb = []
            for t in range(2):
                it_ = consts.tile([PDIM, G], f32, tag=f"indf{t}")
                nc.sync.dma_start(out=it_, in_=indf[t])
                indf_sb.append(it_)
                ib_ = consts.tile([G, PDIM], f32, tag=f"indb{t}")
                nc.sync.dma_start(out=ib_, in_=indb[t])
                indb_sb.append(ib_)
                g_ = consts.tile([PDIM, 1], f32, tag=f"gm{t}")
                nc.sync.dma_start(out=g_, in_=gamma[bass.ts(t, PDIM)].rearrange("(p o) -> p o", o=1))
                gm_sb.append(g_)
                b_ = consts.tile([PDIM, 1], f32, tag=f"bt{t}")
                nc.sync.dma_start(out=b_, in_=beta[bass.ts(t, PDIM)].rearrange("(p o) -> p o", o=1))
                bt_sb.append(b_)
                bp_ = consts.tile([PDIM, 1], f32, tag=f"bp{t}")
                nc.sync.dma_start(out=bp_, in_=bproj[bass.ts(t, PDIM)].rearrange("(p o) -> p o", o=1))
                bp_sb.append(bp_)

            # weight tiles: fp8, [p, c-half, 256 outputs] (DR k-subtile layout)
            wq_t = consts.tile([PDIM, 2, C], fp8e4, tag="wq")
            nc.sync.dma_start(out=wq_t, in_=wq4[:, :, :])
            wk_t = consts.tile([PDIM, 2, C], fp8e4, tag="wk")
            nc.sync.dma_start(out=wk_t, in_=wk4[:, :, :])
            wv_t = consts.tile([PDIM, 2, C], fp8e4, tag="wv")
            nc.sync.dma_start(out=wv_t, in_=wv4[:, :, :])
            r4 = consts.tile([PDIM, 2, 1], fp8e4, tag="r4")
            wq_s = consts.tile([PDIM, 2, C], fp8e4, tag="wqs")
            wk_s = consts.tile([PDIM, 2, C], fp8e4, tag="wks")
            wv_s = consts.tile([PDIM, 2, C], fp8e4, tag="wvs")
            wp_sb = []
            for h in range(NH):
                wt = consts.tile([D, C], bf16, tag=f"wp{h}", name=f"wp{h}")
                nc.sync.dma_start(out=wt, in_=wpT[h * D : (h + 1) * D, :])
                wp_sb.append(wt)

            from contextlib import ExitStack

            ps_stack = ExitStack()
            gn_ps = ps_stack.enter_context(tc.tile_pool(name="gn_ps", bufs=1, space="PSUM"))

            # ---------------- PE pre-warm (discarded f32 matmuls) ----------------
            # The PE HAM clock gate needs ~3.4us of sustained activity to release
            # 2.4 GHz.  Two slow f32 matmuls on late x chunks put the PE in the
            # busy state right before the GN/QKV/attention stream begins.
            if PREWARM:
                warm = gn_ps.tile([PDIM, 2, CW], f32, tag="warm")
                for w in range(4):
                    nc.tensor.matmul(
                        out=warm[:, w % 2, :],
                        lhsT=x_sb[0][:, (NIC - 4 + w) * CW : (NIC - 4 + w) * CW + PDIM],
                        rhs=x_sb[1][:, bass.ts(NIC - 4 + w, CW)],
                        start=True,
                        stop=True,
                    )

            # ---------------- GroupNorm stats ----------------
            gst_full = gn_ps.tile([PDIM, 2], f32, tag="gnps")
            gst_ps = gst_full[0:G, :]
            for t in range(2):
                stats = gn_sm.tile([PDIM, NBN, 6], f32, tag="bnst")
                for s in range(NBN):
                    nc.vector.bn_stats(out=stats[:, s, :], in_=x_sb[t][:, bass.ts(s, BNW)])
                mv = gn_sm.tile([PDIM, 2], f32, tag="mv")
                nc.vector.bn_aggr(out=mv, in_=stats)
                st2 = gn_sm.tile([PDIM, 2], f32, tag="st2")
                nc.vector.tensor_copy(st2[:, 0:1], mv[:, 0:1])
                sq = gn_sm.tile([PDIM, 1], f32, tag="sq")
                nc.vector.tensor_mul(sq, mv[:, 0:1], mv[:, 0:1])
                nc.vector.tensor_add(st2[:, 1:2], mv[:, 1:2], sq)
                nc.tensor.matmul(
                    out=gst_ps, lhsT=indf_sb[t], rhs=st2, start=(t == 0), stop=(t == 1)
                )

            gst = gn_sm.tile([G, 2], f32, tag="gst_sb")
            nc.vector.tensor_copy(gst, gst_ps)
            mu2 = gn_sm.tile([G, 1], f32, tag="mu2")
            nc.vector.tensor_mul(mu2, gst[:, 0:1], gst[:, 0:1])
            var = gn_sm.tile([G, 1], f32, tag="var")
            nc.vector.tensor_sub(var, gst[:, 1:2], mu2)
            sd = gn_sm.tile([G, 1], f32, tag="sd")
            nc.scalar.activation(out=sd, in_=var, func=Sqrt, bias=eps_t[0:G, :], scale=1.0)
            rstd = gn_sm.tile([G, 1], f32, tag="rstd")
            nc.vector.reciprocal(out=rstd, in_=sd)
            gmr = gn_sm.tile([G, 2], f32, tag="gmr")
            nc.vector.tensor_copy(gmr[:, 0:1], gst[:, 0:1])
            nc.vector.tensor_copy(gmr[:, 1:2], rstd)

            # per-channel affine params.  The normalized-x pass is GONE: the
            # A-scale folds into the fp8 weights (per-partition multiply) and
            # the B-offset folds into per-out-channel drain biases.
            ABs = []
            for t in range(2):
                gb_ps = gn_ps.tile([PDIM, 2], f32, tag="gnps")
                nc.tensor.matmul(out=gb_ps, lhsT=indb_sb[t], rhs=gmr, start=True, stop=True)
                gb = gn_sm.tile([PDIM, 2], f32, tag="gb_sb")
                nc.vector.tensor_copy(gb, gb_ps)
                A_t = gn_sm.tile([PDIM, 1], f32, tag=f"A{t}")
                nc.vector.tensor_mul(A_t, gb[:, 1:2], gm_sb[t])
                tmp = gn_sm.tile([PDIM, 1], f32, tag="tmp")
                nc.vector.tensor_mul(tmp, gb[:, 0:1], A_t)
                B_t = gn_sm.tile([PDIM, 1], f32, tag=f"B{t}")
                nc.vector.tensor_sub(B_t, bt_sb[t], tmp)
                B2_t = gn_sm.tile([PDIM, 1], f32, tag=f"B2{t}")
                nc.vector.tensor_add(B2_t, B_t, bp_sb[t])
                # r' = 256*B/A, fp8, feeds the bias mini-matmuls
                rA = gn_sm.tile([PDIM, 1], f32, tag=f"rA{t}")
                nc.vector.reciprocal(out=rA, in_=A_t)
                rB = gn_sm.tile([PDIM, 1], f32, tag=f"rB{t}")
                nc.vector.tensor_mul(rB, B_t, rA)
                nc.vector.tensor_scalar(r4[:, t, :], rB, 256.0, 0.0, MUL, ADD)
                ABs.append((A_t, B_t, B2_t))

            # scale weights by A (per input channel = per partition, per half)
            for wsrc, wdst in ((wq_t, wq_s), (wk_t, wk_s), (wv_t, wv_s)):
                for t in range(2):
                    nc.scalar.activation(
                        out=wdst[:, t, :], in_=wsrc[:, t, :],
                        func=Ident, bias=0.0, scale=ABs[t][0],
                    )

            ps_stack.close()  # release GN PSUM banks
            ps_stack = ExitStack()
            qk_ps = ps_stack.enter_context(tc.tile_pool(name="qk_ps", bufs=3, space="PSUM"))

            # bias mini-matmuls: bias_o = sum_c w_oc * B_c, computed from the
            # scaled weights against r' (psum = 16384 * bias -> tiny descale)
            def bias_minis(w_s, dst0, dst1, dscale):
                ps = qk_ps.tile([PDIM, 2, CW], f32, tag="k2", name="bmini")
                for co in range(2):
                    for t in range(2):
                        nc.tensor.matmul(
                            out=ps[:, co, 0:1],
                            lhsT=w_s[:, t, bass.ts(co, PDIM)],
                            rhs=r4[:, t, :],
                            start=(t == 0),
                            stop=(t == 1),
                        )
                nc.scalar.activation(out=dst0, in_=ps[:, 0, 0:1], func=Ident, bias=0.0, scale=dscale)
                nc.scalar.activation(out=dst1, in_=ps[:, 1, 0:1], func=Ident, bias=0.0, scale=dscale)

            qb = [gn_sm.tile([PDIM, 1], f32, tag=f"qb{co}", name=f"qb{co}") for co in range(2)]
            kb = [gn_sm.tile([PDIM, 1], f32, tag=f"kb{co}", name=f"kb{co}") for co in range(2)]
            vbb = [gn_sm.tile([PDIM, 1], bf16, tag=f"vb{co}", name=f"vb{co}") for co in range(2)]
            bias_minis(wq_s, qb[0], qb[1], 1.0 / 16384.0)
            bias_minis(wk_s, kb[0], kb[1], 1.0 / 16384.0)
            bias_minis(wv_s, vbb[0], vbb[1], 1.0 / 16384.0)
            # v-bias propagates through softmax normalization unchanged, so it
            # folds into the residual constant: B2' = B2 + wp @ vb
            vbh_odd = [gn_sm.tile([D, 1], bf16, tag=f"vbh{i}", name=f"vbh{i}") for i in range(2)]
            for i in range(2):
                nc.vector.tensor_copy(vbh_odd[i], vbb[i][D : 2 * D, :])
            wpvb = qk_ps.tile([PDIM, 2, CW], f32, tag="k2", name="wpvb")
            for co in range(2):
                for h in range(NH):
                    nc.tensor.matmul(
                        out=wpvb[:, co, 0:1],
                        lhsT=wp_sb[h][:, bass.ts(co, PDIM)],
                        rhs=vbb[h // 2][0:D, :] if h % 2 == 0 else vbh_odd[h // 2],
                        start=(h == 0),
                        stop=(h == NH - 1),
                    )
            resid_sb = []
            for t in range(2):
                B2f = gn_sm.tile([PDIM, 1], f32, tag=f"B2f{t}")
                nc.vector.tensor_scalar(B2f, wpvb[:, t, 0:1], 1.0, ABs[t][2], MUL, ADD)
                rs_t = xnpool.tile([PDIM, OWN], f32, tag=f"res{t}")
                nc.gpsimd.tensor_scalar(rs_t, x_sb[t][:, 0:OWN], ABs[t][0], B2f, MUL, ADD)
                resid_sb.append(rs_t)

            # pre-load the gpsimd partition_broadcast ucode lib while the PE/
            # engines are still in the prologue; the main loop's only gpsimd
            # compute is partition_broadcast, so the lib stays resident.
            warmbc = gn_sm.tile([D, G], f32, tag="warmbc")
            nc.gpsimd.partition_broadcast(warmbc, eps_t[0:1, :].broadcast_to([1, G]))

            # ---------------- k, q, v production ----------------
            # k/q: [256 out-ch = 4 heads x 64, cols]; heads 0,1 in out-half 0.
            # Each PSUM tile holds TWO column chunks -> one big drain each.
            drain_engs = [nc.scalar, nc.vector, nc.vector]
            dei = 0

            def drain(dst, src, bias=None):
                # qkv ran on 256x-scaled fp8 operands: descale + GN bias
                nonlocal dei
                eng = drain_engs[dei % 3]
                dei += 1
                if eng is nc.scalar:
                    eng.activation(
                        out=dst, in_=src, func=Ident,
                        bias=0.0 if bias is None else bias, scale=1.0 / 256.0,
                    )
                else:
                    eng.tensor_scalar(
                        dst, src, 1.0 / 256.0, 0.0 if bias is None else bias, MUL, ADD
                    )

            ku = [qkpool.tile([PDIM, HW], bf16, tag=f"ku{co}", name=f"ku{co}") for co in range(2)]
            qu = [qkpool.tile([PDIM, OWN], bf16, tag=f"qu{co}", name=f"qu{co}") for co in range(2)]
            kx = [qkpool.tile([PDIM, HW], bf16, tag=f"kx{cp}", name=f"kx{cp}") for cp in range(2)]
            qx = [qkpool.tile([PDIM, OWN], bf16, tag=f"qx{cp}", name=f"qx{cp}") for cp in range(2)]
            v4 = qkpool.tile([PDIM, NJT, NH, VP], fp8e4, tag="v4")
            nc.vector.memset(v4[:, :, :, D : D + 1], 1.0)
            xdi = [0]

            def emit_k(co, c2, pool, tag):
                # one k2 tile = two column chunks; drain + the swapped-half
                # companion DMAs for those chunks (heads need k in BOTH halves)
                ps = pool.tile([PDIM, 2, CW], f32, tag=tag, name=f"k{co}_{c2}")
                for s in range(2):
                    nc.tensor.matmul(
                        out=ps[:, s, :],
                        lhsT=wk_s[:, :, bass.ts(co, PDIM)],
                        rhs=x4[:, :, bass.ts(2 * c2 + s, CW)],
                        start=True,
                        stop=True,
                        perf_mode=DR,
                    )
                drain(ku[co][:, bass.ts(c2, 2 * CW)], ps.rearrange("p s w -> p (s w)"), kb[co])
                for c in (2 * c2, 2 * c2 + 1):
                    eng = [nc.sync, nc.gpsimd][xdi[0] % 2]
                    xdi[0] += 1
                    eng.dma_start(out=kx[co][0:D, bass.ts(c, CW)], in_=ku[co][D : 2 * D, bass.ts(c, CW)])
                    eng.dma_start(out=kx[co][D : 2 * D, bass.ts(c, CW)], in_=ku[co][0:D, bass.ts(c, CW)])

            def emit_q(co, pool, tag):
                ps = pool.tile([PDIM, 2, CW], f32, tag=tag, name=f"q{co}")
                for s in range(NOC):
                    nc.tensor.matmul(
                        out=ps[:, s, :],
                        lhsT=wq_s[:, :, bass.ts(co, PDIM)],
                        rhs=x4[:, :, bass.ts(s, CW)],
                        start=True,
                        stop=True,
                        perf_mode=DR,
                    )
                drain(qu[co], ps.rearrange("p s w -> p (s w)"), qb[co])
                nc.sync.dma_start(out=qx[co][0:D, :], in_=qu[co][D : 2 * D, :])
                nc.gpsimd.dma_start(out=qx[co][D : 2 * D, :], in_=qu[co][0:D, :])

            def emit_v(p, pool, tag, w=C, bufs=None):
                # v for key-tile pair p, all 4 heads, strided into v4 slots
                kw = {"bufs": bufs} if bufs else {}
                ps = pool.tile([PDIM, 2, w], f32, tag=tag, name=f"v{p}", **kw)
                for s in range(2):
                    for t in range(2):
                        nc.tensor.matmul(
                            out=ps[:, s, 0:C],
                            lhsT=x4[:, t, bass.ts(2 * p + s, PDIM)],
                            rhs=wv_s[:, t, :],
                            start=(t == 0),
                            stop=(t == 1),
                        )
                drain(
                    v4[:, 2 * p : 2 * p + 2, :, 0:D],
                    ps[:, :, 0:C].rearrange("p s (h d) -> p s h d", h=NH),
                )

            # chunk-woven production (x4 is ready from the load window)
            for c2 in range(NIC // 2):
                if c2 == 0:
                    emit_q(0, qk_ps, "k2")
                emit_k(0, c2, qk_ps, "k2")
                for p in range(4 * c2, 4 * c2 + 4):
                    emit_v(p, qk_ps, "v2", bufs=2)
            for c2 in range(NIC // 2):
                emit_k(1, c2, qk_ps, "k2")
            emit_q(1, qk_ps, "k2")

            def k_src(h, s):
                # head h's k at partition half s
                return (ku if (h % 2) == s else kx)[h // 2]

            def q_src(h, s):
                return (qu if (h % 2) == s else qx)[h // 2]

            # ---------------- main attention loop ----------------
            ps_stack.close()  # release GN/QKV PSUM banks
            ps_stack2 = ExitStack()
            sc_ps = ps_stack2.enter_context(tc.tile_pool(name="sc_ps", bufs=3, space="PSUM"))
            pv_ps_pool = ps_stack2.enter_context(tc.tile_pool(name="pv_ps", bufs=2, space="PSUM"))

            # Per (i-chunk, head) "vchunk": 16 score-pair/exp/PV-DR steps,
            # pipelined LA pairs deep.  Each vchunk's normalization chain
            # (recip -> broadcast -> onorm) is DEFERRED into the next vchunk's
            # pair loop; the projection (4-head PSUM accumulation in a
            # score-pool slot + fused residual) emits once its chunk's 4
            # onorms exist.
            onorms_by_cc = [[] for _ in range(NOC)]

            def emit_proj(cc):
                cslice = bass.ts(cc, CW)
                for co in range(2):
                    pj = sc_ps.tile([PDIM, 2, CW], f32, tag="sc", name=f"pj{co}")
                    for h in range(NH):
                        nc.tensor.matmul(
                            out=pj[:, 0, :],
                            lhsT=wp_sb[h][:, bass.ts(co, PDIM)],
                            rhs=onorms_by_cc[cc][h],
                            start=(h == 0),
                            stop=(h == NH - 1),
                        )
                    yf = ypool.tile([PDIM, CW], f32, tag="yf", name="yf")
                    nc.vector.tensor_add(yf, pj[:, 0, :], resid_sb[co][:, cslice])
                    nc.sync.dma_start(out=y[bass.ts(co, PDIM), cslice], in_=yf)

            def make_chain(cc, pv):
                state = {}

                def stage1():
                    den = mlsm.tile([1, CW], f32, tag="den", name="den")
                    nc.scalar.copy(den, pv[D : D + 1, :])
                    rden = mlsm.tile([1, CW], f32, tag="rden", name="rden")
                    nc.vector.reciprocal_approx_fast(out=rden, in_=den)
                    rdb = mlsm.tile([D, CW], f32, tag="rdb", name="rdb", bufs=2)
                    nc.gpsimd.partition_broadcast(rdb, rden[:, :])
                    state["rdb"] = rdb

                def stage2():
                    onorm = mlsm.tile([D, CW], bf16, tag="onorm", bufs=5, name="onorm")
                    nc.vector.tensor_mul(onorm, state["rdb"], pv[0:D, :])
                    onorms_by_cc[cc].append(onorm)
                    if len(onorms_by_cc[cc]) == NH:
                        emit_proj(cc)
                return stage1, stage2

            # One flat pair-stream over (chunk, head): the pend queue carries
            # ACROSS vchunk boundaries, so the final PV of one head interleaves
            # with the next head's first score/exp pairs and the exp engines
            # never drain at a boundary.  post_q holds the deferred norm-chain
            # stages, drained one per pair-step so they fill pipeline slack.
            pend = []
            post_q = []
            pv_cur = None
            stream = [(cc, h, p) for cc in range(NOC) for h in range(NH) for p in range(NP)]
            for idx, (cc, h, p) in enumerate(stream + [(None, None, q) for q in range(LA)]):
                tail = cc is None
                if not tail:
                    if p == 0:
                        pv_cur = (pv_ps_pool.tile([D + 1, CW], f32, tag="pv", name="pv"), cc, h)
                    cslice = bass.ts(cc, CW)
                    # the pair's two K=64 score matmuls are row-packed into
                    # disjoint PE row-groups (base_partition 0/64) and run
                    # concurrently in one PE pass, writing the two banks of
                    # ONE PSUM tile; a single pair-wide exp drains both.
                    sc = sc_ps.tile([PDIM, 2, CW], f32, tag="sc", name="sc")
                    for s in range(2):
                        jt = 2 * p + s
                        nc.tensor.matmul(
                            out=sc[:, s, :],
                            lhsT=k_src(h, s)[s * D : (s + 1) * D, bass.ts(jt, PDIM)],
                            rhs=q_src(h, s)[s * D : (s + 1) * D, cslice],
                            start=True,
                            stop=True,
                        )
                    es = espool.tile([PDIM, 2, CW], fp8e5, tag="es")
                    if EXP_PATTERN[p % len(EXP_PATTERN)] == "S":
                        # q pre-scaled by d^-0.5*log2(e) host-side: 2^t = exp(ln2*t)
                        nc.scalar.activation(out=es, in_=sc, func=Exp, scale=LN2)
                    else:
                        # 2^t as fp8e5 bits: int8(4t + 60.5); t in [-8.4, 8.4] always
                        # maps to [27, 94] -- never negative/NaN codes.  (The e4m3
                        # variant is UNSAFE: int8 in [-128,-1] hits fp8e4 NaN codes.)
                        nc.vector.tensor_scalar(es.bitcast(i8), sc, 4.0, 60.5, MUL, ADD)
                    pend.append((pv_cur, p, es))
                while len(pend) > (0 if tail and p == LA - 1 else LA) or (tail and len(pend) > LA - 1 - p):
                    (pvt, pcc, ph), p0, es0 = pend.pop(0)
                    nc.tensor.matmul(
                        out=pvt,
                        lhsT=v4[:, 2 * p0 : 2 * p0 + 2, ph, 0 : D + 1],
                        rhs=es0,
                        start=(p0 == 0),
                        stop=(p0 == NP - 1),
                        perf_mode=DR,
                    )
                    if p0 == NP - 1:
                        st1, st2 = make_chain(pcc, pvt)
                        post_q.append(st1)
                        post_q.append(st2)
                if post_q:
                    post_q.pop(0)()
            while post_q:
                post_q.pop(0)()

            ps_stack2.close()

    nc.compile()
    return nc


def make_in_maps(x, gn_gamma, gn_beta, w_qkv, w_proj, b_proj, HW):
    """Per-core input dicts. Core c = (b = c//4, quarter s = c%4).
    x columns are rotated so the core's own quarter comes first."""
    import ml_dtypes

    bf16 = ml_dtypes.bfloat16
    OWN = HW // 4
    log2e = np.log2(np.e)
    x2 = np.ascontiguousarray(x.reshape(B, C, HW).astype(np.float32))
    w_qkv = np.asarray(w_qkv, dtype=np.float32)
    w_proj = np.asarray(w_proj, dtype=np.float32)
    indf = np.zeros((2, PDIM, G), dtype=np.float32)
    indb = np.zeros((2, G, PDIM), dtype=np.float32)
    gsz = C // G  # 32 channels per group
    for t in range(2):
        for p in range(PDIM):
            g = (t * PDIM + p) // gsz
            indf[t, p, g] = 1.0 / gsz
            indb[t, g, p] = 1.0
    fp8 = ml_dtypes.float8_e4m3

    def w4(wslice, scale):
        # [C_in, C_out] -> [128, 2, C_out] fp8, x16 (qkv runs on 16x operands)
        wT = wslice.T * scale
        return np.ascontiguousarray(wT.reshape(2, PDIM, C).transpose(1, 0, 2)).astype(fp8)

    wq4 = w4(w_qkv[0:C, :], 64.0 * (D ** -0.5 * log2e))
    wk4 = w4(w_qkv[C : 2 * C, :], 64.0)
    wv4 = w4(w_qkv[2 * C : 3 * C, :], 64.0)
    wpT = np.ascontiguousarray(w_proj.T).astype(bf16)
    in_maps = []
    for c in range(NCORES):
        b, s = c // 4, c % 4
        xrot = np.roll(x2[b], -s * OWN, axis=1)
        in_maps.append(
            {
                "xb": np.ascontiguousarray(xrot),
                "wq4": wq4,
                "wk4": wk4,
                "wv4": wv4,
                "wpT": wpT,
                "gamma": np.asarray(gn_gamma, dtype=np.float32),
                "beta": np.asarray(gn_beta, dtype=np.float32),
                "bproj": np.asarray(b_proj, dtype=np.float32),
                "indf": indf,
                "indb": indb,
            }
        )
    return in_maps


def assemble_output(results, HW, Himg, Wimg):
    OWN = HW // 4
    y = np.empty((B, C, HW), dtype=np.float32)
    for c in range(NCORES):
        b, s = c // 4, c % 4
        y[b][:, s * OWN : (s + 1) * OWN] = results[c]["y"]
    return y.reshape(B, C, Himg, Wimg)


_NC_CACHE = {}


def kernel(x, gn_gamma, gn_beta, w_qkv, w_proj, b_proj):
    from concourse.bass_utils import run_bass_kernel_spmd

    Himg, Wimg = x.shape[2], x.shape[3]
    HW = Himg * Wimg
    if HW not in _NC_CACHE:
        _NC_CACHE[HW] = build_nc(HW)
    nc = _NC_CACHE[HW]
    in_maps = make_in_maps(x, gn_gamma, gn_beta, w_qkv, w_proj, b_proj, HW)
    res = run_bass_kernel_spmd(nc, in_maps, list(range(NCORES)))
    return assemble_output(res.results, HW, Himg, Wimg)


# revision 25
# speedup vs baseline: 1.3027x; 1.0188x over previous
"""Trainium2 Bass kernel for an AttentionBlock (GroupNorm + single-layer MHA + proj residual).

Reference computation (per batch b):
    xn = GroupNorm(x[b])                        # 8 groups over C=256, HW spatial
    qkv = w_qkv @ xn                            # per-pixel 1x1 conv
    per head h (4 heads, d=64):
        scores = q_h^T k_h * d^-0.5             # [HW, HW]
        attn = softmax(scores, axis=keys)
        out_h = v_h @ attn^T                    # [d, HW]
    y = xn + w_proj @ concat(out_h) + b_proj

Sharding: 8 cores = (batch b in {0,1}) x (query quarter s in {0..3}).  Each
core runs GroupNorm, computes k/v for ALL spatial positions and q for its
own quarter, then runs all 4 heads' attention for its own 1024 query
columns.  The head sum of the projection is a local PSUM accumulation, so
there is NO collective at all: each core writes its own [C, 1024] slice of
the output, with the residual fused into the PSUM drain.

Key kernel-level layout choices:
 - x columns are permuted host-side so each core's OWN quarter comes first;
   attention is permutation-invariant over keys, so k/v/score column order
   doesn't matter (no separate x_own load; residual slices x directly).
 - x is converted to fp8e4 (fixed 4x scale) DURING the HBM load; the
   GroupNorm affine folds into the fp8 qkv weights (per-partition A-scale)
   and per-out-channel drain biases (B via tiny bias matmuls); the v-bias
   passes through softmax normalization unchanged and folds into the
   residual constant via wp @ vb.
 - q/k are fp8 DoubleRow matmuls (256-deep contraction per instruction);
   v is plain fp8 matmuls (its stationary operand changes every tile, so
   DoubleRow's ldweights cost would dominate).
 - scores are computed TRANSPOSED (keys j on partitions, queries i on the
   free axis); softmax denominator comes free as a 65th "ones" column of V.
 - softmax skips max-subtraction; scores live in the log2 domain (q
   pre-scaled by d^-0.5*log2 e host-side).
 - each score PAIR (2 key tiles x 512 queries) lands in ONE 2-bank PSUM
   tile [128,2,512]; ONE pair-wide exp instruction (Scalar native EXP or
   Vector int8 bit-trick) converts it to fp8e5 `es`.  e5m2's 4 steps/octave
   means the bit-trick value range is always a safe positive int8 (an e4m3
   variant is UNSAFE: int8 values in [-128,-1] alias fp8e4 NaN codes).
 - PV runs as a single fp8 DoubleRow matmul per pair (v4 fp8e4 stationary,
   es fp8e5 moving), halving PE time vs two bf16 matmuls.
 - the norm chain's only gpsimd op is partition_broadcast and its ucode
   lib is pre-loaded in the prologue (lib swaps cost ~7us stalls).
 - projection accumulates in a score-pool PSUM slot; residual fused in
   the drain.  Discarded f32 matmuls on late x chunks pre-warm the PE
   HAM clock gate.
"""

import numpy as np

C = 256
NH = 4
D = 64
G = 8
EPS = 1e-5
B = 2
NCORES = 8
PDIM = 128  # partitions
VP = 68     # v4 per-(jt,head) stride: 4*68=272 bytes, dual-fp8 ldweights needs %16==0

PREWARM = True
# per-vchunk exp engine pattern (16 pairs): S=scalar native exp, V=vector trick
EXP_PATTERN = "SVSVSVSSVSVSVSSV"


def build_nc(HW: int):
    import concourse.bass as bass
    import concourse.mybir as mybir
    import concourse.tile as tile
    from concourse import bacc

    f32 = mybir.dt.float32
    bf16 = mybir.dt.bfloat16
    fp8e4 = mybir.dt.float8e4
    fp8e5 = mybir.dt.float8e5
    i8 = mybir.dt.int8
    DR = mybir.MatmulPerfMode.DoubleRow
    CW = min(512, HW)          # i-chunk width (matmul moving-operand max)
    NIC = HW // CW             # number of column chunks of the full image
    OWN = HW // 4              # query columns owned per core
    NOC = OWN // CW            # own-column chunks
    NJT = HW // PDIM           # number of key tiles (128 keys each)
    NP = NJT // 2              # pairs of key tiles
    LA = 5                     # pv lookahead in pairs

    nc = bacc.Bacc(
        "TRN2", target_bir_lowering=False, debug=False, num_devices=NCORES
    )

    xb = nc.declare_dram_parameter("xb", [C, HW], f32, isOutput=False)
    wq4 = nc.declare_dram_parameter("wq4", [PDIM, 2, C], fp8e4, isOutput=False)
    wk4 = nc.declare_dram_parameter("wk4", [PDIM, 2, C], fp8e4, isOutput=False)
    wv4 = nc.declare_dram_parameter("wv4", [PDIM, 2, C], fp8e4, isOutput=False)
    wpT = nc.declare_dram_parameter("wpT", [C, C], bf16, isOutput=False)
    gamma = nc.declare_dram_parameter("gamma", [C], f32, isOutput=False)
    beta = nc.declare_dram_parameter("beta", [C], f32, isOutput=False)
    bproj = nc.declare_dram_parameter("bproj", [C], f32, isOutput=False)
    indf = nc.declare_dram_parameter("indf", [2, PDIM, G], f32, isOutput=False)
    indb = nc.declare_dram_parameter("indb", [2, G, PDIM], f32, isOutput=False)
    y = nc.declare_dram_parameter("y", [C, OWN], f32, isOutput=True)

    Exp = mybir.ActivationFunctionType.Exp
    Sqrt = mybir.ActivationFunctionType.Sqrt
    Ident = mybir.ActivationFunctionType.Identity
    MUL = mybir.AluOpType.mult
    ADD = mybir.AluOpType.add

    BNW = min(512, HW)         # bn_stats max free dim
    NBN = HW // BNW
    LN2 = 0.6931471805599453

    with tile.TileContext(nc) as tc:
        with (
            tc.tile_pool(name="consts", bufs=1) as consts,
            tc.tile_pool(name="xpool", bufs=1) as xpool,
            tc.tile_pool(name="xnpool", bufs=1) as xnpool,
            tc.tile_pool(name="gn_sm", bufs=2) as gn_sm,
            tc.tile_pool(name="qkpool", bufs=1) as qkpool,
            tc.tile_pool(name="espool", bufs=8) as espool,
            tc.tile_pool(name="mlsm", bufs=3) as mlsm,
            tc.tile_pool(name="ypool", bufs=4) as ypool,
        ):
            # ---------------- x load (biggest transfer, gates GN) ----------------
            # Interleave the two channel-halves chunk-by-chunk across the three
            # DMA-capable queues so bn_stats for BOTH halves trail the load by
            # only one chunk.
            dma_engines = [nc.sync, nc.scalar, nc.gpsimd]
            x_sb = [
                xpool.tile([PDIM, HW], f32, tag=f"x{t}", name=f"x{t}") for t in range(2)
            ]
            x4 = xnpool.tile([PDIM, 2, HW], fp8e4, tag="x4")
            di = 0
            for c in range(NIC):
                for t in range(2):
                    dma_engines[di % 3].dma_start(
                        out=x_sb[t][:, bass.ts(c, CW)],
                        in_=xb[bass.ts(t, PDIM), bass.ts(c, CW)],
                    )
                    di += 1

            # 4*x in fp8e4 (fixed scale -- no stats dependency, so it runs
            # inside the load window; the GN affine folds into the weights)
            for c in range(NIC):
                for t in range(2):
                    nc.scalar.activation(
                        out=x4[:, t, bass.ts(c, CW)],
                        in_=x_sb[t][:, bass.ts(c, CW)],
                        func=Ident, bias=0.0, scale=4.0,
                    )

            # ---------------- constants / small loads ----------------
            eps_t = consts.tile([PDIM, 1], f32)
            nc.vector.memset(eps_t, EPS)
            nln2 = consts.tile([PDIM, 1], f32, tag="nln2")
            nc.vector.memset(nln2, -2.0 * 0.6931471805599453)

            indf_sb = []
            indb_sb = []
            gm_sb = []
            bt_sb = []
            bp_sb = []
            for t in range(2):
                it_ = consts.tile([PDIM, G], f32, tag=f"indf{t}")
                nc.sync.dma_start(out=it_, in_=indf[t])
                indf_sb.append(it_)
                ib_ = consts.tile([G, PDIM], f32, tag=f"indb{t}")
                nc.sync.dma_start(out=ib_, in_=indb[t])
                indb_sb.append(ib_)
                g_ = consts.tile([PDIM, 1], f32, tag=f"gm{t}")
                nc.sync.dma_start(out=g_, in_=gamma[bass.ts(t, PDIM)].rearrange("(p o) -> p o", o=1))
                gm_sb.append(g_)
                b_ = consts.tile([PDIM, 1], f32, tag=f"bt{t}")
                nc.sync.dma_start(out=b_, in_=beta[bass.ts(t, PDIM)].rearrange("(p o) -> p o", o=1))
                bt_sb.append(b_)
                bp_ = consts.tile([PDIM, 1], f32, tag=f"bp{t}")
                nc.sync.dma_start(out=bp_, in_=bproj[bass.ts(t, PDIM)].rearrange("(p o) -> p o", o=1))
                bp_sb.append(bp_)

            # weight tiles: fp8, [p, c-half, 256 outputs] (DR k-subtile layout)
            wq_t = consts.tile([PDIM, 2, C], fp8e4, tag="wq")
            nc.sync.dma_start(out=wq_t, in_=wq4[:, :, :])
            wk_t = consts.tile([PDIM, 2, C], fp8e4, tag="wk")
            nc.sync.dma_start(out=wk_t, in_=wk4[:, :, :])
            wv_t = consts.tile([PDIM, 2, C], fp8e4, tag="wv")
            nc.sync.dma_start(out=wv_t, in_=wv4[:, :, :])
            r4 = consts.tile([PDIM, 2, 1], fp8e4, tag="r4")
            wq_s = consts.tile([PDIM, 2, C], fp8e4, tag="wqs")
            wk_s = consts.tile([PDIM, 2, C], fp8e4, tag="wks")
            wv_s = consts.tile([PDIM, 2, C], fp8e4, tag="wvs")
            wp_sb = []
            for h in range(NH):
                wt = consts.tile([D, C], bf16, tag=f"wp{h}", name=f"wp{h}")
                nc.sync.dma_start(out=wt, in_=wpT[h * D : (h + 1) * D, :])
                wp_sb.append(wt)

            from contextlib import ExitStack

            ps_stack = ExitStack()
            gn_ps = ps_stack.enter_context(tc.tile_pool(name="gn_ps", bufs=1, space="PSUM"))

            # ---------------- PE pre-warm (discarded f32 matmuls) ----------------
            # The PE HAM clock gate needs ~3.4us of sustained activity to release
            # 2.4 GHz.  Two slow f32 matmuls on late x chunks put the PE in the
            # busy state right before the GN/QKV/attention stream begins.
            if PREWARM:
                warm = gn_ps.tile([PDIM, 2, CW], f32, tag="warm")
                for w in range(4):
                    nc.tensor.matmul(
                        out=warm[:, w % 2, :],
                        lhsT=x_sb[0][:, (NIC - 4 + w) * CW : (NIC - 4 + w) * CW + PDIM],
                        rhs=x_sb[1][:, bass.ts(NIC - 4 + w, CW)],
                        start=True,
                        stop=True,
                    )

            # ---------------- GroupNorm stats ----------------
            gst_full = gn_ps.tile([PDIM, 2], f32, tag="gnps")
            gst_ps = gst_full[0:G, :]
            for t in range(2):
                stats = gn_sm.tile([PDIM, NBN, 6], f32, tag="bnst")
                for s in range(NBN):
                    nc.vector.bn_stats(out=stats[:, s, :], in_=x_sb[t][:, bass.ts(s, BNW)])
                mv = gn_sm.tile([PDIM, 2], f32, tag="mv")
                nc.vector.bn_aggr(out=mv, in_=stats)
                st2 = gn_sm.tile([PDIM, 2], f32, tag="st2")
                nc.vector.tensor_copy(st2[:, 0:1], mv[:, 0:1])
                sq = gn_sm.tile([PDIM, 1], f32, tag="sq")
                nc.vector.tensor_mul(sq, mv[:, 0:1], mv[:, 0:1])
                nc.vector.tensor_add(st2[:, 1:2], mv[:, 1:2], sq)
                nc.tensor.matmul(
                    out=gst_ps, lhsT=indf_sb[t], rhs=st2, start=(t == 0), stop=(t == 1)
                )

            gst = gn_sm.tile([G, 2], f32, tag="gst_sb")
            nc.vector.tensor_copy(gst, gst_ps)
            mu2 = gn_sm.tile([G, 1], f32, tag="mu2")
            nc.vector.tensor_mul(mu2, gst[:, 0:1], gst[:, 0:1])
            var = gn_sm.tile([G, 1], f32, tag="var")
            nc.vector.tensor_sub(var, gst[:, 1:2], mu2)
            sd = gn_sm.tile([G, 1], f32, tag="sd")
            nc.scalar.activation(out=sd, in_=var, func=Sqrt, bias=eps_t[0:G, :], scale=1.0)
            rstd = gn_sm.tile([G, 1], f32, tag="rstd")
            nc.vector.reciprocal(out=rstd, in_=sd)
            gmr = gn_sm.tile([G, 2], f32, tag="gmr")
            nc.vector.tensor_copy(gmr[:, 0:1], gst[:, 0:1])
            nc.vector.tensor_copy(gmr[:, 1:2], rstd)

            # per-channel affine params.  The normalized-x pass is GONE: the
            # A-scale folds into the fp8 weights (per-partition multiply) and
            # the B-offset folds into per-out-channel drain biases.
            ABs = []
            for t in range(2):
                gb_ps = gn_ps.tile([PDIM, 2], f32, tag="gnps")
                nc.tensor.matmul(out=gb_ps, lhsT=indb_sb[t], rhs=gmr, start=True, stop=True)
                gb = gn_sm.tile([PDIM, 2], f32, tag="gb_sb")
                nc.vector.tensor_copy(gb, gb_ps)
                A_t = gn_sm.tile([PDIM, 1], f32, tag=f"A{t}")
                nc.vector.tensor_mul(A_t, gb[:, 1:2], gm_sb[t])
                tmp = gn_sm.tile([PDIM, 1], f32, tag="tmp")
                nc.vector.tensor_mul(tmp, gb[:, 0:1], A_t)
                B_t = gn_sm.tile([PDIM, 1], f32, tag=f"B{t}")
                nc.vector.tensor_sub(B_t, bt_sb[t], tmp)
                B2_t = gn_sm.tile([PDIM, 1], f32, tag=f"B2{t}")
                nc.vector.tensor_add(B2_t, B_t, bp_sb[t])
                # r' = 256*B/A, fp8, feeds the bias mini-matmuls
                rA = gn_sm.tile([PDIM, 1], f32, tag=f"rA{t}")
                nc.vector.reciprocal(out=rA, in_=A_t)
                rB = gn_sm.tile([PDIM, 1], f32, tag=f"rB{t}")
                nc.vector.tensor_mul(rB, B_t, rA)
                nc.vector.tensor_scalar(r4[:, t, :], rB, 256.0, 0.0, MUL, ADD)
                ABs.append((A_t, B_t, B2_t))

            # scale weights by A (per input channel = per partition, per half)
            for wsrc, wdst in ((wq_t, wq_s), (wk_t, wk_s), (wv_t, wv_s)):
                for t in range(2):
                    nc.scalar.activation(
                        out=wdst[:, t, :], in_=wsrc[:, t, :],
                        func=Ident, bias=0.0, scale=ABs[t][0],
                    )

            ps_stack.close()  # release GN PSUM banks
            ps_stack = ExitStack()
            qk_ps = ps_stack.enter_context(tc.tile_pool(name="qk_ps", bufs=3, space="PSUM"))

            # bias mini-matmuls: bias_o = sum_c w_oc * B_c, computed from the
            # scaled weights against r' (psum = 16384 * bias -> tiny descale)
            def bias_minis(w_s, dst0, dst1, dscale):
                ps = qk_ps.tile([PDIM, 2, CW], f32, tag="k2", name="bmini")
                for co in range(2):
                    for t in range(2):
                        nc.tensor.matmul(
                            out=ps[:, co, 0:1],
                            lhsT=w_s[:, t, bass.ts(co, PDIM)],
                            rhs=r4[:, t, :],
                            start=(t == 0),
                            stop=(t == 1),
                        )
                nc.scalar.activation(out=dst0, in_=ps[:, 0, 0:1], func=Ident, bias=0.0, scale=dscale)
                nc.scalar.activation(out=dst1, in_=ps[:, 1, 0:1], func=Ident, bias=0.0, scale=dscale)

            qb = [gn_sm.tile([PDIM, 1], f32, tag=f"qb{co}", name=f"qb{co}") for co in range(2)]
            kb = [gn_sm.tile([PDIM, 1], f32, tag=f"kb{co}", name=f"kb{co}") for co in range(2)]
            vbb = [gn_sm.tile([PDIM, 1], bf16, tag=f"vb{co}", name=f"vb{co}") for co in range(2)]
            bias_minis(wq_s, qb[0], qb[1], 1.0 / 16384.0)
            bias_minis(wk_s, kb[0], kb[1], 1.0 / 16384.0)
            bias_minis(wv_s, vbb[0], vbb[1], 1.0 / 16384.0)
            # v-bias propagates through softmax normalization unchanged, so it
            # folds into the residual constant: B2' = B2 + wp @ vb
            vbh_odd = [gn_sm.tile([D, 1], bf16, tag=f"vbh{i}", name=f"vbh{i}") for i in range(2)]
            for i in range(2):
                nc.vector.tensor_copy(vbh_odd[i], vbb[i][D : 2 * D, :])
            wpvb = qk_ps.tile([PDIM, 2, CW], f32, tag="k2", name="wpvb")
            for co in range(2):
                for h in range(NH):
                    nc.tensor.matmul(
                        out=wpvb[:, co, 0:1],
                        lhsT=wp_sb[h][:, bass.ts(co, PDIM)],
                        rhs=vbb[h // 2][0:D, :] if h % 2 == 0 else vbh_odd[h // 2],
                        start=(h == 0),
                        stop=(h == NH - 1),
                    )
            resid_sb = []
            for t in range(2):
                B2f = gn_sm.tile([PDIM, 1], f32, tag=f"B2f{t}")
                nc.vector.tensor_scalar(B2f, wpvb[:, t, 0:1], 1.0, ABs[t][2], MUL, ADD)
                rs_t = xnpool.tile([PDIM, OWN], f32, tag=f"res{t}")
                nc.gpsimd.tensor_scalar(rs_t, x_sb[t][:, 0:OWN], ABs[t][0], B2f, MUL, ADD)
                resid_sb.append(rs_t)

            # pre-load the gpsimd partition_broadcast ucode lib while the PE/
            # engines are still in the prologue; the main loop's only gpsimd
            # compute is partition_broadcast, so the lib stays resident.
            warmbc = gn_sm.tile([D, G], f32, tag="warmbc")
            nc.gpsimd.partition_broadcast(warmbc, eps_t[0:1, :].broadcast_to([1, G]))

            # ---------------- k, q, v production ----------------
            # k/q: [256 out-ch = 4 heads x 64, cols]; heads 0,1 in out-half 0.
            # Each PSUM tile holds TWO column chunks -> one big drain each.
            drain_engs = [nc.scalar, nc.vector, nc.vector]
            dei = 0

            def drain(dst, src, bias=None):
                # qkv ran on 256x-scaled fp8 operands: descale + GN bias
                nonlocal dei
                eng = drain_engs[dei % 3]
                dei += 1
                if eng is nc.scalar:
                    eng.activation(
                        out=dst, in_=src, func=Ident,
                        bias=0.0 if bias is None else bias, scale=1.0 / 256.0,
                    )
                else:
                    eng.tensor_scalar(
                        dst, src, 1.0 / 256.0, 0.0 if bias is None else bias, MUL, ADD
                    )

            ku = [qkpool.tile([PDIM, HW], bf16, tag=f"ku{co}", name=f"ku{co}") for co in range(2)]
            qu = [qkpool.tile([PDIM, OWN], bf16, tag=f"qu{co}", name=f"qu{co}") for co in range(2)]
            kx = [qkpool.tile([PDIM, HW], bf16, tag=f"kx{cp}", name=f"kx{cp}") for cp in range(2)]
            qx = [qkpool.tile([PDIM, OWN], bf16, tag=f"qx{cp}", name=f"qx{cp}") for cp in range(2)]
            v4 = qkpool.tile([PDIM, NJT, NH, VP], fp8e4, tag="v4")
            nc.vector.memset(v4[:, :, :, D : D + 1], 1.0)
            xdi = [0]

            def emit_k(co, c2, pool, tag):
                # one k2 tile = two column chunks; drain + the swapped-half
                # companion DMAs for those chunks (heads need k in BOTH halves)
                ps = pool.tile([PDIM, 2, CW], f32, tag=tag, name=f"k{co}_{c2}")
                for s in range(2):
                    nc.tensor.matmul(
                        out=ps[:, s, :],
                        lhsT=wk_s[:, :, bass.ts(co, PDIM)],
                        rhs=x4[:, :, bass.ts(2 * c2 + s, CW)],
                        start=True,
                        stop=True,
                        perf_mode=DR,
                    )
                drain(ku[co][:, bass.ts(c2, 2 * CW)], ps.rearrange("p s w -> p (s w)"), kb[co])
                for c in (2 * c2, 2 * c2 + 1):
                    eng = [nc.sync, nc.gpsimd][xdi[0] % 2]
                    xdi[0] += 1
                    eng.dma_start(out=kx[co][0:D, bass.ts(c, CW)], in_=ku[co][D : 2 * D, bass.ts(c, CW)])
                    eng.dma_start(out=kx[co][D : 2 * D, bass.ts(c, CW)], in_=ku[co][0:D, bass.ts(c, CW)])

            def emit_q(co, pool, tag):
                ps = pool.tile([PDIM, 2, CW], f32, tag=tag, name=f"q{co}")
                for s in range(NOC):
                    nc.tensor.matmul(
                        out=ps[:, s, :],
                        lhsT=wq_s[:, :, bass.ts(co, PDIM)],
                        rhs=x4[:, :, bass.ts(s, CW)],
                        start=True,
                        stop=True,
                        perf_mode=DR,
                    )
                drain(qu[co], ps.rearrange("p s w -> p (s w)"), qb[co])
                nc.sync.dma_start(out=qx[co][0:D, :], in_=qu[co][D : 2 * D, :])
                nc.gpsimd.dma_start(out=qx[co][D : 2 * D, :], in_=qu[co][0:D, :])

            def emit_v(p, pool, tag, w=C, bufs=None):
                # v for key-tile pair p, all 4 heads, strided into v4 slots
                kw = {"bufs": bufs} if bufs else {}
                ps = pool.tile([PDIM, 2, w], f32, tag=tag, name=f"v{p}", **kw)
                for s in range(2):
                    for t in range(2):
                        nc.tensor.matmul(
                            out=ps[:, s, 0:C],
                            lhsT=x4[:, t, bass.ts(2 * p + s, PDIM)],
                            rhs=wv_s[:, t, :],
                            start=(t == 0),
                            stop=(t == 1),
                        )
                drain(
                    v4[:, 2 * p : 2 * p + 2, :, 0:D],
                    ps[:, :, 0:C].rearrange("p s (h d) -> p s h d", h=NH),
                )

            # chunk-woven production (x4 is ready from the load window)
            for c2 in range(NIC // 2):
                if c2 == 0:
                    emit_q(0, qk_ps, "k2")
                emit_k(0, c2, qk_ps, "k2")
                for p in range(4 * c2, 4 * c2 + 4):
                    emit_v(p, qk_ps, "v2", bufs=2)
            for c2 in range(NIC // 2):
                emit_k(1, c2, qk_ps, "k2")
            emit_q(1, qk_ps, "k2")

            def k_src(h, s):
                # head h's k at partition half s
                return (ku if (h % 2) == s else kx)[h // 2]

            def q_src(h, s):
                return (qu if (h % 2) == s else qx)[h // 2]

            # ---------------- main attention loop ----------------
            ps_stack.close()  # release GN/QKV PSUM banks
            ps_stack2 = ExitStack()
            sc_ps = ps_stack2.enter_context(tc.tile_pool(name="sc_ps", bufs=3, space="PSUM"))
            pv_ps_pool = ps_stack2.enter_context(tc.tile_pool(name="pv_ps", bufs=2, space="PSUM"))

            # Per (i-chunk, head) "vchunk": 16 score-pair/exp/PV-DR steps,
            # pipelined LA pairs deep.  Each vchunk's normalization chain
            # (recip -> broadcast -> onorm) is DEFERRED into the next vchunk's
            # pair loop; the projection (4-head PSUM accumulation in a
            # score-pool slot + fused residual) emits once its chunk's 4
            # onorms exist.
            onorms_by_cc = [[] for _ in range(NOC)]

            def emit_proj(cc):
                cslice = bass.ts(cc, CW)
                for co in range(2):
                    pj = sc_ps.tile([PDIM, 2, CW], f32, tag="sc", name=f"pj{co}")
                    for h in range(NH):
                        nc.tensor.matmul(
                            out=pj[:, 0, :],
                            lhsT=wp_sb[h][:, bass.ts(co, PDIM)],
                            rhs=onorms_by_cc[cc][h],
                            start=(h == 0),
                            stop=(h == NH - 1),
                        )
                    yf = ypool.tile([PDIM, CW], f32, tag="yf", name="yf")
                    nc.vector.tensor_add(yf, pj[:, 0, :], resid_sb[co][:, cslice])
                    nc.sync.dma_start(out=y[bass.ts(co, PDIM), cslice], in_=yf)

            def make_chain(cc, pv):
                state = {}

                def stage1():
                    den = mlsm.tile([1, CW], f32, tag="den", name="den")
                    nc.scalar.copy(den, pv[D : D + 1, :])
                    rden = mlsm.tile([1, CW], f32, tag="rden", name="rden")
                    nc.vector.reciprocal_approx_fast(out=rden, in_=den)
                    rdb = mlsm.tile([D, CW], f32, tag="rdb", name="rdb", bufs=2)
                    nc.gpsimd.partition_broadcast(rdb, rden[:, :])
                    state["rdb"] = rdb

                def stage2():
                    onorm = mlsm.tile([D, CW], bf16, tag="onorm", bufs=5, name="onorm")
                    nc.vector.tensor_mul(onorm, state["rdb"], pv[0:D, :])
                    onorms_by_cc[cc].append(onorm)
                    if len(onorms_by_cc[cc]) == NH:
                        emit_proj(cc)
                return stage1, stage2

            # One flat pair-stream over (chunk, head): the pend queue carries
            # ACROSS vchunk boundaries, so the final PV of one head interleaves
            # with the next head's first score/exp pairs and the exp engines
            # never drain at a boundary.  post_q holds the deferred norm-chain
            # stages, drained one per pair-step so they fill pipeline slack.
            pend = []
            post_q = []
            pv_cur = None
            stream = [(cc, h, p) for cc in range(NOC) for h in range(NH) for p in range(NP)]
            for idx, (cc, h, p) in enumerate(stream + [(None, None, q) for q in range(LA)]):
                tail = cc is None
                if not tail:
                    if p == 0:
                        pv_cur = (pv_ps_pool.tile([D + 1, CW], f32, tag="pv", name="pv"), cc, h)
                    cslice = bass.ts(cc, CW)
                    # the pair's two K=64 score matmuls are row-packed into
                    # disjoint PE row-groups (base_partition 0/64) and run
                    # concurrently in one PE pass, writing the two banks of
                    # ONE PSUM tile; a single pair-wide exp drains both.
                    sc = sc_ps.tile([PDIM, 2, CW], f32, tag="sc", name="sc")
                    for s in range(2):
                        jt = 2 * p + s
                        nc.tensor.matmul(
                            out=sc[:, s, :],
                            lhsT=k_src(h, s)[s * D : (s + 1) * D, bass.ts(jt, PDIM)],
                            rhs=q_src(h, s)[s * D : (s + 1) * D, cslice],
                            start=True,
                            stop=True,
                        )
                    es = espool.tile([PDIM, 2, CW], fp8e5, tag="es")
                    if EXP_PATTERN[p % len(EXP_PATTERN)] == "S":
                        # q pre-scaled by d^-0.5*log2(e) host-side: 2^t = exp(ln2*t)
                        nc.scalar.activation(out=es, in_=sc, func=Exp, scale=LN2)
                    else:
                        # 2^t as fp8e5 bits: int8(4t + 60.5); t in [-8.4, 8.4] always
                        # maps to [27, 94] -- never negative/NaN codes.  (The e4m3
                        # variant is UNSAFE: int8 in [-128,-1] hits fp8e4 NaN codes.)
                        nc.vector.tensor_scalar(es.bitcast(i8), sc, 4.0, 60.5, MUL, ADD)
                    pend.append((pv_cur, p, es))
                while len(pend) > (0 if tail and p == LA - 1 else LA) or (tail and len(pend) > LA - 1 - p):
                    (pvt, pcc, ph), p0, es0 = pend.pop(0)
                    nc.tensor.matmul(
                        out=pvt,
                        lhsT=v4[:, 2 * p0 : 2 * p0 + 2, ph, 0 : D + 1],
                        rhs=es0,
                        start=(p0 == 0),
                        stop=(p0 == NP - 1),
                        perf_mode=DR,
                    )
                    if p0 == NP - 1:
                        st1, st2 = make_chain(pcc, pvt)
                        post_q.append(st1)
                        post_q.append(st2)
                if post_q:
                    post_q.pop(0)()
            while post_q:
                post_q.pop(0)()

            ps_stack2.close()

    nc.compile()
    return nc


def make_in_maps(x, gn_gamma, gn_beta, w_qkv, w_proj, b_proj, HW):
    """Per-core input dicts. Core c = (b = c//4, quarter s = c%4).
    x columns are rotated so the core's own quarter comes first."""
    import ml_dtypes

    bf16 = ml_dtypes.bfloat16
    OWN = HW // 4
    log2e = np.log2(np.e)
    x2 = np.ascontiguousarray(x.reshape(B, C, HW).astype(np.float32))
    w_qkv = np.asarray(w_qkv, dtype=np.float32)
    w_proj = np.asarray(w_proj, dtype=np.float32)
    indf = np.zeros((2, PDIM, G), dtype=np.float32)
    indb = np.zeros((2, G, PDIM), dtype=np.float32)
    gsz = C // G  # 32 channels per group
    for t in range(2):
        for p in range(PDIM):
            g = (t * PDIM + p) // gsz
            indf[t, p, g] = 1.0 / gsz
            indb[t, g, p] = 1.0
    fp8 = ml_dtypes.float8_e4m3

    def w4(wslice, scale):
        # [C_in, C_out] -> [128, 2, C_out] fp8, x16 (qkv runs on 16x operands)
        wT = wslice.T * scale
        return np.ascontiguousarray(wT.reshape(2, PDIM, C).transpose(1, 0, 2)).astype(fp8)

    wq4 = w4(w_qkv[0:C, :], 64.0 * (D ** -0.5 * log2e))
    wk4 = w4(w_qkv[C : 2 * C, :], 64.0)
    wv4 = w4(w_qkv[2 * C : 3 * C, :], 64.0)
    wpT = np.ascontiguousarray(w_proj.T).astype(bf16)
    in_maps = []
    for c in range(NCORES):
        b, s = c // 4, c % 4
        xrot = np.roll(x2[b], -s * OWN, axis=1)
        in_maps.append(
            {
                "xb": np.ascontiguousarray(xrot),
                "wq4": wq4,
                "wk4": wk4,
                "wv4": wv4,
                "wpT": wpT,
                "gamma": np.asarray(gn_gamma, dtype=np.float32),
                "beta": np.asarray(gn_beta, dtype=np.float32),
                "bproj": np.asarray(b_proj, dtype=np.float32),
                "indf": indf,
                "indb": indb,
            }
        )
    return in_maps


def assemble_output(results, HW, Himg, Wimg):
    OWN = HW // 4
    y = np.empty((B, C, HW), dtype=np.float32)
    for c in range(NCORES):
        b, s = c // 4, c % 4
        y[b][:, s * OWN : (s + 1) * OWN] = results[c]["y"]
    return y.reshape(B, C, Himg, Wimg)


_NC_CACHE = {}


def kernel(x, gn_gamma, gn_beta, w_qkv, w_proj, b_proj):
    from concourse.bass_utils import run_bass_kernel_spmd

    Himg, Wimg = x.shape[2], x.shape[3]
    HW = Himg * Wimg
    if HW not in _NC_CACHE:
        _NC_CACHE[HW] = build_nc(HW)
    nc = _NC_CACHE[HW]
    in_maps = make_in_maps(x, gn_gamma, gn_beta, w_qkv, w_proj, b_proj, HW)
    res = run_bass_kernel_spmd(nc, in_maps, list(range(NCORES)))
    return assemble_output(res.results, HW, Himg, Wimg)


# revision 26
# speedup vs baseline: 1.3060x; 1.0025x over previous
"""Trainium2 Bass kernel for an AttentionBlock (GroupNorm + single-layer MHA + proj residual).

Reference computation (per batch b):
    xn = GroupNorm(x[b])                        # 8 groups over C=256, HW spatial
    qkv = w_qkv @ xn                            # per-pixel 1x1 conv
    per head h (4 heads, d=64):
        scores = q_h^T k_h * d^-0.5             # [HW, HW]
        attn = softmax(scores, axis=keys)
        out_h = v_h @ attn^T                    # [d, HW]
    y = xn + w_proj @ concat(out_h) + b_proj

Sharding: 8 cores = (batch b in {0,1}) x (query quarter s in {0..3}).  Each
core runs GroupNorm, computes k/v for ALL spatial positions and q for its
own quarter, then runs all 4 heads' attention for its own 1024 query
columns.  The head sum of the projection is a local PSUM accumulation, so
there is NO collective at all: each core writes its own [C, 1024] slice of
the output, with the residual fused into the PSUM drain.

Key kernel-level layout choices:
 - x columns are permuted host-side so each core's OWN quarter comes first;
   attention is permutation-invariant over keys, so k/v/score column order
   doesn't matter (no separate x_own load; residual slices x directly).
 - x is converted to fp8e4 (fixed 4x scale) DURING the HBM load; the
   GroupNorm affine folds into the fp8 qkv weights (per-partition A-scale)
   and per-out-channel drain biases (B via tiny bias matmuls); the v-bias
   passes through softmax normalization unchanged and folds into the
   residual constant via wp @ vb.
 - q/k are fp8 DoubleRow matmuls (256-deep contraction per instruction);
   v is plain fp8 matmuls (its stationary operand changes every tile, so
   DoubleRow's ldweights cost would dominate).
 - scores are computed TRANSPOSED (keys j on partitions, queries i on the
   free axis); softmax denominator comes free as a 65th "ones" column of V.
 - softmax skips max-subtraction; scores live in the log2 domain (q
   pre-scaled by d^-0.5*log2 e host-side).
 - each score PAIR (2 key tiles x 512 queries) lands in ONE 2-bank PSUM
   tile [128,2,512]; ONE pair-wide exp instruction (Scalar native EXP or
   Vector int8 bit-trick) converts it to fp8e5 `es`.  e5m2's 4 steps/octave
   means the bit-trick value range is always a safe positive int8 (an e4m3
   variant is UNSAFE: int8 values in [-128,-1] alias fp8e4 NaN codes).
 - PV runs as a single fp8 DoubleRow matmul per pair (v4 fp8e4 stationary,
   es fp8e5 moving), halving PE time vs two bf16 matmuls.
 - the norm chain's only gpsimd op is partition_broadcast and its ucode
   lib is pre-loaded in the prologue (lib swaps cost ~7us stalls).
 - projection accumulates in a score-pool PSUM slot; residual fused in
   the drain.  Discarded f32 matmuls on late x chunks pre-warm the PE
   HAM clock gate.
"""

import numpy as np

C = 256
NH = 4
D = 64
G = 8
EPS = 1e-5
B = 2
NCORES = 8
PDIM = 128  # partitions
VP = 68     # v4 per-(jt,head) stride: 4*68=272 bytes, dual-fp8 ldweights needs %16==0

PREWARM = True
# per-vchunk exp engine pattern (16 pairs): S=scalar native exp, V=vector trick
EXP_PATTERN = "SVSVSVSSVSVSVSSV"


def build_nc(HW: int):
    import concourse.bass as bass
    import concourse.mybir as mybir
    import concourse.tile as tile
    from concourse import bacc

    f32 = mybir.dt.float32
    bf16 = mybir.dt.bfloat16
    fp8e4 = mybir.dt.float8e4
    fp8e5 = mybir.dt.float8e5
    i8 = mybir.dt.int8
    DR = mybir.MatmulPerfMode.DoubleRow
    CW = min(512, HW)          # i-chunk width (matmul moving-operand max)
    NIC = HW // CW             # number of column chunks of the full image
    OWN = HW // 4              # query columns owned per core
    NOC = OWN // CW            # own-column chunks
    NJT = HW // PDIM           # number of key tiles (128 keys each)
    NP = NJT // 2              # pairs of key tiles
    LA = 6                     # pv lookahead in pairs

    nc = bacc.Bacc(
        "TRN2", target_bir_lowering=False, debug=False, num_devices=NCORES
    )

    xb = nc.declare_dram_parameter("xb", [C, HW], f32, isOutput=False)
    wq4 = nc.declare_dram_parameter("wq4", [PDIM, 2, C], fp8e4, isOutput=False)
    wk4 = nc.declare_dram_parameter("wk4", [PDIM, 2, C], fp8e4, isOutput=False)
    wv4 = nc.declare_dram_parameter("wv4", [PDIM, 2, C], fp8e4, isOutput=False)
    wpT = nc.declare_dram_parameter("wpT", [C, C], bf16, isOutput=False)
    gamma = nc.declare_dram_parameter("gamma", [C], f32, isOutput=False)
    beta = nc.declare_dram_parameter("beta", [C], f32, isOutput=False)
    bproj = nc.declare_dram_parameter("bproj", [C], f32, isOutput=False)
    indf = nc.declare_dram_parameter("indf", [2, PDIM, G], f32, isOutput=False)
    indb = nc.declare_dram_parameter("indb", [2, G, PDIM], f32, isOutput=False)
    y = nc.declare_dram_parameter("y", [C, OWN], f32, isOutput=True)

    Exp = mybir.ActivationFunctionType.Exp
    Sqrt = mybir.ActivationFunctionType.Sqrt
    Ident = mybir.ActivationFunctionType.Identity
    MUL = mybir.AluOpType.mult
    ADD = mybir.AluOpType.add

    BNW = min(512, HW)         # bn_stats max free dim
    NBN = HW // BNW
    LN2 = 0.6931471805599453

    with tile.TileContext(nc) as tc:
        with (
            tc.tile_pool(name="consts", bufs=1) as consts,
            tc.tile_pool(name="xpool", bufs=1) as xpool,
            tc.tile_pool(name="xnpool", bufs=1) as xnpool,
            tc.tile_pool(name="gn_sm", bufs=2) as gn_sm,
            tc.tile_pool(name="qkpool", bufs=1) as qkpool,
            tc.tile_pool(name="espool", bufs=10) as espool,
            tc.tile_pool(name="mlsm", bufs=3) as mlsm,
            tc.tile_pool(name="ypool", bufs=4) as ypool,
        ):
            # ---------------- x load (biggest transfer, gates GN) ----------------
            # Interleave the two channel-halves chunk-by-chunk across the three
            # DMA-capable queues so bn_stats for BOTH halves trail the load by
            # only one chunk.
            dma_engines = [nc.sync, nc.scalar, nc.gpsimd]
            x_sb = [
                xpool.tile([PDIM, HW], f32, tag=f"x{t}", name=f"x{t}") for t in range(2)
            ]
            x4 = xnpool.tile([PDIM, 2, HW], fp8e4, tag="x4")
            di = 0
            for c in range(NIC):
                for t in range(2):
                    dma_engines[di % 3].dma_start(
                        out=x_sb[t][:, bass.ts(c, CW)],
                        in_=xb[bass.ts(t, PDIM), bass.ts(c, CW)],
                    )
                    di += 1

            # 4*x in fp8e4 (fixed scale -- no stats dependency, so it runs
            # inside the load window; the GN affine folds into the weights)
            for c in range(NIC):
                for t in range(2):
                    nc.scalar.activation(
                        out=x4[:, t, bass.ts(c, CW)],
                        in_=x_sb[t][:, bass.ts(c, CW)],
                        func=Ident, bias=0.0, scale=4.0,
                    )

            # ---------------- constants / small loads ----------------
            eps_t = consts.tile([PDIM, 1], f32)
            nc.vector.memset(eps_t, EPS)
            nln2 = consts.tile([PDIM, 1], f32, tag="nln2")
            nc.vector.memset(nln2, -2.0 * 0.6931471805599453)

            indf_sb = []
            indb_sb = []
            gm_sb = []
            bt_sb = []
            bp_sb = []
            for t in range(2):
                it_ = consts.tile([PDIM, G], f32, tag=f"indf{t}")
                nc.sync.dma_start(out=it_, in_=indf[t])
                indf_sb.append(it_)
                ib_ = consts.tile([G, PDIM], f32, tag=f"indb{t}")
                nc.sync.dma_start(out=ib_, in_=indb[t])
                indb_sb.append(ib_)
                g_ = consts.tile([PDIM, 1], f32, tag=f"gm{t}")
                nc.sync.dma_start(out=g_, in_=gamma[bass.ts(t, PDIM)].rearrange("(p o) -> p o", o=1))
                gm_sb.append(g_)
                b_ = consts.tile([PDIM, 1], f32, tag=f"bt{t}")
                nc.sync.dma_start(out=b_, in_=beta[bass.ts(t, PDIM)].rearrange("(p o) -> p o", o=1))
                bt_sb.append(b_)
                bp_ = consts.tile([PDIM, 1], f32, tag=f"bp{t}")
                nc.sync.dma_start(out=bp_, in_=bproj[bass.ts(t, PDIM)].rearrange("(p o) -> p o", o=1))
                bp_sb.append(bp_)

            # weight tiles: fp8, [p, c-half, 256 outputs] (DR k-subtile layout)
            wq_t = consts.tile([PDIM, 2, C], fp8e4, tag="wq")
            nc.sync.dma_start(out=wq_t, in_=wq4[:, :, :])
            wk_t = consts.tile([PDIM, 2, C], fp8e4, tag="wk")
            nc.sync.dma_start(out=wk_t, in_=wk4[:, :, :])
            wv_t = consts.tile([PDIM, 2, C], fp8e4, tag="wv")
            nc.sync.dma_start(out=wv_t, in_=wv4[:, :, :])
            r4 = consts.tile([PDIM, 2, 1], fp8e4, tag="r4")
            wq_s = consts.tile([PDIM, 2, C], fp8e4, tag="wqs")
            wk_s = consts.tile([PDIM, 2, C], fp8e4, tag="wks")
            wv_s = consts.tile([PDIM, 2, C], fp8e4, tag="wvs")
            wp_sb = []
            for h in range(NH):
                wt = consts.tile([D, C], bf16, tag=f"wp{h}", name=f"wp{h}")
                nc.sync.dma_start(out=wt, in_=wpT[h * D : (h + 1) * D, :])
                wp_sb.append(wt)

            from contextlib import ExitStack

            ps_stack = ExitStack()
            gn_ps = ps_stack.enter_context(tc.tile_pool(name="gn_ps", bufs=1, space="PSUM"))

            # ---------------- PE pre-warm (discarded f32 matmuls) ----------------
            # The PE HAM clock gate needs ~3.4us of sustained activity to release
            # 2.4 GHz.  Two slow f32 matmuls on late x chunks put the PE in the
            # busy state right before the GN/QKV/attention stream begins.
            if PREWARM:
                warm = gn_ps.tile([PDIM, 2, CW], f32, tag="warm")
                for w in range(4):
                    nc.tensor.matmul(
                        out=warm[:, w % 2, :],
                        lhsT=x_sb[0][:, (NIC - 4 + w) * CW : (NIC - 4 + w) * CW + PDIM],
                        rhs=x_sb[1][:, bass.ts(NIC - 4 + w, CW)],
                        start=True,
                        stop=True,
                    )

            # ---------------- GroupNorm stats ----------------
            gst_full = gn_ps.tile([PDIM, 2], f32, tag="gnps")
            gst_ps = gst_full[0:G, :]
            for t in range(2):
                stats = gn_sm.tile([PDIM, NBN, 6], f32, tag="bnst")
                for s in range(NBN):
                    nc.vector.bn_stats(out=stats[:, s, :], in_=x_sb[t][:, bass.ts(s, BNW)])
                mv = gn_sm.tile([PDIM, 2], f32, tag="mv")
                nc.vector.bn_aggr(out=mv, in_=stats)
                st2 = gn_sm.tile([PDIM, 2], f32, tag="st2")
                nc.vector.tensor_copy(st2[:, 0:1], mv[:, 0:1])
                sq = gn_sm.tile([PDIM, 1], f32, tag="sq")
                nc.vector.tensor_mul(sq, mv[:, 0:1], mv[:, 0:1])
                nc.vector.tensor_add(st2[:, 1:2], mv[:, 1:2], sq)
                nc.tensor.matmul(
                    out=gst_ps, lhsT=indf_sb[t], rhs=st2, start=(t == 0), stop=(t == 1)
                )

            gst = gn_sm.tile([G, 2], f32, tag="gst_sb")
            nc.vector.tensor_copy(gst, gst_ps)
            mu2 = gn_sm.tile([G, 1], f32, tag="mu2")
            nc.vector.tensor_mul(mu2, gst[:, 0:1], gst[:, 0:1])
            var = gn_sm.tile([G, 1], f32, tag="var")
            nc.vector.tensor_sub(var, gst[:, 1:2], mu2)
            sd = gn_sm.tile([G, 1], f32, tag="sd")
            nc.scalar.activation(out=sd, in_=var, func=Sqrt, bias=eps_t[0:G, :], scale=1.0)
            rstd = gn_sm.tile([G, 1], f32, tag="rstd")
            nc.vector.reciprocal(out=rstd, in_=sd)
            gmr = gn_sm.tile([G, 2], f32, tag="gmr")
            nc.vector.tensor_copy(gmr[:, 0:1], gst[:, 0:1])
            nc.vector.tensor_copy(gmr[:, 1:2], rstd)

            # per-channel affine params.  The normalized-x pass is GONE: the
            # A-scale folds into the fp8 weights (per-partition multiply) and
            # the B-offset folds into per-out-channel drain biases.
            ABs = []
            for t in range(2):
                gb_ps = gn_ps.tile([PDIM, 2], f32, tag="gnps")
                nc.tensor.matmul(out=gb_ps, lhsT=indb_sb[t], rhs=gmr, start=True, stop=True)
                gb = gn_sm.tile([PDIM, 2], f32, tag="gb_sb")
                nc.vector.tensor_copy(gb, gb_ps)
                A_t = gn_sm.tile([PDIM, 1], f32, tag=f"A{t}")
                nc.vector.tensor_mul(A_t, gb[:, 1:2], gm_sb[t])
                tmp = gn_sm.tile([PDIM, 1], f32, tag="tmp")
                nc.vector.tensor_mul(tmp, gb[:, 0:1], A_t)
                B_t = gn_sm.tile([PDIM, 1], f32, tag=f"B{t}")
                nc.vector.tensor_sub(B_t, bt_sb[t], tmp)
                B2_t = gn_sm.tile([PDIM, 1], f32, tag=f"B2{t}")
                nc.vector.tensor_add(B2_t, B_t, bp_sb[t])
                # r' = 256*B/A, fp8, feeds the bias mini-matmuls
                rA = gn_sm.tile([PDIM, 1], f32, tag=f"rA{t}")
                nc.vector.reciprocal(out=rA, in_=A_t)
                rB = gn_sm.tile([PDIM, 1], f32, tag=f"rB{t}")
                nc.vector.tensor_mul(rB, B_t, rA)
                nc.vector.tensor_scalar(r4[:, t, :], rB, 256.0, 0.0, MUL, ADD)
                ABs.append((A_t, B_t, B2_t))

            # scale weights by A (per input channel = per partition, per half)
            for wsrc, wdst in ((wq_t, wq_s), (wk_t, wk_s), (wv_t, wv_s)):
                for t in range(2):
                    nc.scalar.activation(
                        out=wdst[:, t, :], in_=wsrc[:, t, :],
                        func=Ident, bias=0.0, scale=ABs[t][0],
                    )

            ps_stack.close()  # release GN PSUM banks
            ps_stack = ExitStack()
            qk_ps = ps_stack.enter_context(tc.tile_pool(name="qk_ps", bufs=3, space="PSUM"))

            # bias mini-matmuls: bias_o = sum_c w_oc * B_c, computed from the
            # scaled weights against r' (psum = 16384 * bias -> tiny descale)
            def bias_minis(w_s, dst0, dst1, dscale):
                ps = qk_ps.tile([PDIM, 2, CW], f32, tag="k2", name="bmini")
                for co in range(2):
                    for t in range(2):
                        nc.tensor.matmul(
                            out=ps[:, co, 0:1],
                            lhsT=w_s[:, t, bass.ts(co, PDIM)],
                            rhs=r4[:, t, :],
                            start=(t == 0),
                            stop=(t == 1),
                        )
                nc.scalar.activation(out=dst0, in_=ps[:, 0, 0:1], func=Ident, bias=0.0, scale=dscale)
                nc.scalar.activation(out=dst1, in_=ps[:, 1, 0:1], func=Ident, bias=0.0, scale=dscale)

            qb = [gn_sm.tile([PDIM, 1], f32, tag=f"qb{co}", name=f"qb{co}") for co in range(2)]
            kb = [gn_sm.tile([PDIM, 1], f32, tag=f"kb{co}", name=f"kb{co}") for co in range(2)]
            vbb = [gn_sm.tile([PDIM, 1], bf16, tag=f"vb{co}", name=f"vb{co}") for co in range(2)]
            bias_minis(wq_s, qb[0], qb[1], 1.0 / 16384.0)
            bias_minis(wk_s, kb[0], kb[1], 1.0 / 16384.0)
            bias_minis(wv_s, vbb[0], vbb[1], 1.0 / 16384.0)
            # v-bias propagates through softmax normalization unchanged, so it
            # folds into the residual constant: B2' = B2 + wp @ vb
            vbh_odd = [gn_sm.tile([D, 1], bf16, tag=f"vbh{i}", name=f"vbh{i}") for i in range(2)]
            for i in range(2):
                nc.vector.tensor_copy(vbh_odd[i], vbb[i][D : 2 * D, :])
            wpvb = qk_ps.tile([PDIM, 2, CW], f32, tag="k2", name="wpvb")
            for co in range(2):
                for h in range(NH):
                    nc.tensor.matmul(
                        out=wpvb[:, co, 0:1],
                        lhsT=wp_sb[h][:, bass.ts(co, PDIM)],
                        rhs=vbb[h // 2][0:D, :] if h % 2 == 0 else vbh_odd[h // 2],
                        start=(h == 0),
                        stop=(h == NH - 1),
                    )
            resid_sb = []
            for t in range(2):
                B2f = gn_sm.tile([PDIM, 1], f32, tag=f"B2f{t}")
                nc.vector.tensor_scalar(B2f, wpvb[:, t, 0:1], 1.0, ABs[t][2], MUL, ADD)
                rs_t = xnpool.tile([PDIM, OWN], f32, tag=f"res{t}")
                nc.gpsimd.tensor_scalar(rs_t, x_sb[t][:, 0:OWN], ABs[t][0], B2f, MUL, ADD)
                resid_sb.append(rs_t)

            # pre-load the gpsimd partition_broadcast ucode lib while the PE/
            # engines are still in the prologue; the main loop's only gpsimd
            # compute is partition_broadcast, so the lib stays resident.
            warmbc = gn_sm.tile([D, G], f32, tag="warmbc")
            nc.gpsimd.partition_broadcast(warmbc, eps_t[0:1, :].broadcast_to([1, G]))

            # ---------------- k, q, v production ----------------
            # k/q: [256 out-ch = 4 heads x 64, cols]; heads 0,1 in out-half 0.
            # Each PSUM tile holds TWO column chunks -> one big drain each.
            drain_engs = [nc.scalar, nc.vector, nc.vector]
            dei = 0

            def drain(dst, src, bias=None):
                # qkv ran on 256x-scaled fp8 operands: descale + GN bias
                nonlocal dei
                eng = drain_engs[dei % 3]
                dei += 1
                if eng is nc.scalar:
                    eng.activation(
                        out=dst, in_=src, func=Ident,
                        bias=0.0 if bias is None else bias, scale=1.0 / 256.0,
                    )
                else:
                    eng.tensor_scalar(
                        dst, src, 1.0 / 256.0, 0.0 if bias is None else bias, MUL, ADD
                    )

            ku = [qkpool.tile([PDIM, HW], bf16, tag=f"ku{co}", name=f"ku{co}") for co in range(2)]
            qu = [qkpool.tile([PDIM, OWN], bf16, tag=f"qu{co}", name=f"qu{co}") for co in range(2)]
            kx = [qkpool.tile([PDIM, HW], bf16, tag=f"kx{cp}", name=f"kx{cp}") for cp in range(2)]
            qx = [qkpool.tile([PDIM, OWN], bf16, tag=f"qx{cp}", name=f"qx{cp}") for cp in range(2)]
            v4 = qkpool.tile([PDIM, NJT, NH, VP], fp8e4, tag="v4")
            nc.vector.memset(v4[:, :, :, D : D + 1], 1.0)
            xdi = [0]

            def emit_k(co, c2, pool, tag):
                # one k2 tile = two column chunks; drain + the swapped-half
                # companion DMAs for those chunks (heads need k in BOTH halves)
                ps = pool.tile([PDIM, 2, CW], f32, tag=tag, name=f"k{co}_{c2}")
                for s in range(2):
                    nc.tensor.matmul(
                        out=ps[:, s, :],
                        lhsT=wk_s[:, :, bass.ts(co, PDIM)],
                        rhs=x4[:, :, bass.ts(2 * c2 + s, CW)],
                        start=True,
                        stop=True,
                        perf_mode=DR,
                    )
                drain(ku[co][:, bass.ts(c2, 2 * CW)], ps.rearrange("p s w -> p (s w)"), kb[co])
                for c in (2 * c2, 2 * c2 + 1):
                    eng = [nc.sync, nc.gpsimd][xdi[0] % 2]
                    xdi[0] += 1
                    eng.dma_start(out=kx[co][0:D, bass.ts(c, CW)], in_=ku[co][D : 2 * D, bass.ts(c, CW)])
                    eng.dma_start(out=kx[co][D : 2 * D, bass.ts(c, CW)], in_=ku[co][0:D, bass.ts(c, CW)])

            def emit_q(co, pool, tag):
                ps = pool.tile([PDIM, 2, CW], f32, tag=tag, name=f"q{co}")
                for s in range(NOC):
                    nc.tensor.matmul(
                        out=ps[:, s, :],
                        lhsT=wq_s[:, :, bass.ts(co, PDIM)],
                        rhs=x4[:, :, bass.ts(s, CW)],
                        start=True,
                        stop=True,
                        perf_mode=DR,
                    )
                drain(qu[co], ps.rearrange("p s w -> p (s w)"), qb[co])
                nc.sync.dma_start(out=qx[co][0:D, :], in_=qu[co][D : 2 * D, :])
                nc.gpsimd.dma_start(out=qx[co][D : 2 * D, :], in_=qu[co][0:D, :])

            def emit_v(p, pool, tag, w=C, bufs=None):
                # v for key-tile pair p, all 4 heads, strided into v4 slots
                kw = {"bufs": bufs} if bufs else {}
                ps = pool.tile([PDIM, 2, w], f32, tag=tag, name=f"v{p}", **kw)
                for s in range(2):
                    for t in range(2):
                        nc.tensor.matmul(
                            out=ps[:, s, 0:C],
                            lhsT=x4[:, t, bass.ts(2 * p + s, PDIM)],
                            rhs=wv_s[:, t, :],
                            start=(t == 0),
                            stop=(t == 1),
                        )
                drain(
                    v4[:, 2 * p : 2 * p + 2, :, 0:D],
                    ps[:, :, 0:C].rearrange("p s (h d) -> p s h d", h=NH),
                )

            # chunk-woven production (x4 is ready from the load window)
            for c2 in range(NIC // 2):
                if c2 == 0:
                    emit_q(0, qk_ps, "k2")
                emit_k(0, c2, qk_ps, "k2")
                for p in range(4 * c2, 4 * c2 + 4):
                    emit_v(p, qk_ps, "v2", bufs=2)
            for c2 in range(NIC // 2):
                emit_k(1, c2, qk_ps, "k2")
            emit_q(1, qk_ps, "k2")

            def k_src(h, s):
                # head h's k at partition half s
                return (ku if (h % 2) == s else kx)[h // 2]

            def q_src(h, s):
                return (qu if (h % 2) == s else qx)[h // 2]

            # ---------------- main attention loop ----------------
            ps_stack.close()  # release GN/QKV PSUM banks
            ps_stack2 = ExitStack()
            sc_ps = ps_stack2.enter_context(tc.tile_pool(name="sc_ps", bufs=3, space="PSUM"))
            pv_ps_pool = ps_stack2.enter_context(tc.tile_pool(name="pv_ps", bufs=2, space="PSUM"))

            # Per (i-chunk, head) "vchunk": 16 score-pair/exp/PV-DR steps,
            # pipelined LA pairs deep.  Each vchunk's normalization chain
            # (recip -> broadcast -> onorm) is DEFERRED into the next vchunk's
            # pair loop; the projection (4-head PSUM accumulation in a
            # score-pool slot + fused residual) emits once its chunk's 4
            # onorms exist.
            onorms_by_cc = [[] for _ in range(NOC)]

            def emit_proj(cc):
                cslice = bass.ts(cc, CW)
                for co in range(2):
                    pj = sc_ps.tile([PDIM, 2, CW], f32, tag="sc", name=f"pj{co}")
                    for h in range(NH):
                        nc.tensor.matmul(
                            out=pj[:, 0, :],
                            lhsT=wp_sb[h][:, bass.ts(co, PDIM)],
                            rhs=onorms_by_cc[cc][h],
                            start=(h == 0),
                            stop=(h == NH - 1),
                        )
                    yf = ypool.tile([PDIM, CW], f32, tag="yf", name="yf")
                    nc.vector.tensor_add(yf, pj[:, 0, :], resid_sb[co][:, cslice])
                    nc.sync.dma_start(out=y[bass.ts(co, PDIM), cslice], in_=yf)

            def make_chain(cc, pv):
                state = {}

                def stage1():
                    den = mlsm.tile([1, CW], f32, tag="den", name="den")
                    nc.scalar.copy(den, pv[D : D + 1, :])
                    rden = mlsm.tile([1, CW], f32, tag="rden", name="rden")
                    nc.vector.reciprocal_approx_fast(out=rden, in_=den)
                    rdb = mlsm.tile([D, CW], f32, tag="rdb", name="rdb", bufs=2)
                    nc.gpsimd.partition_broadcast(rdb, rden[:, :])
                    state["rdb"] = rdb

                def stage2():
                    onorm = mlsm.tile([D, CW], bf16, tag="onorm", bufs=5, name="onorm")
                    nc.vector.tensor_mul(onorm, state["rdb"], pv[0:D, :])
                    onorms_by_cc[cc].append(onorm)
                    if len(onorms_by_cc[cc]) == NH:
                        emit_proj(cc)
                return stage1, stage2

            # One flat pair-stream over (chunk, head): the pend queue carries
            # ACROSS vchunk boundaries, so the final PV of one head interleaves
            # with the next head's first score/exp pairs and the exp engines
            # never drain at a boundary.  post_q holds the deferred norm-chain
            # stages, drained one per pair-step so they fill pipeline slack.
            pend = []
            post_q = []
            pv_cur = None
            stream = [(cc, h, p) for cc in range(NOC) for h in range(NH) for p in range(NP)]
            for idx, (cc, h, p) in enumerate(stream + [(None, None, q) for q in range(LA)]):
                tail = cc is None
                if not tail:
                    if p == 0:
                        pv_cur = (pv_ps_pool.tile([D + 1, CW], f32, tag="pv", name="pv"), cc, h)
                    cslice = bass.ts(cc, CW)
                    # the pair's two K=64 score matmuls are row-packed into
                    # disjoint PE row-groups (base_partition 0/64) and run
                    # concurrently in one PE pass, writing the two banks of
                    # ONE PSUM tile; a single pair-wide exp drains both.
                    sc = sc_ps.tile([PDIM, 2, CW], f32, tag="sc", name="sc")
                    for s in range(2):
                        jt = 2 * p + s
                        nc.tensor.matmul(
                            out=sc[:, s, :],
                            lhsT=k_src(h, s)[s * D : (s + 1) * D, bass.ts(jt, PDIM)],
                            rhs=q_src(h, s)[s * D : (s + 1) * D, cslice],
                            start=True,
                            stop=True,
                        )
                    es = espool.tile([PDIM, 2, CW], fp8e5, tag="es")
                    if EXP_PATTERN[p % len(EXP_PATTERN)] == "S":
                        # q pre-scaled by d^-0.5*log2(e) host-side: 2^t = exp(ln2*t)
                        nc.scalar.activation(out=es, in_=sc, func=Exp, scale=LN2)
                    else:
                        # 2^t as fp8e5 bits: int8(4t + 60.5); t in [-8.4, 8.4] always
                        # maps to [27, 94] -- never negative/NaN codes.  (The e4m3
                        # variant is UNSAFE: int8 in [-128,-1] hits fp8e4 NaN codes.)
                        nc.vector.tensor_scalar(es.bitcast(i8), sc, 4.0, 60.5, MUL, ADD)
                    pend.append((pv_cur, p, es))
                while len(pend) > (0 if tail and p == LA - 1 else LA) or (tail and len(pend) > LA - 1 - p):
                    (pvt, pcc, ph), p0, es0 = pend.pop(0)
                    nc.tensor.matmul(
                        out=pvt,
                        lhsT=v4[:, 2 * p0 : 2 * p0 + 2, ph, 0 : D + 1],
                        rhs=es0,
                        start=(p0 == 0),
                        stop=(p0 == NP - 1),
                        perf_mode=DR,
                    )
                    if p0 == NP - 1:
                        st1, st2 = make_chain(pcc, pvt)
                        post_q.append(st1)
                        post_q.append(st2)
                if post_q:
                    post_q.pop(0)()
            while post_q:
                post_q.pop(0)()

            ps_stack2.close()

    nc.compile()
    return nc


def make_in_maps(x, gn_gamma, gn_beta, w_qkv, w_proj, b_proj, HW):
    """Per-core input dicts. Core c = (b = c//4, quarter s = c%4).
    x columns are rotated so the core's own quarter comes first."""
    import ml_dtypes

    bf16 = ml_dtypes.bfloat16
    OWN = HW // 4
    log2e = np.log2(np.e)
    x2 = np.ascontiguousarray(x.reshape(B, C, HW).astype(np.float32))
    w_qkv = np.asarray(w_qkv, dtype=np.float32)
    w_proj = np.asarray(w_proj, dtype=np.float32)
    indf = np.zeros((2, PDIM, G), dtype=np.float32)
    indb = np.zeros((2, G, PDIM), dtype=np.float32)
    gsz = C // G  # 32 channels per group
    for t in range(2):
        for p in range(PDIM):
            g = (t * PDIM + p) // gsz
            indf[t, p, g] = 1.0 / gsz
            indb[t, g, p] = 1.0
    fp8 = ml_dtypes.float8_e4m3

    def w4(wslice, scale):
        # [C_in, C_out] -> [128, 2, C_out] fp8, x16 (qkv runs on 16x operands)
        wT = wslice.T * scale
        return np.ascontiguousarray(wT.reshape(2, PDIM, C).transpose(1, 0, 2)).astype(fp8)

    wq4 = w4(w_qkv[0:C, :], 64.0 * (D ** -0.5 * log2e))
    wk4 = w4(w_qkv[C : 2 * C, :], 64.0)
    wv4 = w4(w_qkv[2 * C : 3 * C, :], 64.0)
    wpT = np.ascontiguousarray(w_proj.T).astype(bf16)
    in_maps = []
    for c in range(NCORES):
        b, s = c // 4, c % 4
        xrot = np.roll(x2[b], -s * OWN, axis=1)
        in_maps.append(
            {
                "xb": np.ascontiguousarray(xrot),
                "wq4": wq4,
                "wk4": wk4,
                "wv4": wv4,
                "wpT": wpT,
                "gamma": np.asarray(gn_gamma, dtype=np.float32),
                "beta": np.asarray(gn_beta, dtype=np.float32),
                "bproj": np.asarray(b_proj, dtype=np.float32),
                "indf": indf,
                "indb": indb,
            }
        )
    return in_maps


def assemble_output(results, HW, Himg, Wimg):
    OWN = HW // 4
    y = np.empty((B, C, HW), dtype=np.float32)
    for c in range(NCORES):
        b, s = c // 4, c % 4
        y[b][:, s * OWN : (s + 1) * OWN] = results[c]["y"]
    return y.reshape(B, C, Himg, Wimg)


_NC_CACHE = {}


def kernel(x, gn_gamma, gn_beta, w_qkv, w_proj, b_proj):
    from concourse.bass_utils import run_bass_kernel_spmd

    Himg, Wimg = x.shape[2], x.shape[3]
    HW = Himg * Wimg
    if HW not in _NC_CACHE:
        _NC_CACHE[HW] = build_nc(HW)
    nc = _NC_CACHE[HW]
    in_maps = make_in_maps(x, gn_gamma, gn_beta, w_qkv, w_proj, b_proj, HW)
    res = run_bass_kernel_spmd(nc, in_maps, list(range(NCORES)))
    return assemble_output(res.results, HW, Himg, Wimg)
